# revision 1
# baseline (speedup 1.0000x reference)
"""TRN2 Bass kernel for nn_Attention (RMSNorm + QKV + softmax attention + out-proj).

Sharding: 8 cores = 2 batches x 4 head-pairs. Core c handles batch c//4 and
heads (2*(c%4), 2*(c%4)+1). Each core computes its partial out-projection
(contracting only its 128 rows of dim_inner); host sums the 4 partials per batch.

Per-core pipeline (all matmuls f32r, 1 cycle/row at free>=256):
  A) stream tokens [128,512] blocks: RMSNorm stats on DVE, rstd scale, PE
     transpose -> x^T [512, 4096] (rotating [128,4,512] per 512-token chunk)
  B) QKV^T = w^T @ x^T (PE), V^T transposed back to V-natural with a ones
     column appended per head (gives softmax denominator for free)
  C) flash-style attention per (head, query-block of 512):
     S^T tiles [128j, 512i] on PE -> exp on ACT (no max subtraction; scores
     bounded ~45, exp fits fp32) -> o^T accum [65, 512] on PE (row 64 = l)
     -> linv = 1/l (DVE) -> broadcast via ones matmul -> O^T = o^T * linv_b
  D) out-proj: out[i,:] += O^T_h.T @ w_out_h per head, DMA partial out.

Engine discipline: walrus here allows only ONE semaphore wait per instruction,
so producers are assigned to engines such that every instruction has at most
one un-observed cross-engine dependency (see joins / scratch-copy tricks).
"""
import sys
sys.path.insert(0, "/opt/trn_rl_repo")
import numpy as np

B, N, D = 2, 4096, 512
H, DH = 8, 64
DI = H * DH
NCORES = 8
EPS = 1.1920929e-07  # float32 eps (torch nn.RMSNorm default)

_prog_cache = {}


def _patch_drain(tile_mod, mybir):
    """Split the multi-wait tail drain into a chain of single-wait drains
    (this walrus build rejects >1 sync wait per instruction)."""
    if getattr(tile_mod.TileContext, "_drain_patched", False):
        return

    def _patched(self, tick_clock, wait_clock):
        from concourse.vector_clock import ScopedClock
        nc = self.nc
        drain_inst = nc.sync.drain()
        wait_clock.add_sem_waits(drain_inst.ins, ScopedClock({None: tick_clock.global_clock}))
        si = drain_inst.ins.sync_info
        if si is not None and si.on_wait and len(si.on_wait) > 1:
            waits = list(si.on_wait)
            drain_inst.ins.sync_info = mybir.SyncInfo(
                on_wait=waits[:1], on_update=list(si.on_update or []))
            for w in waits[1:]:
                d2 = nc.sync.drain()
                d2.ins.sync_info = mybir.SyncInfo(on_wait=[w], on_update=[])
        nc.all_engine_barrier()
        assert self.sems is not None
        popped = nc._tile_sem_poison_stack.pop()
        assert popped is self._sem_poison
        nc.clear_and_free_semaphores(list(self.sems.allocated().values()))
        nc.all_engine_barrier()

    tile_mod.TileContext._drain_and_barrier = _patched
    tile_mod.TileContext._drain_patched = True


def build_program():
    import concourse.bass as bass
    import concourse.tile as tile
    from concourse import mybir
    from concourse.masks import make_identity

    _patch_drain(tile, mybir)

    F32 = mybir.dt.float32
    F32R = mybir.dt.float32r
    BF16 = mybir.dt.bfloat16
    AF = mybir.ActivationFunctionType
    ALU = mybir.AluOpType
    AX = mybir.AxisListType

    NB = N // 128           # 32 token blocks of 128
    NIC = N // 512          # 8 chunks of 512 tokens
    NJT = N // 128          # 32 key tiles of 128

    nc = bass.Bass(trn_type="TRN2", target_bir_lowering=False)

    tok = nc.dram_tensor("tok", [N, D], F32, kind="ExternalInput")
    wq = nc.dram_tensor("wq", [128, 4, 128], F32R, kind="ExternalInput")
    wk = nc.dram_tensor("wk", [128, 4, 128], F32R, kind="ExternalInput")
    wv = nc.dram_tensor("wv", [128, 4, 128], F32R, kind="ExternalInput")
    wo0 = nc.dram_tensor("wo0", [64, 512], F32R, kind="ExternalInput")
    wo1 = nc.dram_tensor("wo1", [64, 512], F32R, kind="ExternalInput")
    out_part = nc.dram_tensor("out_part", [N, D], F32, kind="ExternalOutput")

    tok_r = tok.rearrange("(ic t p) d -> ic p t d", t=4, p=128)
    out_r = out_part.rearrange("(ib t p) e -> ib p t e", t=4, p=128)

    with tile.TileContext(nc) as tc:
        with tc.tile_pool(name="consts", bufs=1) as consts, \
             tc.tile_pool(name="big", bufs=1) as big, \
             tc.tile_pool(name="wpool", bufs=1) as wpool:

            # ---- constants ----
            ident_f = consts.tile([128, 128], F32)
            make_identity(nc, ident_f)
            ident = consts.tile([128, 128], F32R)
            nc.vector.tensor_copy(ident, ident_f)
            eps_t = consts.tile([128, 1], F32)
            nc.vector.memset(eps_t, EPS)
            ones_f = consts.tile([128, 64], F32)
            nc.vector.memset(ones_f, 1.0)
            ones_r = consts.tile([1, 64], F32R)
            nc.vector.tensor_copy(ones_r, ones_f[0:1, :])
            # preload the Exp ACT table during the otherwise-idle start window
            warm = consts.tile([1, 1], F32)
            nc.scalar.activation(warm, eps_t[0:1, :], AF.Exp)

            # ---- weights ----
            wq_sb = wpool.tile([128, 4, 128], F32R)
            wk_sb = wpool.tile([128, 4, 128], F32R)
            wv_sb = wpool.tile([128, 4, 128], F32R)
            wo0_sb = wpool.tile([64, 512], F32R)
            wo1_sb = wpool.tile([64, 512], F32R)
            nc.sync.dma_start(out=wq_sb, in_=wq[:, :, :])
            nc.sync.dma_start(out=wk_sb, in_=wk[:, :, :])
            nc.sync.dma_start(out=wv_sb, in_=wv[:, :, :])
            nc.sync.dma_start(out=wo0_sb, in_=wo0[:, :])
            nc.sync.dma_start(out=wo1_sb, in_=wo1[:, :])

            # ---- persistent big buffers ----
            QT = big.tile([128, N], F32R)       # [2 heads x 64 qdims, n]
            KT = big.tile([128, N], F32R)
            Vb = big.tile([128, NJT, 2, 65], F32R)  # per j-tile: [v(64)|ones] per head
            nc.vector.tensor_copy(
                Vb[:, :, :, 64:65],
                ones_f.rearrange("p (a b c) -> p a b c", a=NJT, b=2, c=1))

            GROUPS = []
            jt0 = 0
            while jt0 < NJT:
                g = min(3, NJT - jt0)
                GROUPS.append(list(range(jt0, jt0 + g)))
                jt0 += g
            from contextlib import ExitStack
            outer_ctx = ExitStack()
            ptp = outer_ctx.enter_context(tc.tile_pool(name="pt_pool", bufs=4))
            ops = outer_ctx.enter_context(tc.tile_pool(name="o_psum", bufs=1, space="PSUM"))
            mixps = outer_ctx.enter_context(tc.tile_pool(name="mix_psum", bufs=1, space="PSUM"))
            first_pv = [True]

            def emit_flash_group(o_ps, hl, ib, grp, st):
                h0 = hl * 64
                g = len(grp)
                for k, jt in enumerate(grp):
                    nc.tensor.matmul(
                        st[:, k, :],
                        KT[h0:h0 + 64, jt * 128:(jt + 1) * 128],
                        QT[h0:h0 + 64, ib * 512:(ib + 1) * 512],
                        start=True, stop=True)
                pt = ptp.tile([128, 3, 512], F32R, tag="pt", name="ptg")
                nc.scalar.activation(
                    pt[:, 0:g, :].rearrange("p a b -> p (a b)"),
                    st[:, 0:g, :].rearrange("p a b -> p (a b)"),
                    AF.Exp)
                for k, jt in enumerate(grp):
                    nc.tensor.matmul(
                        o_ps, Vb[:, jt, hl, :], pt[:, k, :],
                        start=first_pv[0], stop=(jt == NJT - 1))
                    first_pv[0] = False

            with tc.tile_pool(name="ab_sbuf", bufs=3) as abp, \
                 tc.tile_pool(name="ab_stats", bufs=4) as stp, \
                 tc.tile_pool(name="ab_psum", bufs=3, space="PSUM") as abps, \
                 tc.tile_pool(name="qk_psum", bufs=2, space="PSUM") as qkps, \
                 tc.tile_pool(name="scr_psum", bufs=1, space="PSUM") as scrps:

                # PE joins: absorb each weight-DMA semaphore with a tiny bf16 matmul
                scr = scrps.tile([2, 2], F32, tag="scr", name="scrj")
                for i, wtile in enumerate((wq_sb, wk_sb, wv_sb, wo0_sb, wo1_sb)):
                    # high bf16 halves of 2 consecutive f32 weights (low halves can be NaN bits)
                    if len(wtile.shape) == 3:
                        src = wtile[0:1, 0:1, 0:2].bitcast(BF16)[:, 0, 1::2]
                    else:
                        src = wtile[0:1, 0:2].bitcast(BF16)[:, 1::2]
                    nc.tensor.matmul(scr, src, src, start=(i == 0), stop=(i == 4))

                for ic in range(NIC):
                    tok4 = abp.tile([128, 4, 512], F32, tag="tok4")
                    nc.gpsimd.dma_start(out=tok4, in_=tok_r[ic])
                    xt = abp.tile([128, 4, 512], F32R, tag="xt")
                    for t in range(4):
                        stats = stp.tile([128, 6], F32, tag="stats")
                        mv = stp.tile([128, 2], F32, tag="mv")
                        ms = stp.tile([128, 1], F32, tag="ms")
                        s_t = stp.tile([128, 1], F32, tag="s_t")
                        rstd = stp.tile([128, 1], F32, tag="rstd")
                        nc.vector.bn_stats(stats, tok4[:, t, :])
                        nc.vector.bn_aggr(mv, stats)
                        # E[x^2] = mean^2 + var
                        nc.vector.scalar_tensor_tensor(
                            ms, mv[:, 0:1], mv[:, 0:1], mv[:, 1:2],
                            op0=ALU.mult, op1=ALU.add)
                        nc.scalar.activation(s_t, ms, AF.Sqrt, bias=eps_t, scale=1.0)
                        nc.vector.reciprocal(rstd, s_t)
                        # alternate the scale between DVE and GpSimd to
                        # balance the two phase-A bottleneck engines; per-t
                        # tiles so each transpose depends only on its own scale
                        xn = stp.tile([128, 512], F32R, tag="xn")
                        eng = nc.vector if t % 2 == 0 else nc.gpsimd
                        eng.tensor_scalar_mul(xn, in0=tok4[:, t, :], scalar1=rstd)
                        tp = abps.tile([128, 4, 128], F32R, tag="tp")
                        for c in range(4):
                            nc.tensor.transpose(tp[:, c, :], xn[:, c * 128:(c + 1) * 128], ident)
                        nc.scalar.copy(xt[:, :, t * 128:(t + 1) * 128], tp)

                    # QKV^T for this 512-token chunk (V first: see DVE ordering note)
                    vt = abp.tile([128, 512], F32R, tag="vt")
                    for wtile, dst in ((wv_sb, None), (wq_sb, QT), (wk_sb, KT)):
                        ps = qkps.tile([128, 512], F32, tag="qk")
                        for c in range(4):
                            nc.tensor.matmul(ps, wtile[:, c, :], xt[:, c, :],
                                             start=(c == 0), stop=(c == 3))
                        if dst is None:
                            nc.vector.tensor_copy(vt, ps)
                        else:
                            nc.vector.tensor_copy(dst[:, ic * 512:(ic + 1) * 512], ps)
                    # V^T -> V natural into Vb (j on partitions), all 4 j-tiles
                    vtp = abps.tile([128, 4, 128], F32R, tag="tp")
                    for jl in range(4):
                        nc.tensor.transpose(vtp[:, jl, :], vt[:, jl * 128:(jl + 1) * 128], ident)
                    nc.vector.tensor_copy(
                        Vb[:, ic * 4:(ic + 1) * 4, :, 0:64],
                        vtp.rearrange("p jl (h v) -> p jl h v", h=2))

            # ---- phase C + D ----
            with tc.tile_pool(name="c_sbuf", bufs=3) as cp, \
                 tc.tile_pool(name="osb_pool", bufs=3) as osbp, \
                 tc.tile_pool(name="lc_pool", bufs=3) as lcp, \
                 tc.tile_pool(name="st_psum", bufs=2, space="PSUM") as stps:

                saved = {}

                def emit_tail(o_ps, ib, hl):
                    # l = row 64 of o_ps; o^T = rows 0..63. 1/l is folded into
                    # the out-projection extraction (per-partition scalar),
                    # so we need l transposed to [i-partitions, 1] — done with
                    # a small SBUF->SBUF DMA scatter.
                    l_sb = cp.tile([1, 512], F32, tag="l_sb")
                    nc.vector.tensor_copy(l_sb, o_ps[64:65, :])
                    o_sb = osbp.tile([64, 512], F32R, tag=f"osb{hl}")
                    nc.vector.tensor_copy(o_sb, o_ps[0:64, :])
                    linv_pre = cp.tile([128, 4], F32, tag="linv_pre")
                    for it in range(4):
                        nc.sync.dma_start(
                            out=linv_pre[:, it:it + 1],
                            in_=l_sb[0:1, it * 128:(it + 1) * 128])
                    linv_col = lcp.tile([128, 4], F32, tag=f"lc{hl}")
                    nc.vector.reciprocal(linv_col, linv_pre)
                    saved[(ib, hl)] = (o_sb, linv_col)

                def emit_outproj_tile(ib, it, out_sb, tmp_on_act=False,
                                      use_st=False):
                    o_sb0, lc0 = saved[(ib, 0)]
                    o_sb1, lc1 = saved[(ib, 1)]
                    pool = stps if use_st else mixps
                    tg = "st" if use_st else "mix"
                    op_ps = pool.tile([128, 512], F32, tag=tg)
                    nc.tensor.matmul(op_ps, o_sb0[:, it * 128:(it + 1) * 128],
                                     wo0_sb, start=True, stop=True)
                    tmp = cp.tile([128, 512], F32, tag="tmp")
                    if tmp_on_act:
                        nc.scalar.mul(tmp, op_ps, lc0[:, it:it + 1])
                    else:
                        nc.vector.tensor_scalar_mul(tmp, in0=op_ps, scalar1=lc0[:, it:it + 1])
                    op_ps2 = pool.tile([128, 512], F32, tag=tg)
                    nc.tensor.matmul(op_ps2, o_sb1[:, it * 128:(it + 1) * 128],
                                     wo1_sb, start=True, stop=True)
                    nc.vector.scalar_tensor_tensor(
                        out_sb[:, it, :], op_ps2, lc1[:, it:it + 1], tmp,
                        op0=ALU.mult, op1=ALU.add)

                def emit_st(ib, hl, grp):
                    h0 = hl * 64
                    st = stps.tile([128, 3, 512], F32, tag="st", name="stg")
                    for k, jt in enumerate(grp):
                        nc.tensor.matmul(
                            st[:, k, :],
                            KT[h0:h0 + 64, jt * 128:(jt + 1) * 128],
                            QT[h0:h0 + 64, ib * 512:(ib + 1) * 512],
                            start=True, stop=True)
                    return st

                prev = None
                pending_op = None  # i-block whose out-projection is owed
                out_sbs = {}
                iters = [(ib, hl) for ib in range(NIC) for hl in range(2)]
                # S^T groups are emitted one step ahead of exp/PV so the PE
                # keeps ACT fed across group and iteration boundaries
                st_cur = emit_st(iters[0][0], iters[0][1], GROUPS[0])
                for idx, (ib, hl) in enumerate(iters):
                    o_ps = ops.tile([65, 512], F32, tag="o")
                    first_pv[0] = True
                    for g_idx, grp in enumerate(GROUPS):
                        g = len(grp)
                        st = st_cur
                        pt = ptp.tile([128, 3, 512], F32R, tag="pt", name="ptg")
                        nc.scalar.activation(
                            pt[:, 0:g, :].rearrange("p a b -> p (a b)"),
                            st[:, 0:g, :].rearrange("p a b -> p (a b)"),
                            AF.Exp)
                        if g_idx + 1 < len(GROUPS):
                            st_cur = emit_st(ib, hl, GROUPS[g_idx + 1])
                        elif idx + 1 < len(iters):
                            st_cur = emit_st(iters[idx + 1][0], iters[idx + 1][1],
                                             GROUPS[0])
                        # software-pipelined: previous iteration's tail
                        # BEFORE this iteration's first PV (which reuses the
                        # single o bank and must wait for the tail's reads)
                        if g_idx == 0 and prev is not None:
                            emit_tail(*prev)
                            if prev[2] == 1:
                                pending_op = prev[1]
                                out_sbs[pending_op] = cp.tile(
                                    [128, 4, 512], F32, tag="out_sb",
                                    name=f"outsb{pending_op}")
                        elif 1 <= g_idx <= 4 and pending_op is not None:
                            emit_outproj_tile(pending_op, g_idx - 1,
                                              out_sbs[pending_op])
                            if g_idx == 4:
                                done = pending_op
                                nc.sync.dma_start(out=out_r[done],
                                                  in_=out_sbs.pop(done))
                                saved.pop((done, 0))
                                saved.pop((done, 1))
                                pending_op = None
                        for k, jt in enumerate(grp):
                            nc.tensor.matmul(
                                o_ps, Vb[:, jt, hl, :], pt[:, k, :],
                                start=first_pv[0], stop=(jt == NJT - 1))
                            first_pv[0] = False
                    prev = (o_ps, ib, hl)

                # final tail + out-projection (use the now-free st slots so the
                # 8 matmuls don't serialize on one PSUM bank)
                emit_tail(*prev)
                fin = prev[1]
                out_fin = cp.tile([128, 4, 512], F32, tag="out_sb")
                for it in range(4):
                    emit_outproj_tile(fin, it, out_fin, tmp_on_act=True,
                                      use_st=True)
                nc.sync.dma_start(out=out_r[fin], in_=out_fin)
            outer_ctx.close()

    fix_waits_nc(nc, mybir)
    return nc


def fix_waits_nc(nc, mybir):
    """Post-pass over the scheduled program: (1) remove semaphore waits that
    are transitively implied by earlier waits (Tile emits per-proc-minimal,
    not transitively-minimal, waits), (2) split any instruction still
    carrying more than one wait by injecting single-wait NoOps in front of
    it — this walrus build rejects >1 sync wait per instruction.
    Mutates nc in place so CoreSim and hardware run identical sync."""
    nop_id = [0]

    def _is_ge(w):
        return w.sync_type == "semaphore" and w.wait_mode == "sem-ge-imm"

    for fn in nc.m.functions:
        for blk in fn.blocks:
            insts = list(blk.instructions)
            n = len(insts)

            producers = {}
            cum = {}
            nonmono = set()  # sems ever decremented: counter logic invalid
            for idx, inst in enumerate(insts):
                si = inst.sync_info
                for u in (si.on_update if si else []) or []:
                    if u.sync_type != "semaphore":
                        continue
                    sid = u.id
                    if u.update_mode != "sem-inc":
                        nonmono.add(sid)
                        continue
                    cum[sid] = cum.get(sid, 0) + int(u.update_value)
                    producers.setdefault(sid, []).append((cum[sid], idx))

            def producer_of(sid, val):
                for cv, idx in producers.get(sid, ()):
                    if cv >= val:
                        return idx
                return None

            prev_eng = [None] * n
            last = {}
            for idx, inst in enumerate(insts):
                e = inst.engine
                prev_eng[idx] = last.get(e)
                last[e] = idx

            def get_waits(inst):
                si = inst.sync_info
                return list(si.on_wait) if si and si.on_wait else []

            def is_ge(w):
                return _is_ge(w) and w.id not in nonmono

            know = [dict() for _ in range(n)]
            for _ in range(3):
                changed = False
                for idx, inst in enumerate(insts):
                    k = dict(know[prev_eng[idx]]) if prev_eng[idx] is not None else {}
                    for w in get_waits(inst):
                        if not is_ge(w):
                            continue
                        sid, val = w.id, int(w.wait_value)
                        if k.get(sid, -1) < val:
                            k[sid] = val
                        p = producer_of(sid, val)
                        if p is not None:
                            for s2, v2 in know[p].items():
                                if k.get(s2, -1) < v2:
                                    k[s2] = v2
                    if k != know[idx]:
                        know[idx] = k
                        changed = True
                if not changed:
                    break

            new_insts = []
            dirty = False
            for idx, inst in enumerate(insts):
                si = inst.sync_info
                waits = get_waits(inst)
                if si is not None and waits:
                    base = dict(know[prev_eng[idx]]) if prev_eng[idx] is not None else {}
                    kept = []
                    for w in waits:
                        if is_ge(w):
                            sid, val = w.id, int(w.wait_value)
                            if base.get(sid, -1) >= val:
                                continue
                            base[sid] = val
                            p = producer_of(sid, val)
                            if p is not None:
                                for s2, v2 in know[p].items():
                                    if base.get(s2, -1) < v2:
                                        base[s2] = v2
                        kept.append(w)
                    if len(kept) != len(waits) or len(kept) > 1:
                        dirty = True
                        for w in kept[:-1]:
                            nop_id[0] += 1
                            nop = mybir.InstNoOp(
                                name=f"I-waitfix-{nop_id[0]}", ins=[], outs=[])
                            nop.engine = inst.engine
                            nop.sync_info = mybir.SyncInfo(on_wait=[w], on_update=[])
                            nc.register_instruction(nop)
                            new_insts.append(nop)
                        inst.sync_info = mybir.SyncInfo(
                            on_wait=kept[-1:],
                            on_update=list(si.on_update or []))
                new_insts.append(inst)
            if dirty:
                blk.instructions = new_insts


def get_program():
    if "nc" not in _prog_cache:
        _prog_cache["nc"] = build_program()
    return _prog_cache["nc"]


def _prep_inputs(tokens, norm_weight, w_qkv, w_out):
    tokens = np.ascontiguousarray(np.asarray(tokens, dtype=np.float32))
    norm_weight = np.asarray(norm_weight, dtype=np.float32)
    w_qkv = np.asarray(w_qkv, dtype=np.float32)
    w_out = np.asarray(w_out, dtype=np.float32)

    wp = w_qkv * norm_weight[:, None]  # fold RMSNorm weight into qkv weights

    in_maps = []
    for c in range(NCORES):
        b = c // 4
        h0 = 2 * (c % 4)
        cols = np.r_[h0 * DH:(h0 + 1) * DH, (h0 + 1) * DH:(h0 + 2) * DH]
        m = {}
        m["tok"] = tokens[b]
        for name, off in (("wq", 0), ("wk", DI), ("wv", 2 * DI)):
            w = wp[:, off + h0 * DH: off + (h0 + 2) * DH]       # [512, 128]
            m[name] = np.ascontiguousarray(
                w.reshape(4, 128, 128).transpose(1, 0, 2))       # [128, 4, 128]
        rows = w_out[h0 * DH:(h0 + 2) * DH, :]                   # [128, 512]
        m["wo0"] = np.ascontiguousarray(rows[0:64])
        m["wo1"] = np.ascontiguousarray(rows[64:128])
        in_maps.append(m)
    return in_maps


def run(tokens, norm_weight, w_qkv, w_out, trace=False):
    from concourse.bass_utils import run_bass_kernel_spmd
    nc = get_program()
    in_maps = _prep_inputs(tokens, norm_weight, w_qkv, w_out)
    res = run_bass_kernel_spmd(nc, in_maps, core_ids=list(range(NCORES)), trace=trace)
    parts = [res.results[c]["out_part"] for c in range(NCORES)]
    out = np.empty((B, N, D), dtype=np.float32)
    for b in range(B):
        out[b] = parts[4 * b] + parts[4 * b + 1] + parts[4 * b + 2] + parts[4 * b + 3]
    return out, res


def kernel(tokens, norm_weight, w_qkv, w_out):
    out, _ = run(tokens, norm_weight, w_qkv, w_out, trace=False)
    return out



# revision 11
# speedup vs baseline: 1.2898x; 1.2898x over previous
"""TRN2 Bass kernel for nn_Attention (RMSNorm + QKV + softmax attention + out-proj).

Sharding: 8 cores = 2 batches x 4 head-pairs. Core c handles batch c//4 and
heads (2*(c%4), 2*(c%4)+1). Each core computes its partial out-projection
(contracting only its 128 rows of dim_inner); host sums the 4 partials per batch.

V2 design (cost-model driven):
  A) stream tokens [128,512] blocks: RMSNorm stats on DVE, rstd scale, PE
     transpose -> x^T (rotating [128,4,512] per 512-token chunk)
  B) QKV^T = w^T @ x^T (PE); V^T -> bf16 -> transposed back to V-natural with
     a ones column appended per head (softmax denominator for free)
  C) flash attention per (ib=512 queries, head): for each j-tile of 128 keys:
     S^T [128j, 512i] on PE (f32r, 512 cycles); exp split across THREE engines
     (ACT: exact table exp -> bf16; DVE/GPSIMD: Schraudolph bit-trick exp via
     tensor_scalar -> int16 bits of bf16). PV uses exp(S)^T tiles as the
     STATIONARY [128j, 128i] and bf16 V[128j,64+1] as MOVING: 65-cycle
     matmuls accumulate O-natural [128i, 65] in PSUM (col 64 = l).
  D) tail: linv=1/l (DVE), scale O by linv -> bf16 pair tile [i, 128dh both
     heads] (DVE), PE transpose [128,128], one 128-contraction out-proj matmul
     per i-tile vs bf16 w_out, ACT drains psum->sbuf, DMA out per ib.

Engine budget per core (cost model): PE ~206us (bottleneck), ACT ~150us,
DVE ~180us, Pool ~110us. exp assignment per j-tile keeps all three exp
engines concurrently busy so PE never starves.

Schraudolph exp: bf16_bits(p) = int16(S * (2^7/ln2) + C2B). Bias component
cancels through the softmax denominator; residual sawtooth ~1% std on the
offloaded fraction keeps total rel err well under the 2e-2 gate.

Engine discipline: walrus here allows only ONE semaphore wait per instruction
(fix_waits_nc post-pass splits/dedups), and no mixed 32/16-bit matmuls.
"""
import sys
sys.path.insert(0, "/opt/trn_rl_repo")
import numpy as np

B, N, D = 2, 4096, 512
H, DH = 8, 64
DI = H * DH
NCORES = 8
EPS = 1.1920929e-07  # float32 eps (torch nn.RMSNorm default)

# Schraudolph constants for bf16-bit exp: bits16(e^x) ~= x*C1B + C2B
C1B = 128.0 / 0.6931471805599453          # 2^7 / ln 2
C2B = float(127 * 128) - 366393.0 / 65536.0

_prog_cache = {}


def _patch_drain(tile_mod, mybir):
    """Split the multi-wait tail drain into a chain of single-wait drains
    (this walrus build rejects >1 sync wait per instruction)."""
    if getattr(tile_mod.TileContext, "_drain_patched", False):
        return

    def _patched(self, tick_clock, wait_clock):
        from concourse.vector_clock import ScopedClock
        nc = self.nc
        drain_inst = nc.sync.drain()
        wait_clock.add_sem_waits(drain_inst.ins, ScopedClock({None: tick_clock.global_clock}))
        si = drain_inst.ins.sync_info
        if si is not None and si.on_wait and len(si.on_wait) > 1:
            waits = list(si.on_wait)
            drain_inst.ins.sync_info = mybir.SyncInfo(
                on_wait=waits[:1], on_update=list(si.on_update or []))
            for w in waits[1:]:
                d2 = nc.sync.drain()
                d2.ins.sync_info = mybir.SyncInfo(on_wait=[w], on_update=[])
        nc.all_engine_barrier()
        assert self.sems is not None
        popped = nc._tile_sem_poison_stack.pop()
        assert popped is self._sem_poison
        nc.clear_and_free_semaphores(list(self.sems.allocated().values()))
        nc.all_engine_barrier()

    tile_mod.TileContext._drain_and_barrier = _patched
    tile_mod.TileContext._drain_patched = True


def build_program():
    import concourse.bass as bass
    import concourse.tile as tile
    from concourse import mybir
    from concourse.masks import make_identity

    _patch_drain(tile, mybir)

    F32 = mybir.dt.float32
    F32R = mybir.dt.float32r
    BF16 = mybir.dt.bfloat16
    I16 = mybir.dt.int16
    AF = mybir.ActivationFunctionType
    ALU = mybir.AluOpType

    NIC = N // 512          # 8 chunks of 512 tokens
    NJT = N // 128          # 32 key tiles of 128

    nc = bass.Bass(trn_type="TRN2", target_bir_lowering=False)

    tok = nc.dram_tensor("tok", [N, D], F32, kind="ExternalInput")
    wq = nc.dram_tensor("wq", [128, 4, 128], F32R, kind="ExternalInput")
    wk = nc.dram_tensor("wk", [128, 4, 128], F32R, kind="ExternalInput")
    wv = nc.dram_tensor("wv", [128, 4, 128], F32R, kind="ExternalInput")
    wo = nc.dram_tensor("wo", [128, 512], BF16, kind="ExternalInput")
    out_part = nc.dram_tensor("out_part", [N, D], F32, kind="ExternalOutput")

    tok_r = tok.rearrange("(ic t p) d -> ic p t d", t=4, p=128)
    out_r = out_part.rearrange("(ib t p) e -> ib p t e", t=4, p=128)

    # exp-engine assignment per j-tile within an iteration (A=ACT exact,
    # D=DVE Schraudolph), weighted so both engines finish together just
    # below the PE's per-iteration time. GPSIMD cannot read PSUM, so it
    # cannot join the exp crew; it takes phase-A SBUF work instead.
    W_A, W_D = 17, 15
    ENG = []
    cnt = {"A": 0, "D": 0}
    wgt = {"A": W_A, "D": W_D}
    for _ in range(NJT):
        e = min(cnt, key=lambda k: (cnt[k] + 1) / wgt[k])
        ENG.append(e)
        cnt[e] += 1

    with tile.TileContext(nc) as tc:
        with tc.tile_pool(name="consts", bufs=1) as consts, \
             tc.tile_pool(name="big", bufs=1) as big, \
             tc.tile_pool(name="wpool", bufs=1) as wpool:

            # ---- constants ----
            ident_f = consts.tile([128, 128], F32)
            make_identity(nc, ident_f)
            ident = consts.tile([128, 128], F32R)
            nc.vector.tensor_copy(ident, ident_f)
            identb = consts.tile([128, 128], BF16)
            nc.vector.tensor_copy(identb, ident_f)
            eps_t = consts.tile([128, 1], F32)
            nc.vector.memset(eps_t, EPS)
            # preload the Exp ACT table during the otherwise-idle start window
            warm = consts.tile([1, 1], F32)
            nc.scalar.activation(warm, eps_t[0:1, :], AF.Exp)

            # ---- weights ----
            wq_sb = wpool.tile([128, 4, 128], F32R)
            wk_sb = wpool.tile([128, 4, 128], F32R)
            wv_sb = wpool.tile([128, 4, 128], F32R)
            wo_sb = wpool.tile([128, 512], BF16)
            nc.sync.dma_start(out=wq_sb, in_=wq[:, :, :])
            nc.sync.dma_start(out=wk_sb, in_=wk[:, :, :])
            nc.sync.dma_start(out=wv_sb, in_=wv[:, :, :])
            nc.sync.dma_start(out=wo_sb, in_=wo[:, :])

            # ---- persistent big buffers ----
            QT = big.tile([128, N], F32R)       # [2 heads x 64 qdims, n]
            KT = big.tile([128, N], F32R)
            Vb = big.tile([128, NJT, 2, 65], BF16)  # per j-tile: [v(64)|ones] per head
            ones_bf = consts.tile([128, NJT, 2, 1], BF16)
            nc.vector.memset(ones_bf, 1.0)
            nc.vector.tensor_copy(Vb[:, :, :, 64:65], ones_bf)

            # ---- phase A/B: RMSNorm + x^T + QKV^T + V natural ----
            with tc.tile_pool(name="ab_sbuf", bufs=3) as abp, \
                 tc.tile_pool(name="ab_stats", bufs=4) as stp, \
                 tc.tile_pool(name="ab_psum", bufs=3, space="PSUM") as abps, \
                 tc.tile_pool(name="qk_psum", bufs=2, space="PSUM") as qkps, \
                 tc.tile_pool(name="scr_psum", bufs=1, space="PSUM") as scrps:

                # PE joins: absorb each weight-DMA semaphore with a tiny bf16 matmul
                scr = scrps.tile([2, 2], F32, tag="scr", name="scrj")
                BF = mybir.dt.bfloat16
                for i, wtile in enumerate((wq_sb, wk_sb, wv_sb)):
                    src = wtile[0:1, 0:1, 0:2].bitcast(BF)[:, 0, 1::2]
                    nc.tensor.matmul(scr, src, src, start=(i == 0), stop=False)
                nc.tensor.matmul(scr, wo_sb[0:1, 0:2], wo_sb[0:1, 0:2],
                                 start=False, stop=True)

                for ic in range(NIC):
                    tok4 = abp.tile([128, 4, 512], F32, tag="tok4")
                    nc.sync.dma_start(out=tok4, in_=tok_r[ic])
                    xt = abp.tile([128, 4, 512], F32R, tag="xt")
                    for t in range(4):
                        stats = stp.tile([128, 6], F32, tag="stats")
                        mv = stp.tile([128, 2], F32, tag="mv")
                        ms = stp.tile([128, 1], F32, tag="ms")
                        s_t = stp.tile([128, 1], F32, tag="s_t")
                        rstd = stp.tile([128, 1], F32, tag="rstd")
                        nc.vector.bn_stats(stats, tok4[:, t, :])
                        nc.vector.bn_aggr(mv, stats)
                        # E[x^2] = mean^2 + var
                        nc.vector.scalar_tensor_tensor(
                            ms, mv[:, 0:1], mv[:, 0:1], mv[:, 1:2],
                            op0=ALU.mult, op1=ALU.add)
                        nc.scalar.activation(s_t, ms, AF.Sqrt, bias=eps_t, scale=1.0)
                        nc.vector.reciprocal(rstd, s_t)
                        # normalization scale on GpSimd (SBUF-only engine;
                        # keeps DVE free for the QT/KT psum drains)
                        xn = stp.tile([128, 512], F32R, tag="xn")
                        nc.gpsimd.tensor_scalar_mul(xn, in0=tok4[:, t, :], scalar1=rstd)
                        tp = abps.tile([128, 4, 128], F32R, tag="tp")
                        for c in range(4):
                            nc.tensor.transpose(tp[:, c, :], xn[:, c * 128:(c + 1) * 128], ident)
                        nc.scalar.copy(xt[:, :, t * 128:(t + 1) * 128], tp)

                    # QKV^T for this 512-token chunk (V first: DVE ordering)
                    vt = abp.tile([128, 512], BF16, tag="vt")
                    for wtile, dst in ((wv_sb, None), (wq_sb, QT), (wk_sb, KT)):
                        ps = qkps.tile([128, 512], F32, tag="qk")
                        for c in range(4):
                            nc.tensor.matmul(ps, wtile[:, c, :], xt[:, c, :],
                                             start=(c == 0), stop=(c == 3))
                        if dst is None:
                            nc.scalar.copy(vt, ps)
                        else:
                            nc.vector.tensor_copy(dst[:, ic * 512:(ic + 1) * 512], ps)
                    # V^T -> V natural (bf16) into Vb (j on partitions)
                    vtp = abps.tile([128, 4, 128], BF16, tag="tp")
                    for jl in range(4):
                        nc.tensor.transpose(vtp[:, jl, :], vt[:, jl * 128:(jl + 1) * 128], identb)
                    nc.vector.tensor_copy(
                        Vb[:, ic * 4:(ic + 1) * 4, :, 0:64],
                        vtp.rearrange("p jl (h v) -> p jl h v", h=2))

            # ---- phase C: attention + out-proj ----
            with tc.tile_pool(name="c_sbuf", bufs=3) as cp, \
                 tc.tile_pool(name="pt_pool", bufs=6) as ptp, \
                 tc.tile_pool(name="opair_pool", bufs=2) as oprp, \
                 tc.tile_pool(name="outsb_pool", bufs=2) as outsbp, \
                 tc.tile_pool(name="st_psum", bufs=4, space="PSUM") as stps, \
                 tc.tile_pool(name="o_psum", bufs=2, space="PSUM") as ops, \
                 tc.tile_pool(name="ot_psum", bufs=1, space="PSUM") as otps, \
                 tc.tile_pool(name="out_psum", bufs=1, space="PSUM") as outps:

                iters = [(ib, hl) for ib in range(NIC) for hl in range(2)]
                NIT = len(iters)
                LOOK = 3  # S^T groups emitted ahead of exp/PV

                def emit_st(k):
                    ib, hl = iters[k // NJT]
                    jt = k % NJT
                    h0 = hl * 64
                    st = stps.tile([128, 512], F32, tag="st", name="stg")
                    nc.tensor.matmul(
                        st,
                        KT[h0:h0 + 64, jt * 128:(jt + 1) * 128],
                        QT[h0:h0 + 64, ib * 512:(ib + 1) * 512],
                        start=True, stop=True)
                    return st

                def emit_exp(st, jt):
                    pt = ptp.tile([128, 512], BF16, tag="pt", name="ptg")
                    e = ENG[jt]
                    if e == "A":
                        nc.scalar.activation(pt, st, AF.Exp)
                    else:
                        eng = nc.vector if e == "D" else nc.gpsimd
                        eng.tensor_scalar(pt.bitcast(I16), st, C1B, C2B,
                                          op0=ALU.mult, op1=ALU.add)
                    return pt

                # tail state
                o_accs = {}      # iter idx -> o_acc psum tile
                o_pairs = {}     # ib -> o_pair sbuf tile
                linvs = {}       # iter idx -> linv tile
                out_sbs = {}     # ib -> out_sb tile
                ots_tiles = {}   # it -> ots sbuf tile (transient per ib)

                def emit_tail_step(k_prev, g):
                    """Interleave iter k_prev's tail into iter k_prev+1's
                    group stream at slot g."""
                    ib, hl = iters[k_prev]
                    if g == 0:
                        o_acc = o_accs[k_prev]
                        linv = cp.tile([128, 4], F32, tag="linv", name=f"lv{hl}")
                        nc.vector.reciprocal(linv, o_acc[:, :, 64])
                        linvs[k_prev] = linv
                        if hl == 0:
                            o_pairs[ib] = oprp.tile(
                                [128, 4, 128], BF16, tag="opair", name=f"op{ib}")
                    elif 1 <= g <= 4:
                        it = g - 1
                        o_acc = o_accs[k_prev]
                        nc.vector.tensor_scalar_mul(
                            o_pairs[ib][:, it, hl * 64:(hl + 1) * 64],
                            in0=o_acc[:, it, 0:64],
                            scalar1=linvs[k_prev][:, it:it + 1])
                        if it == 3:
                            o_accs.pop(k_prev)
                            linvs.pop(k_prev)
                    elif hl == 1 and 5 <= g <= 20:
                        # out-proj for completed ib: per it, a 4-op chain
                        # spread across slots so psum/engine deps pipeline
                        it, ph = divmod(g - 5, 4)
                        opr = o_pairs[ib]
                        if ph == 0:
                            ot = otps.tile([128, 128], BF16, tag="ot", name="otp")
                            nc.tensor.transpose(ot, opr[:, it, :], identb)
                            ots_tiles[it] = (ot, None)
                        elif ph == 1:
                            ot, _ = ots_tiles[it]
                            ots = cp.tile([128, 128], BF16, tag="ots", name="otsb")
                            nc.vector.tensor_copy(ots, ot)
                            ots_tiles[it] = (ot, ots)
                        elif ph == 2:
                            _, ots = ots_tiles[it]
                            if it == 0:
                                out_sbs[ib] = outsbp.tile(
                                    [128, 4, 512], F32, tag="outsb", name=f"ob{ib}")
                            op_ps = outps.tile([128, 512], F32, tag="outp", name="opps")
                            nc.tensor.matmul(op_ps, ots, wo_sb, start=True, stop=True)
                            ots_tiles[it] = (op_ps, ots)
                        else:
                            op_ps, _ = ots_tiles.pop(it)
                            nc.scalar.copy(out_sbs[ib][:, it, :], op_ps)
                            if it == 3:
                                nc.sync.dma_start(out=out_r[ib], in_=out_sbs.pop(ib))
                                o_pairs.pop(ib)

                # software pipeline over all (iter, j-tile) steps
                TOT = NIT * NJT
                sts = {}
                for k in range(LOOK):
                    sts[k] = emit_st(k)
                for k in range(TOT):
                    it_idx, g = divmod(k, NJT)
                    ib, hl = iters[it_idx]
                    if g == 0:
                        o_accs[it_idx] = ops.tile(
                            [128, 4, 65], F32, tag="o", name=f"oacc{it_idx % 2}")
                    pt = emit_exp(sts.pop(k), g)
                    if k + LOOK < TOT:
                        sts[k + LOOK] = emit_st(k + LOOK)
                    if it_idx > 0:
                        emit_tail_step(it_idx - 1, g)
                    o_acc = o_accs[it_idx]
                    for it in range(4):
                        # PSUM `start` zeroes the WHOLE bank: only the very
                        # first matmul of this o_acc bank may set it
                        nc.tensor.matmul(
                            o_acc[:, it, :],
                            pt[:, it * 128:(it + 1) * 128],
                            Vb[:, g, hl, :],
                            start=(g == 0 and it == 0),
                            stop=(g == NJT - 1 and it == 3),
                            skip_group_check=True)

                # trailing tail for the final iteration
                for g in range(NJT):
                    emit_tail_step(NIT - 1, g)

    fix_waits_nc(nc, mybir)
    return nc


def fix_waits_nc(nc, mybir):
    """Post-pass over the scheduled program: (1) remove semaphore waits that
    are transitively implied by earlier waits (Tile emits per-proc-minimal,
    not transitively-minimal, waits), (2) split any instruction still
    carrying more than one wait by injecting single-wait NoOps in front of
    it — this walrus build rejects >1 sync wait per instruction.
    Mutates nc in place so CoreSim and hardware run identical sync."""
    nop_id = [0]

    def _is_ge(w):
        return w.sync_type == "semaphore" and w.wait_mode == "sem-ge-imm"

    for fn in nc.m.functions:
        for blk in fn.blocks:
            insts = list(blk.instructions)
            n = len(insts)

            producers = {}
            cum = {}
            nonmono = set()  # sems ever decremented: counter logic invalid
            for idx, inst in enumerate(insts):
                si = inst.sync_info
                for u in (si.on_update if si else []) or []:
                    if u.sync_type != "semaphore":
                        continue
                    sid = u.id
                    if u.update_mode != "sem-inc":
                        nonmono.add(sid)
                        continue
                    cum[sid] = cum.get(sid, 0) + int(u.update_value)
                    producers.setdefault(sid, []).append((cum[sid], idx))

            def producer_of(sid, val):
                for cv, idx in producers.get(sid, ()):
                    if cv >= val:
                        return idx
                return None

            prev_eng = [None] * n
            last = {}
            for idx, inst in enumerate(insts):
                e = inst.engine
                prev_eng[idx] = last.get(e)
                last[e] = idx

            def get_waits(inst):
                si = inst.sync_info
                return list(si.on_wait) if si and si.on_wait else []

            def is_ge(w):
                return _is_ge(w) and w.id not in nonmono

            know = [dict() for _ in range(n)]
            for _ in range(3):
                changed = False
                for idx, inst in enumerate(insts):
                    k = dict(know[prev_eng[idx]]) if prev_eng[idx] is not None else {}
                    for w in get_waits(inst):
                        if not is_ge(w):
                            continue
                        sid, val = w.id, int(w.wait_value)
                        if k.get(sid, -1) < val:
                            k[sid] = val
                        p = producer_of(sid, val)
                        if p is not None:
                            for s2, v2 in know[p].items():
                                if k.get(s2, -1) < v2:
                                    k[s2] = v2
                    if k != know[idx]:
                        know[idx] = k
                        changed = True
                if not changed:
                    break

            new_insts = []
            dirty = False
            for idx, inst in enumerate(insts):
                si = inst.sync_info
                waits = get_waits(inst)
                if si is not None and waits:
                    base = dict(know[prev_eng[idx]]) if prev_eng[idx] is not None else {}
                    kept = []
                    for w in waits:
                        if is_ge(w):
                            sid, val = w.id, int(w.wait_value)
                            if base.get(sid, -1) >= val:
                                continue
                            base[sid] = val
                            p = producer_of(sid, val)
                            if p is not None:
                                for s2, v2 in know[p].items():
                                    if base.get(s2, -1) < v2:
                                        base[s2] = v2
                        kept.append(w)
                    if len(kept) != len(waits) or len(kept) > 1:
                        dirty = True
                        for w in kept[:-1]:
                            nop_id[0] += 1
                            nop = mybir.InstNoOp(
                                name=f"I-waitfix-{nop_id[0]}", ins=[], outs=[])
                            nop.engine = inst.engine
                            nop.sync_info = mybir.SyncInfo(on_wait=[w], on_update=[])
                            nc.register_instruction(nop)
                            new_insts.append(nop)
                        inst.sync_info = mybir.SyncInfo(
                            on_wait=kept[-1:],
                            on_update=list(si.on_update or []))
                new_insts.append(inst)
            if dirty:
                blk.instructions = new_insts


def get_program():
    if "nc" not in _prog_cache:
        _prog_cache["nc"] = build_program()
    return _prog_cache["nc"]


def _prep_inputs(tokens, norm_weight, w_qkv, w_out):
    import ml_dtypes
    tokens = np.ascontiguousarray(np.asarray(tokens, dtype=np.float32))
    norm_weight = np.asarray(norm_weight, dtype=np.float32)
    w_qkv = np.asarray(w_qkv, dtype=np.float32)
    w_out = np.asarray(w_out, dtype=np.float32)

    wp = w_qkv * norm_weight[:, None]  # fold RMSNorm weight into qkv weights

    in_maps = []
    for c in range(NCORES):
        b = c // 4
        h0 = 2 * (c % 4)
        m = {}
        m["tok"] = tokens[b]
        for name, off in (("wq", 0), ("wk", DI), ("wv", 2 * DI)):
            w = wp[:, off + h0 * DH: off + (h0 + 2) * DH]       # [512, 128]
            m[name] = np.ascontiguousarray(
                w.reshape(4, 128, 128).transpose(1, 0, 2))       # [128, 4, 128]
        rows = w_out[h0 * DH:(h0 + 2) * DH, :]                   # [128, 512]
        m["wo"] = np.ascontiguousarray(rows.astype(ml_dtypes.bfloat16))
        in_maps.append(m)
    return in_maps


def run(tokens, norm_weight, w_qkv, w_out, trace=False):
    from concourse.bass_utils import run_bass_kernel_spmd
    nc = get_program()
    in_maps = _prep_inputs(tokens, norm_weight, w_qkv, w_out)
    res = run_bass_kernel_spmd(nc, in_maps, core_ids=list(range(NCORES)), trace=trace)
    parts = [res.results[c]["out_part"] for c in range(NCORES)]
    out = np.empty((B, N, D), dtype=np.float32)
    for b in range(B):
        out[b] = parts[4 * b] + parts[4 * b + 1] + parts[4 * b + 2] + parts[4 * b + 3]
    return out, res


def kernel(tokens, norm_weight, w_qkv, w_out):
    out, _ = run(tokens, norm_weight, w_qkv, w_out, trace=False)
    return out


# revision 27
# speedup vs baseline: 1.3222x; 1.0252x over previous
"""TRN2 Bass kernel for nn_Attention (RMSNorm + QKV + softmax attention + out-proj).

Sharding: 8 cores = 2 batches x 4 head-pairs. Core c handles batch c//4 and
heads (2*(c%4), 2*(c%4)+1). Each core computes its partial out-projection
(contracting only its 128 rows of dim_inner); host sums the 4 partials per batch.

V2 design (cost-model driven):
  A) stream tokens [128,512] blocks: RMSNorm stats on DVE, rstd scale, PE
     transpose -> x^T (rotating [128,4,512] per 512-token chunk)
  B) QKV^T = w^T @ x^T (PE); V^T -> bf16 -> transposed back to V-natural with
     a ones column appended per head (softmax denominator for free)
  C) flash attention per (ib=512 queries, head): for each j-tile of 128 keys:
     S^T [128j, 512i] on PE (f32r, 512 cycles); exp split across THREE engines
     (ACT: exact table exp -> bf16; DVE/GPSIMD: Schraudolph bit-trick exp via
     tensor_scalar -> int16 bits of bf16). PV uses exp(S)^T tiles as the
     STATIONARY [128j, 128i] and bf16 V[128j,64+1] as MOVING: 65-cycle
     matmuls accumulate O-natural [128i, 65] in PSUM (col 64 = l).
  D) tail: linv=1/l (DVE), scale O by linv -> bf16 pair tile [i, 128dh both
     heads] (DVE), PE transpose [128,128], one 128-contraction out-proj matmul
     per i-tile vs bf16 w_out, ACT drains psum->sbuf, DMA out per ib.

Engine budget per core (cost model): PE ~206us (bottleneck), ACT ~150us,
DVE ~180us, Pool ~110us. exp assignment per j-tile keeps all three exp
engines concurrently busy so PE never starves.

Schraudolph exp: bf16_bits(p) = int16(S * (2^7/ln2) + C2B). Bias component
cancels through the softmax denominator; residual sawtooth ~1% std on the
offloaded fraction keeps total rel err well under the 2e-2 gate.

Engine discipline: walrus here allows only ONE semaphore wait per instruction
(fix_waits_nc post-pass splits/dedups), and no mixed 32/16-bit matmuls.
"""
import sys
sys.path.insert(0, "/opt/trn_rl_repo")
import numpy as np

B, N, D = 2, 4096, 512
H, DH = 8, 64
DI = H * DH
NCORES = 8
EPS = 1.1920929e-07  # float32 eps (torch nn.RMSNorm default)

# Schraudolph constants for bf16-bit exp: bits16(e^x) ~= x*C1B + C2B
C1B = 128.0 / 0.6931471805599453          # 2^7 / ln 2
C2B = float(127 * 128) - 366393.0 / 65536.0

_prog_cache = {}


def _patch_drain(tile_mod, mybir):
    """Split the multi-wait tail drain into a chain of single-wait drains
    (this walrus build rejects >1 sync wait per instruction)."""
    if getattr(tile_mod.TileContext, "_drain_patched", False):
        return

    def _patched(self, tick_clock, wait_clock):
        from concourse.vector_clock import ScopedClock
        nc = self.nc
        drain_inst = nc.sync.drain()
        wait_clock.add_sem_waits(drain_inst.ins, ScopedClock({None: tick_clock.global_clock}))
        si = drain_inst.ins.sync_info
        if si is not None and si.on_wait and len(si.on_wait) > 1:
            waits = list(si.on_wait)
            drain_inst.ins.sync_info = mybir.SyncInfo(
                on_wait=waits[:1], on_update=list(si.on_update or []))
            for w in waits[1:]:
                d2 = nc.sync.drain()
                d2.ins.sync_info = mybir.SyncInfo(on_wait=[w], on_update=[])
        nc.all_engine_barrier()
        assert self.sems is not None
        popped = nc._tile_sem_poison_stack.pop()
        assert popped is self._sem_poison
        nc.clear_and_free_semaphores(list(self.sems.allocated().values()))
        nc.all_engine_barrier()

    tile_mod.TileContext._drain_and_barrier = _patched
    tile_mod.TileContext._drain_patched = True


def build_program():
    import concourse.bass as bass
    import concourse.tile as tile
    from concourse import mybir
    from concourse.masks import make_identity

    _patch_drain(tile, mybir)

    F32 = mybir.dt.float32
    F32R = mybir.dt.float32r
    BF16 = mybir.dt.bfloat16
    I16 = mybir.dt.int16
    AF = mybir.ActivationFunctionType
    ALU = mybir.AluOpType

    NIC = N // 512          # 8 chunks of 512 tokens
    NJT = N // 128          # 32 key tiles of 128

    nc = bass.Bass(trn_type="TRN2", target_bir_lowering=False)

    tok = nc.dram_tensor("tok", [N, D], F32, kind="ExternalInput")
    wq = nc.dram_tensor("wq", [128, 4, 128], F32R, kind="ExternalInput")
    wk = nc.dram_tensor("wk", [128, 4, 128], F32R, kind="ExternalInput")
    wv = nc.dram_tensor("wv", [128, 4, 128], F32R, kind="ExternalInput")
    wo = nc.dram_tensor("wo", [128, 512], BF16, kind="ExternalInput")
    out_part = nc.dram_tensor("out_part", [N, D], F32, kind="ExternalOutput")

    tok_r = tok.rearrange("(ic t p) d -> ic p t d", t=4, p=128)
    out_r = out_part.rearrange("(ib t p) e -> ib p t e", t=4, p=128)

    # exp-engine assignment per j-tile PAIR within an iteration (A=ACT exact,
    # D=DVE Schraudolph), weighted so both engines finish together just
    # below the PE's per-iteration time. GPSIMD cannot read PSUM, so it
    # cannot join the exp crew; it takes phase-A SBUF work instead.
    NPAIR = NJT // 2
    W_A, W_D = 9, 7
    ENG = []
    cnt = {"A": 0, "D": 0}
    wgt = {"A": W_A, "D": W_D}
    for _ in range(NPAIR):
        e = min(cnt, key=lambda k: (cnt[k] + 1) / wgt[k])
        ENG.append(e)
        cnt[e] += 1

    with tile.TileContext(nc) as tc:
        with tc.tile_pool(name="consts", bufs=1) as consts, \
             tc.tile_pool(name="big", bufs=1) as big, \
             tc.tile_pool(name="wpool", bufs=1) as wpool:

            # ---- constants ----
            ident_f = consts.tile([128, 128], F32)
            make_identity(nc, ident_f)
            ident = consts.tile([128, 128], F32R)
            nc.vector.tensor_copy(ident, ident_f)
            identb = consts.tile([128, 128], BF16)
            nc.vector.tensor_copy(identb, ident_f)
            eps_t = consts.tile([128, 1], F32)
            nc.vector.memset(eps_t, EPS)
            # preload the Exp ACT table during the otherwise-idle start window
            warm = consts.tile([1, 1], F32)
            nc.scalar.activation(warm, eps_t[0:1, :], AF.Exp)

            # ---- weights ----
            # first token chunk goes ahead of the weights in the DMA queue so
            # the RMSNorm pipeline starts as early as possible
            tok4_first = wpool.tile([128, 4, 512], F32)
            for t in range(4):
                nc.sync.dma_start(out=tok4_first[:, t, :], in_=tok_r[0, :, t, :])
            wq_sb = wpool.tile([128, 4, 128], F32R)
            wk_sb = wpool.tile([128, 4, 128], F32R)
            wv_sb = wpool.tile([128, 4, 128], F32R)
            wo_sb = wpool.tile([128, 512], BF16)
            nc.sync.dma_start(out=wq_sb, in_=wq[:, :, :])
            nc.sync.dma_start(out=wk_sb, in_=wk[:, :, :])
            nc.sync.dma_start(out=wv_sb, in_=wv[:, :, :])
            nc.sync.dma_start(out=wo_sb, in_=wo[:, :])

            # ---- persistent big buffers ----
            QT = big.tile([128, N], F32R)       # [2 heads x 64 qdims, n]
            KT = big.tile([128, N], F32R)
            Vb = big.tile([128, NJT, 2, 65], BF16)  # per j-tile: [v(64)|ones] per head
            ones_bf = consts.tile([128, NJT, 2, 1], BF16)
            nc.vector.memset(ones_bf, 1.0)
            nc.vector.tensor_copy(Vb[:, :, :, 64:65], ones_bf)

            # ---- phase A/B: RMSNorm + x^T + QKV^T + V natural ----
            with tc.tile_pool(name="ab_sbuf", bufs=3) as abp, \
                 tc.tile_pool(name="ab_stats", bufs=4) as stp, \
                 tc.tile_pool(name="ab_psum", bufs=3, space="PSUM") as abps, \
                 tc.tile_pool(name="qk_psum", bufs=2, space="PSUM") as qkps, \
                 tc.tile_pool(name="scr_psum", bufs=1, space="PSUM") as scrps:

                # PE joins: absorb each weight-DMA semaphore with a tiny bf16 matmul
                scr = scrps.tile([2, 2], F32, tag="scr", name="scrj")
                BF = mybir.dt.bfloat16
                for i, wtile in enumerate((wq_sb, wk_sb, wv_sb)):
                    src = wtile[0:1, 0:1, 0:2].bitcast(BF)[:, 0, 1::2]
                    nc.tensor.matmul(scr, src, src, start=(i == 0), stop=False)
                nc.tensor.matmul(scr, wo_sb[0:1, 0:2], wo_sb[0:1, 0:2],
                                 start=False, stop=True)

                for ic in range(NIC):
                    if ic == 0:
                        tok4 = tok4_first
                    else:
                        tok4 = abp.tile([128, 4, 512], F32, tag="tok4")
                        nc.sync.dma_start(out=tok4, in_=tok_r[ic])
                    xt = abp.tile([128, 4, 512], F32R, tag="xt")
                    for t in range(4):
                        stats = stp.tile([128, 6], F32, tag="stats")
                        mv = stp.tile([128, 2], F32, tag="mv")
                        ms = stp.tile([128, 1], F32, tag="ms")
                        s_t = stp.tile([128, 1], F32, tag="s_t")
                        rstd = stp.tile([128, 1], F32, tag="rstd")
                        nc.vector.bn_stats(stats, tok4[:, t, :])
                        nc.vector.bn_aggr(mv, stats)
                        # E[x^2] = mean^2 + var
                        nc.vector.scalar_tensor_tensor(
                            ms, mv[:, 0:1], mv[:, 0:1], mv[:, 1:2],
                            op0=ALU.mult, op1=ALU.add)
                        nc.scalar.activation(s_t, ms, AF.Sqrt, bias=eps_t, scale=1.0)
                        nc.vector.reciprocal(rstd, s_t)
                        # normalization scale on GpSimd (SBUF-only engine)
                        xn = stp.tile([128, 512], F32R, tag="xn")
                        nc.gpsimd.tensor_scalar_mul(xn, in0=tok4[:, t, :], scalar1=rstd)
                        tp = abps.tile([128, 4, 128], F32R, tag="tp")
                        for c in range(4):
                            nc.tensor.transpose(tp[:, c, :], xn[:, c * 128:(c + 1) * 128], ident)
                        nc.scalar.copy(xt[:, :, t * 128:(t + 1) * 128], tp)

                    # QKV^T for this 512-token chunk (V first: DVE ordering)
                    vt = abp.tile([128, 512], BF16, tag="vt")
                    for wtile, dst in ((wv_sb, None), (wq_sb, QT), (wk_sb, KT)):
                        ps = qkps.tile([128, 512], F32, tag="qk")
                        for c in range(4):
                            nc.tensor.matmul(ps, wtile[:, c, :], xt[:, c, :],
                                             start=(c == 0), stop=(c == 3))
                        if dst is None:
                            nc.scalar.copy(vt, ps)
                        else:
                            nc.vector.tensor_copy(dst[:, ic * 512:(ic + 1) * 512], ps)
                    # V^T -> V natural (bf16) into Vb (j on partitions)
                    vtp = abps.tile([128, 4, 128], BF16, tag="tp")
                    for jl in range(4):
                        nc.tensor.transpose(vtp[:, jl, :], vt[:, jl * 128:(jl + 1) * 128], identb)
                    nc.vector.tensor_copy(
                        Vb[:, ic * 4:(ic + 1) * 4, :, 0:64],
                        vtp.rearrange("p jl (h v) -> p jl h v", h=2))

            # ---- phase C: attention + out-proj (j-tile pairs) ----
            with tc.tile_pool(name="c_sbuf", bufs=3) as cp, \
                 tc.tile_pool(name="pt_pool", bufs=4) as ptp, \
                 tc.tile_pool(name="opair_pool", bufs=2) as oprp, \
                 tc.tile_pool(name="outsb_pool", bufs=2) as outsbp, \
                 tc.tile_pool(name="st_psum", bufs=3, space="PSUM") as stps, \
                 tc.tile_pool(name="o_psum", bufs=2, space="PSUM") as ops:

                iters = [(ib, hl) for ib in range(NIC) for hl in range(2)]
                NIT = len(iters)
                LOOK = 2  # S^T pairs emitted ahead of exp/PV

                def emit_st(k):
                    """S^T for j-tile pair p of iteration k//NPAIR."""
                    it_idx, p = divmod(k, NPAIR)
                    ib, hl = iters[it_idx]
                    h0 = hl * 64
                    st = stps.tile([128, 2, 512], F32, tag="st", name="stg")
                    for jl in range(2):
                        jt = 2 * p + jl
                        nc.tensor.matmul(
                            st[:, jl, :],
                            KT[h0:h0 + 64, jt * 128:(jt + 1) * 128],
                            QT[h0:h0 + 64, ib * 512:(ib + 1) * 512],
                            start=True, stop=True)
                    return st

                def emit_exp(st, p):
                    pt = ptp.tile([128, 2, 512], BF16, tag="pt", name="ptg")
                    e = ENG[p]
                    if e == "A":
                        nc.scalar.activation(
                            pt.rearrange("a b c -> a (b c)"),
                            st.rearrange("a b c -> a (b c)"), AF.Exp)
                    else:
                        nc.vector.tensor_scalar(
                            pt.bitcast(I16).rearrange("a b c -> a (b c)"),
                            st.rearrange("a b c -> a (b c)"), C1B, C2B,
                            op0=ALU.mult, op1=ALU.add)
                    return pt

                # tail state
                o_accs = {}      # iter idx -> o_acc psum tile
                o_pairs = {}     # ib -> o_pair sbuf tile
                linvs = {}       # iter idx -> linv tile
                out_sbs = {}     # ib -> out_sb tile
                ot_tiles = {}    # ib -> (ot psum tile, ots sbuf tile)
                op_tiles = {}    # it -> out-proj psum tile (transient per ib)

                def emit_tail_step(k_prev, g):
                    """Interleave iter k_prev's tail into iter k_prev+1's
                    pair-step stream at slot g (16 slots per iteration)."""
                    ib, hl = iters[k_prev]
                    if g == 0:
                        o_acc = o_accs[k_prev]
                        linv = cp.tile([128, 4], F32, tag="linv", name=f"lv{hl}")
                        nc.vector.reciprocal(linv, o_acc[:, :, 64])
                        linvs[k_prev] = linv
                        if hl == 0:
                            o_pairs[ib] = oprp.tile(
                                [128, 4, 128], BF16, tag="opair", name=f"op{ib}")
                    elif 1 <= g <= 4:
                        it = g - 1
                        o_acc = o_accs[k_prev]
                        nc.vector.tensor_scalar_mul(
                            o_pairs[ib][:, it, hl * 64:(hl + 1) * 64],
                            in0=o_acc[:, it, 0:64],
                            scalar1=linvs[k_prev][:, it:it + 1])
                        if it == 3:
                            o_accs.pop(k_prev)
                            linvs.pop(k_prev)
                    elif hl == 1 and g == 5:
                        # transpose the 4 normalized [i,128] pair-tiles into a
                        # single psum bank (start only zeroes on the first);
                        # back-to-back to keep the ring slot held briefly
                        opr = o_pairs[ib]
                        ot = stps.tile([128, 4, 128], BF16, tag="st", name="otp")
                        ot_tiles[ib] = [ot, None]
                        for it in range(4):
                            nc.tensor.matmul(
                                ot[:, it, :], opr[:, it, :], identb,
                                is_transpose=True, start=(it == 0), stop=(it == 3),
                                skip_group_check=True)
                    elif hl == 1 and g == 6:
                        ots = cp.tile([128, 4, 128], BF16, tag="ots", name="otsb")
                        nc.vector.tensor_copy(ots, ot_tiles[ib][0])
                        ot_tiles[ib][1] = ots
                        o_pairs.pop(ib)
                    elif hl == 1 and 7 <= g <= 11:
                        # g=7..10: out-proj matmul it=g-7; g=8..11: drain it=g-8
                        if g >= 8:
                            it_d = g - 8
                            nc.scalar.copy(out_sbs[ib][:, it_d, :],
                                           op_tiles.pop(it_d))
                            if it_d == 3:
                                nc.sync.dma_start(out=out_r[ib],
                                                  in_=out_sbs.pop(ib))
                                ot_tiles.pop(ib)
                        if g <= 10:
                            it = g - 7
                            if it == 0:
                                out_sbs[ib] = outsbp.tile(
                                    [128, 4, 512], F32, tag="outsb", name=f"ob{ib}")
                            op_ps = stps.tile([128, 512], F32, tag="st", name="opps")
                            nc.tensor.matmul(op_ps, ot_tiles[ib][1][:, it, :],
                                             wo_sb, start=True, stop=True)
                            op_tiles[it] = op_ps

                def emit_pv(it_idx, p, pt):
                    _, hl = iters[it_idx]
                    if p == 0:
                        o_accs[it_idx] = ops.tile(
                            [128, 4, 65], F32, tag="o", name=f"oacc{it_idx % 2}")
                    o_acc = o_accs[it_idx]
                    for jl in range(2):
                        jt = 2 * p + jl
                        for it in range(4):
                            # PSUM `start` zeroes the WHOLE bank: only the
                            # very first matmul of this o_acc bank may set it
                            nc.tensor.matmul(
                                o_acc[:, it, :],
                                pt[:, jl, it * 128:(it + 1) * 128],
                                Vb[:, jt, hl, :],
                                start=(jt == 0 and it == 0),
                                stop=(jt == NJT - 1 and it == 3),
                                skip_group_check=True)

                # ---- attention pipeline over (iteration, j-tile pair) steps:
                # PV lags exp by one step so every exp has an extra pair-step
                # of slack before the PE consumes its output ----
                TOT = NIT * NPAIR
                sts = {}
                pts = {}
                for k in range(LOOK):
                    sts[k] = emit_st(k)
                for k in range(TOT + 1):
                    if k < TOT:
                        it_idx, p = divmod(k, NPAIR)
                        pts[k] = emit_exp(sts.pop(k), p)
                        if k + LOOK < TOT:
                            sts[k + LOOK] = emit_st(k + LOOK)
                    if k >= 1:
                        emit_pv(*divmod(k - 1, NPAIR), pts.pop(k - 1))
                    if k < TOT and it_idx > 0:
                        # after the lagged PV: at p==0 the PV above was the
                        # previous iteration's last accumulation, which the
                        # tail's linv must observe
                        emit_tail_step(it_idx - 1, p)

                # trailing tail for the final iteration
                for g in range(NPAIR):
                    emit_tail_step(NIT - 1, g)

    fix_waits_nc(nc, mybir)
    return nc


def fix_waits_nc(nc, mybir):
    """Post-pass over the scheduled program: (1) remove semaphore waits that
    are transitively implied by earlier waits (Tile emits per-proc-minimal,
    not transitively-minimal, waits), (2) split any instruction still
    carrying more than one wait by injecting single-wait NoOps in front of
    it — this walrus build rejects >1 sync wait per instruction.
    Mutates nc in place so CoreSim and hardware run identical sync."""
    nop_id = [0]

    def _is_ge(w):
        return w.sync_type == "semaphore" and w.wait_mode == "sem-ge-imm"

    for fn in nc.m.functions:
        for blk in fn.blocks:
            insts = list(blk.instructions)
            n = len(insts)

            producers = {}
            cum = {}
            nonmono = set()  # sems ever decremented: counter logic invalid
            for idx, inst in enumerate(insts):
                si = inst.sync_info
                for u in (si.on_update if si else []) or []:
                    if u.sync_type != "semaphore":
                        continue
                    sid = u.id
                    if u.update_mode != "sem-inc":
                        nonmono.add(sid)
                        continue
                    cum[sid] = cum.get(sid, 0) + int(u.update_value)
                    producers.setdefault(sid, []).append((cum[sid], idx))

            def producer_of(sid, val):
                for cv, idx in producers.get(sid, ()):
                    if cv >= val:
                        return idx
                return None

            prev_eng = [None] * n
            last = {}
            for idx, inst in enumerate(insts):
                e = inst.engine
                prev_eng[idx] = last.get(e)
                last[e] = idx

            def get_waits(inst):
                si = inst.sync_info
                return list(si.on_wait) if si and si.on_wait else []

            def is_ge(w):
                return _is_ge(w) and w.id not in nonmono

            know = [dict() for _ in range(n)]
            for _ in range(3):
                changed = False
                for idx, inst in enumerate(insts):
                    k = dict(know[prev_eng[idx]]) if prev_eng[idx] is not None else {}
                    for w in get_waits(inst):
                        if not is_ge(w):
                            continue
                        sid, val = w.id, int(w.wait_value)
                        if k.get(sid, -1) < val:
                            k[sid] = val
                        p = producer_of(sid, val)
                        if p is not None:
                            for s2, v2 in know[p].items():
                                if k.get(s2, -1) < v2:
                                    k[s2] = v2
                    if k != know[idx]:
                        know[idx] = k
                        changed = True
                if not changed:
                    break

            new_insts = []
            dirty = False
            for idx, inst in enumerate(insts):
                si = inst.sync_info
                waits = get_waits(inst)
                if si is not None and waits:
                    base = dict(know[prev_eng[idx]]) if prev_eng[idx] is not None else {}
                    kept = []
                    for w in waits:
                        if is_ge(w):
                            sid, val = w.id, int(w.wait_value)
                            if base.get(sid, -1) >= val:
                                continue
                            base[sid] = val
                            p = producer_of(sid, val)
                            if p is not None:
                                for s2, v2 in know[p].items():
                                    if base.get(s2, -1) < v2:
                                        base[s2] = v2
                        kept.append(w)
                    if len(kept) != len(waits) or len(kept) > 1:
                        dirty = True
                        for w in kept[:-1]:
                            nop_id[0] += 1
                            nop = mybir.InstNoOp(
                                name=f"I-waitfix-{nop_id[0]}", ins=[], outs=[])
                            nop.engine = inst.engine
                            nop.sync_info = mybir.SyncInfo(on_wait=[w], on_update=[])
                            nc.register_instruction(nop)
                            new_insts.append(nop)
                        inst.sync_info = mybir.SyncInfo(
                            on_wait=kept[-1:],
                            on_update=list(si.on_update or []))
                new_insts.append(inst)
            if dirty:
                blk.instructions = new_insts


def get_program():
    if "nc" not in _prog_cache:
        _prog_cache["nc"] = build_program()
    return _prog_cache["nc"]


def _prep_inputs(tokens, norm_weight, w_qkv, w_out):
    import ml_dtypes
    tokens = np.ascontiguousarray(np.asarray(tokens, dtype=np.float32))
    norm_weight = np.asarray(norm_weight, dtype=np.float32)
    w_qkv = np.asarray(w_qkv, dtype=np.float32)
    w_out = np.asarray(w_out, dtype=np.float32)

    wp = w_qkv * norm_weight[:, None]  # fold RMSNorm weight into qkv weights

    in_maps = []
    for c in range(NCORES):
        b = c // 4
        h0 = 2 * (c % 4)
        m = {}
        m["tok"] = tokens[b]
        for name, off in (("wq", 0), ("wk", DI), ("wv", 2 * DI)):
            w = wp[:, off + h0 * DH: off + (h0 + 2) * DH]       # [512, 128]
            m[name] = np.ascontiguousarray(
                w.reshape(4, 128, 128).transpose(1, 0, 2))       # [128, 4, 128]
        rows = w_out[h0 * DH:(h0 + 2) * DH, :]                   # [128, 512]
        m["wo"] = np.ascontiguousarray(rows.astype(ml_dtypes.bfloat16))
        in_maps.append(m)
    return in_maps


def run(tokens, norm_weight, w_qkv, w_out, trace=False):
    from concourse.bass_utils import run_bass_kernel_spmd
    nc = get_program()
    in_maps = _prep_inputs(tokens, norm_weight, w_qkv, w_out)
    res = run_bass_kernel_spmd(nc, in_maps, core_ids=list(range(NCORES)), trace=trace)
    parts = [res.results[c]["out_part"] for c in range(NCORES)]
    out = np.empty((B, N, D), dtype=np.float32)
    for b in range(B):
        out[b] = parts[4 * b] + parts[4 * b + 1] + parts[4 * b + 2] + parts[4 * b + 3]
    return out, res


def kernel(tokens, norm_weight, w_qkv, w_out):
    out, _ = run(tokens, norm_weight, w_qkv, w_out, trace=False)
    return out


# revision 31
# speedup vs baseline: 1.3289x; 1.0051x over previous
"""TRN2 Bass kernel for nn_Attention (RMSNorm + QKV + softmax attention + out-proj).

Sharding: 8 cores = 2 batches x 4 head-pairs. Core c handles batch c//4 and
heads (2*(c%4), 2*(c%4)+1). Each core computes its partial out-projection
(contracting only its 128 rows of dim_inner); host sums the 4 partials per batch.

V2 design (cost-model driven):
  A) stream tokens [128,512] blocks: RMSNorm stats on DVE, rstd scale, PE
     transpose -> x^T (rotating [128,4,512] per 512-token chunk)
  B) QKV^T = w^T @ x^T (PE); V^T -> bf16 -> transposed back to V-natural with
     a ones column appended per head (softmax denominator for free)
  C) flash attention per (ib=512 queries, head): for each j-tile of 128 keys:
     S^T [128j, 512i] on PE (f32r, 512 cycles); exp split across THREE engines
     (ACT: exact table exp -> bf16; DVE/GPSIMD: Schraudolph bit-trick exp via
     tensor_scalar -> int16 bits of bf16). PV uses exp(S)^T tiles as the
     STATIONARY [128j, 128i] and bf16 V[128j,64+1] as MOVING: 65-cycle
     matmuls accumulate O-natural [128i, 65] in PSUM (col 64 = l).
  D) tail: linv=1/l (DVE), scale O by linv -> bf16 pair tile [i, 128dh both
     heads] (DVE), PE transpose [128,128], one 128-contraction out-proj matmul
     per i-tile vs bf16 w_out, ACT drains psum->sbuf, DMA out per ib.

Engine budget per core (cost model): PE ~206us (bottleneck), ACT ~150us,
DVE ~180us, Pool ~110us. exp assignment per j-tile keeps all three exp
engines concurrently busy so PE never starves.

Schraudolph exp: bf16_bits(p) = int16(S * (2^7/ln2) + C2B). Bias component
cancels through the softmax denominator; residual sawtooth ~1% std on the
offloaded fraction keeps total rel err well under the 2e-2 gate.

Engine discipline: walrus here allows only ONE semaphore wait per instruction
(fix_waits_nc post-pass splits/dedups), and no mixed 32/16-bit matmuls.
"""
import sys
sys.path.insert(0, "/opt/trn_rl_repo")
import numpy as np

B, N, D = 2, 4096, 512
H, DH = 8, 64
DI = H * DH
NCORES = 8
EPS = 1.1920929e-07  # float32 eps (torch nn.RMSNorm default)

# Schraudolph constants for bf16-bit exp: bits16(e^x) ~= x*C1B + C2B
C1B = 128.0 / 0.6931471805599453          # 2^7 / ln 2
C2B = float(127 * 128) - 366393.0 / 65536.0

_prog_cache = {}


def _patch_drain(tile_mod, mybir):
    """Split the multi-wait tail drain into a chain of single-wait drains
    (this walrus build rejects >1 sync wait per instruction)."""
    if getattr(tile_mod.TileContext, "_drain_patched", False):
        return

    def _patched(self, tick_clock, wait_clock):
        from concourse.vector_clock import ScopedClock
        nc = self.nc
        drain_inst = nc.sync.drain()
        wait_clock.add_sem_waits(drain_inst.ins, ScopedClock({None: tick_clock.global_clock}))
        si = drain_inst.ins.sync_info
        if si is not None and si.on_wait and len(si.on_wait) > 1:
            waits = list(si.on_wait)
            drain_inst.ins.sync_info = mybir.SyncInfo(
                on_wait=waits[:1], on_update=list(si.on_update or []))
            for w in waits[1:]:
                d2 = nc.sync.drain()
                d2.ins.sync_info = mybir.SyncInfo(on_wait=[w], on_update=[])
        nc.all_engine_barrier()
        assert self.sems is not None
        popped = nc._tile_sem_poison_stack.pop()
        assert popped is self._sem_poison
        nc.clear_and_free_semaphores(list(self.sems.allocated().values()))
        nc.all_engine_barrier()

    tile_mod.TileContext._drain_and_barrier = _patched
    tile_mod.TileContext._drain_patched = True


def build_program():
    import concourse.bass as bass
    import concourse.tile as tile
    from concourse import mybir
    from concourse.masks import make_identity

    _patch_drain(tile, mybir)

    F32 = mybir.dt.float32
    F32R = mybir.dt.float32r
    BF16 = mybir.dt.bfloat16
    I16 = mybir.dt.int16
    AF = mybir.ActivationFunctionType
    ALU = mybir.AluOpType

    NIC = N // 512          # 8 chunks of 512 tokens
    NJT = N // 128          # 32 key tiles of 128

    nc = bass.Bass(trn_type="TRN2", target_bir_lowering=False)

    tok = nc.dram_tensor("tok", [N, D], F32, kind="ExternalInput")
    wq = nc.dram_tensor("wq", [128, 4, 128], F32R, kind="ExternalInput")
    wk = nc.dram_tensor("wk", [128, 4, 128], F32R, kind="ExternalInput")
    wv = nc.dram_tensor("wv", [128, 4, 128], F32R, kind="ExternalInput")
    wo = nc.dram_tensor("wo", [128, 512], BF16, kind="ExternalInput")
    out_part = nc.dram_tensor("out_part", [N, D], F32, kind="ExternalOutput")

    tok_r = tok.rearrange("(ic t p) d -> ic p t d", t=4, p=128)
    out_r = out_part.rearrange("(ib t p) e -> ib p t e", t=4, p=128)

    # exp-engine assignment per j-tile PAIR within an iteration (A=ACT exact,
    # D=DVE Schraudolph), weighted so both engines finish together just
    # below the PE's per-iteration time. GPSIMD cannot read PSUM, so it
    # cannot join the exp crew; it takes phase-A SBUF work instead.
    NPAIR = NJT // 2
    W_A, W_D = 9, 7
    ENG = []
    cnt = {"A": 0, "D": 0}
    wgt = {"A": W_A, "D": W_D}
    for _ in range(NPAIR):
        e = min(cnt, key=lambda k: (cnt[k] + 1) / wgt[k])
        ENG.append(e)
        cnt[e] += 1

    with tile.TileContext(nc) as tc:
        with tc.tile_pool(name="consts", bufs=1) as consts, \
             tc.tile_pool(name="big", bufs=1) as big, \
             tc.tile_pool(name="wpool", bufs=1) as wpool:

            # ---- constants ----
            ident_f = consts.tile([128, 128], F32)
            make_identity(nc, ident_f)
            ident = consts.tile([128, 128], F32R)
            nc.vector.tensor_copy(ident, ident_f)
            identb = consts.tile([128, 128], BF16)
            nc.vector.tensor_copy(identb, ident_f)
            eps_t = consts.tile([128, 1], F32)
            nc.vector.memset(eps_t, EPS)
            # preload the Exp ACT table during the otherwise-idle start window
            warm = consts.tile([1, 1], F32)
            nc.scalar.activation(warm, eps_t[0:1, :], AF.Exp)

            # ---- weights ----
            # first token chunk goes ahead of the weights in the DMA queue so
            # the RMSNorm pipeline starts as early as possible
            tok4_first = wpool.tile([128, 4, 512], F32)
            for t in range(4):
                nc.sync.dma_start(out=tok4_first[:, t, :], in_=tok_r[0, :, t, :])
            wq_sb = wpool.tile([128, 4, 128], F32R)
            wk_sb = wpool.tile([128, 4, 128], F32R)
            wv_sb = wpool.tile([128, 4, 128], F32R)
            wo_sb = wpool.tile([128, 512], BF16)
            nc.sync.dma_start(out=wq_sb, in_=wq[:, :, :])
            nc.sync.dma_start(out=wk_sb, in_=wk[:, :, :])
            nc.sync.dma_start(out=wv_sb, in_=wv[:, :, :])
            nc.sync.dma_start(out=wo_sb, in_=wo[:, :])

            # ---- persistent big buffers ----
            QT = big.tile([128, N], F32R)       # [2 heads x 64 qdims, n]
            KT = big.tile([128, N], F32R)
            Vb = big.tile([128, NJT, 2, 65], BF16)  # per j-tile: [v(64)|ones] per head
            ones_bf = consts.tile([128, NJT, 2, 1], BF16)
            nc.vector.memset(ones_bf, 1.0)
            nc.vector.tensor_copy(Vb[:, :, :, 64:65], ones_bf)

            # ---- phase A/B: RMSNorm + x^T + QKV^T + V natural ----
            with tc.tile_pool(name="ab_sbuf", bufs=3) as abp, \
                 tc.tile_pool(name="ab_stats", bufs=4) as stp, \
                 tc.tile_pool(name="ab_psum", bufs=3, space="PSUM") as abps, \
                 tc.tile_pool(name="qk_psum", bufs=2, space="PSUM") as qkps, \
                 tc.tile_pool(name="scr_psum", bufs=1, space="PSUM") as scrps:

                # PE joins: absorb each weight-DMA semaphore with a tiny bf16 matmul
                scr = scrps.tile([2, 2], F32, tag="scr", name="scrj")
                BF = mybir.dt.bfloat16
                for i, wtile in enumerate((wq_sb, wk_sb, wv_sb)):
                    src = wtile[0:1, 0:1, 0:2].bitcast(BF)[:, 0, 1::2]
                    nc.tensor.matmul(scr, src, src, start=(i == 0), stop=False)
                nc.tensor.matmul(scr, wo_sb[0:1, 0:2], wo_sb[0:1, 0:2],
                                 start=False, stop=True)

                for ic in range(NIC):
                    if ic == 0:
                        tok4 = tok4_first
                    else:
                        tok4 = abp.tile([128, 4, 512], F32, tag="tok4")
                        nc.sync.dma_start(out=tok4, in_=tok_r[ic])
                    xt = abp.tile([128, 4, 512], F32R, tag="xt")
                    for t in range(4):
                        stats = stp.tile([128, 6], F32, tag="stats")
                        mv = stp.tile([128, 2], F32, tag="mv")
                        ms = stp.tile([128, 1], F32, tag="ms")
                        s_t = stp.tile([128, 1], F32, tag="s_t")
                        rstd = stp.tile([128, 1], F32, tag="rstd")
                        nc.vector.bn_stats(stats, tok4[:, t, :])
                        nc.vector.bn_aggr(mv, stats)
                        # E[x^2] = mean^2 + var
                        nc.vector.scalar_tensor_tensor(
                            ms, mv[:, 0:1], mv[:, 0:1], mv[:, 1:2],
                            op0=ALU.mult, op1=ALU.add)
                        nc.scalar.activation(s_t, ms, AF.Sqrt, bias=eps_t, scale=1.0)
                        nc.vector.reciprocal(rstd, s_t)
                        # alternate the normalization scale between DVE and
                        # GpSimd: DVE's copy is faster (shorter chain to the
                        # transpose) but it also drains QT/KT psum each chunk
                        xn = stp.tile([128, 512], F32R, tag="xn")
                        eng = nc.vector if t % 2 == 0 else nc.gpsimd
                        eng.tensor_scalar_mul(xn, in0=tok4[:, t, :], scalar1=rstd)
                        tp = abps.tile([128, 4, 128], F32R, tag="tp")
                        for c in range(4):
                            nc.tensor.transpose(tp[:, c, :], xn[:, c * 128:(c + 1) * 128], ident)
                        nc.scalar.copy(xt[:, :, t * 128:(t + 1) * 128], tp)

                    # QKV^T for this 512-token chunk (V first: DVE ordering)
                    vt = abp.tile([128, 512], BF16, tag="vt")
                    for wtile, dst in ((wv_sb, None), (wq_sb, QT), (wk_sb, KT)):
                        ps = qkps.tile([128, 512], F32, tag="qk")
                        for c in range(4):
                            nc.tensor.matmul(ps, wtile[:, c, :], xt[:, c, :],
                                             start=(c == 0), stop=(c == 3))
                        if dst is None:
                            nc.scalar.copy(vt, ps)
                        else:
                            nc.vector.tensor_copy(dst[:, ic * 512:(ic + 1) * 512], ps)
                    # V^T -> V natural (bf16) into Vb (j on partitions)
                    vtp = abps.tile([128, 4, 128], BF16, tag="tp")
                    for jl in range(4):
                        nc.tensor.transpose(vtp[:, jl, :], vt[:, jl * 128:(jl + 1) * 128], identb)
                    nc.vector.tensor_copy(
                        Vb[:, ic * 4:(ic + 1) * 4, :, 0:64],
                        vtp.rearrange("p jl (h v) -> p jl h v", h=2))

            # ---- phase C: attention + out-proj (j-tile pairs) ----
            with tc.tile_pool(name="c_sbuf", bufs=3) as cp, \
                 tc.tile_pool(name="pt_pool", bufs=4) as ptp, \
                 tc.tile_pool(name="opair_pool", bufs=2) as oprp, \
                 tc.tile_pool(name="outsb_pool", bufs=2) as outsbp, \
                 tc.tile_pool(name="st_psum", bufs=3, space="PSUM") as stps, \
                 tc.tile_pool(name="o_psum", bufs=2, space="PSUM") as ops:

                iters = [(ib, hl) for ib in range(NIC) for hl in range(2)]
                NIT = len(iters)
                LOOK = 2  # S^T pairs emitted ahead of exp/PV

                def emit_st(k):
                    """S^T for j-tile pair p of iteration k//NPAIR."""
                    it_idx, p = divmod(k, NPAIR)
                    ib, hl = iters[it_idx]
                    h0 = hl * 64
                    st = stps.tile([128, 2, 512], F32, tag="st", name="stg")
                    for jl in range(2):
                        jt = 2 * p + jl
                        nc.tensor.matmul(
                            st[:, jl, :],
                            KT[h0:h0 + 64, jt * 128:(jt + 1) * 128],
                            QT[h0:h0 + 64, ib * 512:(ib + 1) * 512],
                            start=True, stop=True)
                    return st

                def emit_exp(st, p):
                    pt = ptp.tile([128, 2, 512], BF16, tag="pt", name="ptg")
                    e = ENG[p]
                    if e == "A":
                        nc.scalar.activation(
                            pt.rearrange("a b c -> a (b c)"),
                            st.rearrange("a b c -> a (b c)"), AF.Exp)
                    else:
                        nc.vector.tensor_scalar(
                            pt.bitcast(I16).rearrange("a b c -> a (b c)"),
                            st.rearrange("a b c -> a (b c)"), C1B, C2B,
                            op0=ALU.mult, op1=ALU.add)
                    return pt

                # tail state
                o_accs = {}      # iter idx -> o_acc psum tile
                o_pairs = {}     # ib -> o_pair sbuf tile
                linvs = {}       # iter idx -> linv tile
                out_sbs = {}     # ib -> out_sb tile
                ot_tiles = {}    # ib -> (ot psum tile, ots sbuf tile)
                op_tiles = {}    # it -> out-proj psum tile (transient per ib)

                def emit_tail_step(k_prev, g):
                    """Interleave iter k_prev's tail into iter k_prev+1's
                    pair-step stream at slot g (16 slots per iteration)."""
                    ib, hl = iters[k_prev]
                    if g == 0:
                        o_acc = o_accs[k_prev]
                        linv = cp.tile([128, 4], F32, tag="linv", name=f"lv{hl}")
                        nc.vector.reciprocal(linv, o_acc[:, :, 64])
                        linvs[k_prev] = linv
                        if hl == 0:
                            o_pairs[ib] = oprp.tile(
                                [128, 4, 128], BF16, tag="opair", name=f"op{ib}")
                    elif 1 <= g <= 4:
                        it = g - 1
                        o_acc = o_accs[k_prev]
                        nc.vector.tensor_scalar_mul(
                            o_pairs[ib][:, it, hl * 64:(hl + 1) * 64],
                            in0=o_acc[:, it, 0:64],
                            scalar1=linvs[k_prev][:, it:it + 1])
                        if it == 3:
                            o_accs.pop(k_prev)
                            linvs.pop(k_prev)
                    elif hl == 1 and g == 5:
                        # transpose the 4 normalized [i,128] pair-tiles into a
                        # single psum bank (start only zeroes on the first);
                        # back-to-back to keep the ring slot held briefly
                        opr = o_pairs[ib]
                        ot = stps.tile([128, 4, 128], BF16, tag="st", name="otp")
                        ot_tiles[ib] = [ot, None]
                        for it in range(4):
                            nc.tensor.matmul(
                                ot[:, it, :], opr[:, it, :], identb,
                                is_transpose=True, start=(it == 0), stop=(it == 3),
                                skip_group_check=True)
                    elif hl == 1 and g == 6:
                        ots = cp.tile([128, 4, 128], BF16, tag="ots", name="otsb")
                        nc.vector.tensor_copy(ots, ot_tiles[ib][0])
                        ot_tiles[ib][1] = ots
                        o_pairs.pop(ib)
                    elif hl == 1 and 7 <= g <= 11:
                        # g=7..10: out-proj matmul it=g-7; g=8..11: drain it=g-8
                        if g >= 8:
                            it_d = g - 8
                            nc.scalar.copy(out_sbs[ib][:, it_d, :],
                                           op_tiles.pop(it_d))
                            # per-it DMA so the store overlaps later drains
                            nc.sync.dma_start(out=out_r[ib, :, it_d, :],
                                              in_=out_sbs[ib][:, it_d, :])
                            if it_d == 3:
                                out_sbs.pop(ib)
                                ot_tiles.pop(ib)
                        if g <= 10:
                            it = g - 7
                            if it == 0:
                                out_sbs[ib] = outsbp.tile(
                                    [128, 4, 512], F32, tag="outsb", name=f"ob{ib}")
                            op_ps = stps.tile([128, 512], F32, tag="st", name="opps")
                            nc.tensor.matmul(op_ps, ot_tiles[ib][1][:, it, :],
                                             wo_sb, start=True, stop=True)
                            op_tiles[it] = op_ps

                def emit_pv(it_idx, p, pt):
                    _, hl = iters[it_idx]
                    if p == 0:
                        o_accs[it_idx] = ops.tile(
                            [128, 4, 65], F32, tag="o", name=f"oacc{it_idx % 2}")
                    o_acc = o_accs[it_idx]
                    for jl in range(2):
                        jt = 2 * p + jl
                        for it in range(4):
                            # PSUM `start` zeroes the WHOLE bank: only the
                            # very first matmul of this o_acc bank may set it
                            nc.tensor.matmul(
                                o_acc[:, it, :],
                                pt[:, jl, it * 128:(it + 1) * 128],
                                Vb[:, jt, hl, :],
                                start=(jt == 0 and it == 0),
                                stop=(jt == NJT - 1 and it == 3),
                                skip_group_check=True)

                # ---- attention pipeline over (iteration, j-tile pair) steps:
                # PV lags exp by one step so every exp has an extra pair-step
                # of slack before the PE consumes its output ----
                TOT = NIT * NPAIR
                sts = {}
                pts = {}
                for k in range(LOOK):
                    sts[k] = emit_st(k)
                for k in range(TOT + 1):
                    if k < TOT:
                        it_idx, p = divmod(k, NPAIR)
                        pts[k] = emit_exp(sts.pop(k), p)
                        if k + LOOK < TOT:
                            sts[k + LOOK] = emit_st(k + LOOK)
                    if k >= 1:
                        emit_pv(*divmod(k - 1, NPAIR), pts.pop(k - 1))
                    if k < TOT and it_idx > 0:
                        # after the lagged PV: at p==0 the PV above was the
                        # previous iteration's last accumulation, which the
                        # tail's linv must observe
                        emit_tail_step(it_idx - 1, p)

                # trailing tail for the final iteration
                for g in range(NPAIR):
                    emit_tail_step(NIT - 1, g)

    fix_waits_nc(nc, mybir)
    return nc


def fix_waits_nc(nc, mybir):
    """Post-pass over the scheduled program: (1) remove semaphore waits that
    are transitively implied by earlier waits (Tile emits per-proc-minimal,
    not transitively-minimal, waits), (2) split any instruction still
    carrying more than one wait by injecting single-wait NoOps in front of
    it — this walrus build rejects >1 sync wait per instruction.
    Mutates nc in place so CoreSim and hardware run identical sync."""
    nop_id = [0]

    def _is_ge(w):
        return w.sync_type == "semaphore" and w.wait_mode == "sem-ge-imm"

    for fn in nc.m.functions:
        for blk in fn.blocks:
            insts = list(blk.instructions)
            n = len(insts)

            producers = {}
            cum = {}
            nonmono = set()  # sems ever decremented: counter logic invalid
            for idx, inst in enumerate(insts):
                si = inst.sync_info
                for u in (si.on_update if si else []) or []:
                    if u.sync_type != "semaphore":
                        continue
                    sid = u.id
                    if u.update_mode != "sem-inc":
                        nonmono.add(sid)
                        continue
                    cum[sid] = cum.get(sid, 0) + int(u.update_value)
                    producers.setdefault(sid, []).append((cum[sid], idx))

            def producer_of(sid, val):
                for cv, idx in producers.get(sid, ()):
                    if cv >= val:
                        return idx
                return None

            prev_eng = [None] * n
            last = {}
            for idx, inst in enumerate(insts):
                e = inst.engine
                prev_eng[idx] = last.get(e)
                last[e] = idx

            def get_waits(inst):
                si = inst.sync_info
                return list(si.on_wait) if si and si.on_wait else []

            def is_ge(w):
                return _is_ge(w) and w.id not in nonmono

            know = [dict() for _ in range(n)]
            for _ in range(3):
                changed = False
                for idx, inst in enumerate(insts):
                    k = dict(know[prev_eng[idx]]) if prev_eng[idx] is not None else {}
                    for w in get_waits(inst):
                        if not is_ge(w):
                            continue
                        sid, val = w.id, int(w.wait_value)
                        if k.get(sid, -1) < val:
                            k[sid] = val
                        p = producer_of(sid, val)
                        if p is not None:
                            for s2, v2 in know[p].items():
                                if k.get(s2, -1) < v2:
                                    k[s2] = v2
                    if k != know[idx]:
                        know[idx] = k
                        changed = True
                if not changed:
                    break

            new_insts = []
            dirty = False
            for idx, inst in enumerate(insts):
                si = inst.sync_info
                waits = get_waits(inst)
                if si is not None and waits:
                    base = dict(know[prev_eng[idx]]) if prev_eng[idx] is not None else {}
                    kept = []
                    for w in waits:
                        if is_ge(w):
                            sid, val = w.id, int(w.wait_value)
                            if base.get(sid, -1) >= val:
                                continue
                            base[sid] = val
                            p = producer_of(sid, val)
                            if p is not None:
                                for s2, v2 in know[p].items():
                                    if base.get(s2, -1) < v2:
                                        base[s2] = v2
                        kept.append(w)
                    if len(kept) != len(waits) or len(kept) > 1:
                        dirty = True
                        for w in kept[:-1]:
                            nop_id[0] += 1
                            nop = mybir.InstNoOp(
                                name=f"I-waitfix-{nop_id[0]}", ins=[], outs=[])
                            nop.engine = inst.engine
                            nop.sync_info = mybir.SyncInfo(on_wait=[w], on_update=[])
                            nc.register_instruction(nop)
                            new_insts.append(nop)
                        inst.sync_info = mybir.SyncInfo(
                            on_wait=kept[-1:],
                            on_update=list(si.on_update or []))
                new_insts.append(inst)
            if dirty:
                blk.instructions = new_insts


def get_program():
    if "nc" not in _prog_cache:
        _prog_cache["nc"] = build_program()
    return _prog_cache["nc"]


def _prep_inputs(tokens, norm_weight, w_qkv, w_out):
    import ml_dtypes
    tokens = np.ascontiguousarray(np.asarray(tokens, dtype=np.float32))
    norm_weight = np.asarray(norm_weight, dtype=np.float32)
    w_qkv = np.asarray(w_qkv, dtype=np.float32)
    w_out = np.asarray(w_out, dtype=np.float32)

    wp = w_qkv * norm_weight[:, None]  # fold RMSNorm weight into qkv weights

    in_maps = []
    for c in range(NCORES):
        b = c // 4
        h0 = 2 * (c % 4)
        m = {}
        m["tok"] = tokens[b]
        for name, off in (("wq", 0), ("wk", DI), ("wv", 2 * DI)):
            w = wp[:, off + h0 * DH: off + (h0 + 2) * DH]       # [512, 128]
            m[name] = np.ascontiguousarray(
                w.reshape(4, 128, 128).transpose(1, 0, 2))       # [128, 4, 128]
        rows = w_out[h0 * DH:(h0 + 2) * DH, :]                   # [128, 512]
        m["wo"] = np.ascontiguousarray(rows.astype(ml_dtypes.bfloat16))
        in_maps.append(m)
    return in_maps


def run(tokens, norm_weight, w_qkv, w_out, trace=False):
    from concourse.bass_utils import run_bass_kernel_spmd
    nc = get_program()
    in_maps = _prep_inputs(tokens, norm_weight, w_qkv, w_out)
    res = run_bass_kernel_spmd(nc, in_maps, core_ids=list(range(NCORES)), trace=trace)
    parts = [res.results[c]["out_part"] for c in range(NCORES)]
    out = np.empty((B, N, D), dtype=np.float32)
    for b in range(B):
        out[b] = parts[4 * b] + parts[4 * b + 1] + parts[4 * b + 2] + parts[4 * b + 3]
    return out, res


def kernel(tokens, norm_weight, w_qkv, w_out):
    out, _ = run(tokens, norm_weight, w_qkv, w_out, trace=False)
    return out


# revision 34
# speedup vs baseline: 1.3423x; 1.0101x over previous
"""TRN2 Bass kernel for nn_Attention (RMSNorm + QKV + softmax attention + out-proj).

Sharding: 8 cores = 2 batches x 4 head-pairs. Core c handles batch c//4 and
heads (2*(c%4), 2*(c%4)+1). Each core computes its partial out-projection
(contracting only its 128 rows of dim_inner); host sums the 4 partials per batch.

V2 design (cost-model driven):
  A) stream tokens [128,512] blocks: RMSNorm stats on DVE, rstd scale, PE
     transpose -> x^T (rotating [128,4,512] per 512-token chunk)
  B) QKV^T = w^T @ x^T (PE); V^T -> bf16 -> transposed back to V-natural with
     a ones column appended per head (softmax denominator for free)
  C) flash attention per (ib=512 queries, head): for each j-tile of 128 keys:
     S^T [128j, 512i] on PE (f32r, 512 cycles); exp split across THREE engines
     (ACT: exact table exp -> bf16; DVE/GPSIMD: Schraudolph bit-trick exp via
     tensor_scalar -> int16 bits of bf16). PV uses exp(S)^T tiles as the
     STATIONARY [128j, 128i] and bf16 V[128j,64+1] as MOVING: 65-cycle
     matmuls accumulate O-natural [128i, 65] in PSUM (col 64 = l).
  D) tail: linv=1/l (DVE), scale O by linv -> bf16 pair tile [i, 128dh both
     heads] (DVE), PE transpose [128,128], one 128-contraction out-proj matmul
     per i-tile vs bf16 w_out, ACT drains psum->sbuf, DMA out per ib.

Engine budget per core (cost model): PE ~206us (bottleneck), ACT ~150us,
DVE ~180us, Pool ~110us. exp assignment per j-tile keeps all three exp
engines concurrently busy so PE never starves.

Schraudolph exp: bf16_bits(p) = int16(S * (2^7/ln2) + C2B). Bias component
cancels through the softmax denominator; residual sawtooth ~1% std on the
offloaded fraction keeps total rel err well under the 2e-2 gate.

Engine discipline: walrus here allows only ONE semaphore wait per instruction
(fix_waits_nc post-pass splits/dedups), and no mixed 32/16-bit matmuls.
"""
import sys
sys.path.insert(0, "/opt/trn_rl_repo")
import numpy as np

B, N, D = 2, 4096, 512
H, DH = 8, 64
DI = H * DH
NCORES = 8
EPS = 1.1920929e-07  # float32 eps (torch nn.RMSNorm default)

# Schraudolph constants for bf16-bit exp: bits16(e^x) ~= x*C1B + C2B
C1B = 128.0 / 0.6931471805599453          # 2^7 / ln 2
C2B = float(127 * 128) - 366393.0 / 65536.0

_prog_cache = {}


def _patch_drain(tile_mod, mybir):
    """Split the multi-wait tail drain into a chain of single-wait drains
    (this walrus build rejects >1 sync wait per instruction)."""
    if getattr(tile_mod.TileContext, "_drain_patched", False):
        return

    def _patched(self, tick_clock, wait_clock):
        from concourse.vector_clock import ScopedClock
        nc = self.nc
        drain_inst = nc.sync.drain()
        wait_clock.add_sem_waits(drain_inst.ins, ScopedClock({None: tick_clock.global_clock}))
        si = drain_inst.ins.sync_info
        if si is not None and si.on_wait and len(si.on_wait) > 1:
            waits = list(si.on_wait)
            drain_inst.ins.sync_info = mybir.SyncInfo(
                on_wait=waits[:1], on_update=list(si.on_update or []))
            for w in waits[1:]:
                d2 = nc.sync.drain()
                d2.ins.sync_info = mybir.SyncInfo(on_wait=[w], on_update=[])
        nc.all_engine_barrier()
        assert self.sems is not None
        popped = nc._tile_sem_poison_stack.pop()
        assert popped is self._sem_poison
        nc.clear_and_free_semaphores(list(self.sems.allocated().values()))
        nc.all_engine_barrier()

    tile_mod.TileContext._drain_and_barrier = _patched
    tile_mod.TileContext._drain_patched = True


def build_program():
    import concourse.bass as bass
    import concourse.tile as tile
    from concourse import mybir
    from concourse.masks import make_identity

    _patch_drain(tile, mybir)

    F32 = mybir.dt.float32
    F32R = mybir.dt.float32r
    BF16 = mybir.dt.bfloat16
    I16 = mybir.dt.int16
    AF = mybir.ActivationFunctionType
    ALU = mybir.AluOpType

    NIC = N // 512          # 8 chunks of 512 tokens
    NJT = N // 128          # 32 key tiles of 128

    nc = bass.Bass(trn_type="TRN2", target_bir_lowering=False)

    tok = nc.dram_tensor("tok", [N, D], F32, kind="ExternalInput")
    wq = nc.dram_tensor("wq", [128, 4, 128], F32R, kind="ExternalInput")
    wk = nc.dram_tensor("wk", [128, 4, 128], F32R, kind="ExternalInput")
    wv = nc.dram_tensor("wv", [128, 4, 128], F32R, kind="ExternalInput")
    wo = nc.dram_tensor("wo", [128, 512], BF16, kind="ExternalInput")
    out_part = nc.dram_tensor("out_part", [N, D], F32, kind="ExternalOutput")

    tok_r = tok.rearrange("(ic t p) d -> ic p t d", t=4, p=128)
    out_r = out_part.rearrange("(ib t p) e -> ib p t e", t=4, p=128)

    # exp-engine assignment per j-tile PAIR within an iteration (A=ACT exact,
    # D=DVE Schraudolph), weighted so both engines finish together just
    # below the PE's per-iteration time. GPSIMD cannot read PSUM, so it
    # cannot join the exp crew; it takes phase-A SBUF work instead.
    NPAIR = NJT // 2
    def _mk_eng(w_a, w_d):
        pat, cnt = [], {"A": 0, "D": 0}
        wgt = {"A": w_a, "D": w_d}
        for _ in range(NPAIR):
            e = min(cnt, key=lambda k: (cnt[k] + 1) / wgt[k])
            pat.append(e)
            cnt[e] += 1
        return pat
    # alternate the split between iterations so ACT (which also does the
    # out-proj drains) keeps headroom below the PE's per-iteration pace
    ENG_EVEN = _mk_eng(9, 7)
    ENG_ODD = _mk_eng(8, 8)

    with tile.TileContext(nc) as tc:
        with tc.tile_pool(name="consts", bufs=1) as consts, \
             tc.tile_pool(name="big", bufs=1) as big, \
             tc.tile_pool(name="wpool", bufs=1) as wpool:

            # ---- constants ----
            ident_f = consts.tile([128, 128], F32)
            make_identity(nc, ident_f)
            ident = consts.tile([128, 128], F32R)
            nc.vector.tensor_copy(ident, ident_f)
            identb = consts.tile([128, 128], BF16)
            nc.vector.tensor_copy(identb, ident_f)
            eps_t = consts.tile([128, 1], F32)
            nc.vector.memset(eps_t, EPS)
            # preload the Exp ACT table during the otherwise-idle start window
            warm = consts.tile([1, 1], F32)
            nc.scalar.activation(warm, eps_t[0:1, :], AF.Exp)

            # ---- weights ----
            # first token chunk goes ahead of the weights in the DMA queue so
            # the RMSNorm pipeline starts as early as possible
            tok4_first = wpool.tile([128, 4, 512], F32)
            for t in range(4):
                nc.sync.dma_start(out=tok4_first[:, t, :], in_=tok_r[0, :, t, :])
            wq_sb = wpool.tile([128, 4, 128], F32R)
            wk_sb = wpool.tile([128, 4, 128], F32R)
            wv_sb = wpool.tile([128, 4, 128], F32R)
            wo_sb = wpool.tile([128, 512], BF16)
            nc.sync.dma_start(out=wq_sb, in_=wq[:, :, :])
            nc.sync.dma_start(out=wk_sb, in_=wk[:, :, :])
            nc.sync.dma_start(out=wv_sb, in_=wv[:, :, :])
            nc.sync.dma_start(out=wo_sb, in_=wo[:, :])

            # ---- persistent big buffers ----
            QT = big.tile([128, N], F32R)       # [2 heads x 64 qdims, n]
            KT = big.tile([128, N], F32R)
            Vb = big.tile([128, NJT, 2, 65], BF16)  # per j-tile: [v(64)|ones] per head
            ones_bf = consts.tile([128, NJT, 2, 1], BF16)
            nc.vector.memset(ones_bf, 1.0)
            nc.vector.tensor_copy(Vb[:, :, :, 64:65], ones_bf)

            # ---- phase A/B: RMSNorm + x^T + QKV^T + V natural ----
            with tc.tile_pool(name="ab_sbuf", bufs=3) as abp, \
                 tc.tile_pool(name="ab_stats", bufs=8) as stp, \
                 tc.tile_pool(name="ab_psum", bufs=4, space="PSUM") as abps, \
                 tc.tile_pool(name="qk_psum", bufs=2, space="PSUM") as qkps, \
                 tc.tile_pool(name="scr_psum", bufs=1, space="PSUM") as scrps:

                # PE joins: absorb each weight-DMA semaphore with a tiny bf16 matmul
                scr = scrps.tile([2, 2], F32, tag="scr", name="scrj")
                BF = mybir.dt.bfloat16
                for i, wtile in enumerate((wq_sb, wk_sb, wv_sb)):
                    src = wtile[0:1, 0:1, 0:2].bitcast(BF)[:, 0, 1::2]
                    nc.tensor.matmul(scr, src, src, start=(i == 0), stop=False)
                nc.tensor.matmul(scr, wo_sb[0:1, 0:2], wo_sb[0:1, 0:2],
                                 start=False, stop=True)

                def emit_stats_chain(tok4, t):
                    """RMSNorm rstd for one 128-token block."""
                    stats = stp.tile([128, 6], F32, tag="stats")
                    mv = stp.tile([128, 2], F32, tag="mv")
                    ms = stp.tile([128, 1], F32, tag="ms")
                    s_t = stp.tile([128, 1], F32, tag="s_t")
                    rstd = stp.tile([128, 1], F32, tag="rstd")
                    nc.vector.bn_stats(stats, tok4[:, t, :])
                    nc.vector.bn_aggr(mv, stats)
                    # E[x^2] = mean^2 + var
                    nc.vector.scalar_tensor_tensor(
                        ms, mv[:, 0:1], mv[:, 0:1], mv[:, 1:2],
                        op0=ALU.mult, op1=ALU.add)
                    nc.scalar.activation(s_t, ms, AF.Sqrt, bias=eps_t, scale=1.0)
                    nc.vector.reciprocal(rstd, s_t)
                    return rstd

                def emit_stage2_t(icp, tok4p, rstds, xtp, t):
                    """normalize + transpose one 128-token block of the
                    PREVIOUS chunk (its rstd is long since ready)."""
                    xn = stp.tile([128, 512], F32R, tag="xn")
                    eng = nc.vector if t % 2 == 0 else nc.gpsimd
                    eng.tensor_scalar_mul(xn, in0=tok4p[:, t, :], scalar1=rstds[t])
                    tp = abps.tile([128, 4, 128], F32R, tag="tp")
                    for c in range(4):
                        nc.tensor.transpose(tp[:, c, :], xn[:, c * 128:(c + 1) * 128], ident)
                    nc.scalar.copy(xtp[:, :, t * 128:(t + 1) * 128], tp)

                def emit_qkv(icp, xtp):
                    # QKV^T for one 512-token chunk (V first: DVE ordering)
                    vt = abp.tile([128, 512], BF16, tag="vt")
                    for wtile, dst in ((wv_sb, None), (wq_sb, QT), (wk_sb, KT)):
                        ps = qkps.tile([128, 512], F32, tag="qk")
                        for c in range(4):
                            nc.tensor.matmul(ps, wtile[:, c, :], xtp[:, c, :],
                                             start=(c == 0), stop=(c == 3))
                        if dst is None:
                            nc.scalar.copy(vt, ps)
                        else:
                            # ACT drains these: DVE is the phase-A critical path
                            nc.scalar.copy(dst[:, icp * 512:(icp + 1) * 512], ps)
                    # V^T -> V natural (bf16) into Vb (j on partitions)
                    vtp = abps.tile([128, 4, 128], BF16, tag="tp")
                    for jl in range(4):
                        nc.tensor.transpose(vtp[:, jl, :], vt[:, jl * 128:(jl + 1) * 128], identb)
                    nc.vector.tensor_copy(
                        Vb[:, icp * 4:(icp + 1) * 4, :, 0:64],
                        vtp.rearrange("p jl (h v) -> p jl h v", h=2))

                # one-chunk software pipeline: stats chains for chunk ic run
                # while chunk ic-1 is normalized/transposed/projected, so the
                # PE never waits on the DVE->ACT->DVE norm chain
                prev = None
                for ic in range(NIC + 1):
                    if ic < NIC:
                        if ic == 0:
                            tok4 = tok4_first
                        else:
                            tok4 = abp.tile([128, 4, 512], F32, tag="tok4")
                            nc.sync.dma_start(out=tok4, in_=tok_r[ic])
                        cur_rstds = []
                    for t in range(4):
                        if ic < NIC:
                            cur_rstds.append(emit_stats_chain(tok4, t))
                        if prev is not None:
                            emit_stage2_t(prev[0], prev[1], prev[2], prev[3], t)
                    if prev is not None:
                        emit_qkv(prev[0], prev[3])
                    if ic < NIC:
                        xt = abp.tile([128, 4, 512], F32R, tag="xt")
                        prev = (ic, tok4, cur_rstds, xt)

            # ---- phase C: attention + out-proj (j-tile pairs) ----
            with tc.tile_pool(name="c_sbuf", bufs=3) as cp, \
                 tc.tile_pool(name="pt_pool", bufs=6) as ptp, \
                 tc.tile_pool(name="opair_pool", bufs=2) as oprp, \
                 tc.tile_pool(name="outsb_pool", bufs=2) as outsbp, \
                 tc.tile_pool(name="st_psum", bufs=3, space="PSUM") as stps, \
                 tc.tile_pool(name="o_psum", bufs=2, space="PSUM") as ops:

                iters = [(ib, hl) for ib in range(NIC) for hl in range(2)]
                NIT = len(iters)
                LOOK = 2  # S^T pairs emitted ahead of exp/PV

                def emit_st(k):
                    """S^T for j-tile pair p of iteration k//NPAIR."""
                    it_idx, p = divmod(k, NPAIR)
                    ib, hl = iters[it_idx]
                    h0 = hl * 64
                    st = stps.tile([128, 2, 512], F32, tag="st", name="stg")
                    for jl in range(2):
                        jt = 2 * p + jl
                        nc.tensor.matmul(
                            st[:, jl, :],
                            KT[h0:h0 + 64, jt * 128:(jt + 1) * 128],
                            QT[h0:h0 + 64, ib * 512:(ib + 1) * 512],
                            start=True, stop=True)
                    return st

                def emit_exp(st, p, it_idx):
                    pt = ptp.tile([128, 2, 512], BF16, tag="pt", name="ptg")
                    e = (ENG_EVEN if it_idx % 2 == 0 else ENG_ODD)[p]
                    if e == "A":
                        nc.scalar.activation(
                            pt.rearrange("a b c -> a (b c)"),
                            st.rearrange("a b c -> a (b c)"), AF.Exp)
                    else:
                        nc.vector.tensor_scalar(
                            pt.bitcast(I16).rearrange("a b c -> a (b c)"),
                            st.rearrange("a b c -> a (b c)"), C1B, C2B,
                            op0=ALU.mult, op1=ALU.add)
                    return pt

                # tail state
                o_accs = {}      # iter idx -> o_acc psum tile
                o_pairs = {}     # ib -> o_pair sbuf tile
                linvs = {}       # iter idx -> linv tile
                out_sbs = {}     # ib -> out_sb tile
                ot_tiles = {}    # ib -> (ot psum tile, ots sbuf tile)
                op_tiles = {}    # it -> out-proj psum tile (transient per ib)

                def emit_tail_step(k_prev, g):
                    """Interleave iter k_prev's tail into iter k_prev+1's
                    pair-step stream at slot g (16 slots per iteration)."""
                    ib, hl = iters[k_prev]
                    if g == 0:
                        o_acc = o_accs[k_prev]
                        linv = cp.tile([128, 4], F32, tag="linv", name=f"lv{hl}")
                        nc.vector.reciprocal(linv, o_acc[:, :, 64])
                        linvs[k_prev] = linv
                        if hl == 0:
                            o_pairs[ib] = oprp.tile(
                                [128, 4, 128], BF16, tag="opair", name=f"op{ib}")
                    elif 1 <= g <= 4:
                        it = g - 1
                        o_acc = o_accs[k_prev]
                        nc.vector.tensor_scalar_mul(
                            o_pairs[ib][:, it, hl * 64:(hl + 1) * 64],
                            in0=o_acc[:, it, 0:64],
                            scalar1=linvs[k_prev][:, it:it + 1])
                        if it == 3:
                            o_accs.pop(k_prev)
                            linvs.pop(k_prev)
                    elif hl == 1 and g == 5:
                        # transpose the 4 normalized [i,128] pair-tiles into a
                        # single psum bank (start only zeroes on the first);
                        # back-to-back to keep the ring slot held briefly
                        opr = o_pairs[ib]
                        ot = stps.tile([128, 4, 128], BF16, tag="st", name="otp")
                        ot_tiles[ib] = [ot, None]
                        for it in range(4):
                            nc.tensor.matmul(
                                ot[:, it, :], opr[:, it, :], identb,
                                is_transpose=True, start=(it == 0), stop=(it == 3),
                                skip_group_check=True)
                    elif hl == 1 and g == 6:
                        ots = cp.tile([128, 4, 128], BF16, tag="ots", name="otsb")
                        nc.vector.tensor_copy(ots, ot_tiles[ib][0])
                        ot_tiles[ib][1] = ots
                        o_pairs.pop(ib)
                    elif hl == 1 and 7 <= g <= 11:
                        # g=7..10: out-proj matmul it=g-7; g=8..11: drain it=g-8
                        if g >= 8:
                            it_d = g - 8
                            nc.scalar.copy(out_sbs[ib][:, it_d, :],
                                           op_tiles.pop(it_d))
                            # per-it DMA so the store overlaps later drains
                            nc.sync.dma_start(out=out_r[ib, :, it_d, :],
                                              in_=out_sbs[ib][:, it_d, :])
                            if it_d == 3:
                                out_sbs.pop(ib)
                                ot_tiles.pop(ib)
                        if g <= 10:
                            it = g - 7
                            if it == 0:
                                out_sbs[ib] = outsbp.tile(
                                    [128, 4, 512], F32, tag="outsb", name=f"ob{ib}")
                            op_ps = stps.tile([128, 512], F32, tag="st", name="opps")
                            nc.tensor.matmul(op_ps, ot_tiles[ib][1][:, it, :],
                                             wo_sb, start=True, stop=True)
                            op_tiles[it] = op_ps

                def emit_pv(it_idx, p, pt):
                    _, hl = iters[it_idx]
                    if p == 0:
                        o_accs[it_idx] = ops.tile(
                            [128, 4, 65], F32, tag="o", name=f"oacc{it_idx % 2}")
                    o_acc = o_accs[it_idx]
                    for jl in range(2):
                        jt = 2 * p + jl
                        for it in range(4):
                            # PSUM `start` zeroes the WHOLE bank: only the
                            # very first matmul of this o_acc bank may set it
                            nc.tensor.matmul(
                                o_acc[:, it, :],
                                pt[:, jl, it * 128:(it + 1) * 128],
                                Vb[:, jt, hl, :],
                                start=(jt == 0 and it == 0),
                                stop=(jt == NJT - 1 and it == 3),
                                skip_group_check=True)

                # ---- attention pipeline over (iteration, j-tile pair) steps:
                # PV lags exp by one step so every exp has an extra pair-step
                # of slack before the PE consumes its output ----
                TOT = NIT * NPAIR
                sts = {}
                pts = {}
                for k in range(LOOK):
                    sts[k] = emit_st(k)
                for k in range(TOT + 1):
                    if k < TOT:
                        it_idx, p = divmod(k, NPAIR)
                        pts[k] = emit_exp(sts.pop(k), p, it_idx)
                        if k + LOOK < TOT:
                            sts[k + LOOK] = emit_st(k + LOOK)
                    if k >= 1:
                        emit_pv(*divmod(k - 1, NPAIR), pts.pop(k - 1))
                    if k < TOT and it_idx > 0:
                        # after the lagged PV: at p==0 the PV above was the
                        # previous iteration's last accumulation, which the
                        # tail's linv must observe
                        emit_tail_step(it_idx - 1, p)

                # trailing tail for the final iteration
                for g in range(NPAIR):
                    emit_tail_step(NIT - 1, g)

    fix_waits_nc(nc, mybir)
    return nc


def fix_waits_nc(nc, mybir):
    """Post-pass over the scheduled program: (1) remove semaphore waits that
    are transitively implied by earlier waits (Tile emits per-proc-minimal,
    not transitively-minimal, waits), (2) split any instruction still
    carrying more than one wait by injecting single-wait NoOps in front of
    it — this walrus build rejects >1 sync wait per instruction.
    Mutates nc in place so CoreSim and hardware run identical sync."""
    nop_id = [0]

    def _is_ge(w):
        return w.sync_type == "semaphore" and w.wait_mode == "sem-ge-imm"

    for fn in nc.m.functions:
        for blk in fn.blocks:
            insts = list(blk.instructions)
            n = len(insts)

            producers = {}
            cum = {}
            nonmono = set()  # sems ever decremented: counter logic invalid
            for idx, inst in enumerate(insts):
                si = inst.sync_info
                for u in (si.on_update if si else []) or []:
                    if u.sync_type != "semaphore":
                        continue
                    sid = u.id
                    if u.update_mode != "sem-inc":
                        nonmono.add(sid)
                        continue
                    cum[sid] = cum.get(sid, 0) + int(u.update_value)
                    producers.setdefault(sid, []).append((cum[sid], idx))

            def producer_of(sid, val):
                for cv, idx in producers.get(sid, ()):
                    if cv >= val:
                        return idx
                return None

            prev_eng = [None] * n
            last = {}
            for idx, inst in enumerate(insts):
                e = inst.engine
                prev_eng[idx] = last.get(e)
                last[e] = idx

            def get_waits(inst):
                si = inst.sync_info
                return list(si.on_wait) if si and si.on_wait else []

            def is_ge(w):
                return _is_ge(w) and w.id not in nonmono

            know = [dict() for _ in range(n)]
            for _ in range(3):
                changed = False
                for idx, inst in enumerate(insts):
                    k = dict(know[prev_eng[idx]]) if prev_eng[idx] is not None else {}
                    for w in get_waits(inst):
                        if not is_ge(w):
                            continue
                        sid, val = w.id, int(w.wait_value)
                        if k.get(sid, -1) < val:
                            k[sid] = val
                        p = producer_of(sid, val)
                        if p is not None:
                            for s2, v2 in know[p].items():
                                if k.get(s2, -1) < v2:
                                    k[s2] = v2
                    if k != know[idx]:
                        know[idx] = k
                        changed = True
                if not changed:
                    break

            new_insts = []
            dirty = False
            for idx, inst in enumerate(insts):
                si = inst.sync_info
                waits = get_waits(inst)
                if si is not None and waits:
                    base = dict(know[prev_eng[idx]]) if prev_eng[idx] is not None else {}
                    kept = []
                    for w in waits:
                        if is_ge(w):
                            sid, val = w.id, int(w.wait_value)
                            if base.get(sid, -1) >= val:
                                continue
                            base[sid] = val
                            p = producer_of(sid, val)
                            if p is not None:
                                for s2, v2 in know[p].items():
                                    if base.get(s2, -1) < v2:
                                        base[s2] = v2
                        kept.append(w)
                    if len(kept) != len(waits) or len(kept) > 1:
                        dirty = True
                        for w in kept[:-1]:
                            nop_id[0] += 1
                            nop = mybir.InstNoOp(
                                name=f"I-waitfix-{nop_id[0]}", ins=[], outs=[])
                            nop.engine = inst.engine
                            nop.sync_info = mybir.SyncInfo(on_wait=[w], on_update=[])
                            nc.register_instruction(nop)
                            new_insts.append(nop)
                        inst.sync_info = mybir.SyncInfo(
                            on_wait=kept[-1:],
                            on_update=list(si.on_update or []))
                new_insts.append(inst)
            if dirty:
                blk.instructions = new_insts


def get_program():
    if "nc" not in _prog_cache:
        _prog_cache["nc"] = build_program()
    return _prog_cache["nc"]


def _prep_inputs(tokens, norm_weight, w_qkv, w_out):
    import ml_dtypes
    tokens = np.ascontiguousarray(np.asarray(tokens, dtype=np.float32))
    norm_weight = np.asarray(norm_weight, dtype=np.float32)
    w_qkv = np.asarray(w_qkv, dtype=np.float32)
    w_out = np.asarray(w_out, dtype=np.float32)

    wp = w_qkv * norm_weight[:, None]  # fold RMSNorm weight into qkv weights

    in_maps = []
    for c in range(NCORES):
        b = c // 4
        h0 = 2 * (c % 4)
        m = {}
        m["tok"] = tokens[b]
        for name, off in (("wq", 0), ("wk", DI), ("wv", 2 * DI)):
            w = wp[:, off + h0 * DH: off + (h0 + 2) * DH]       # [512, 128]
            m[name] = np.ascontiguousarray(
                w.reshape(4, 128, 128).transpose(1, 0, 2))       # [128, 4, 128]
        rows = w_out[h0 * DH:(h0 + 2) * DH, :]                   # [128, 512]
        m["wo"] = np.ascontiguousarray(rows.astype(ml_dtypes.bfloat16))
        in_maps.append(m)
    return in_maps


def run(tokens, norm_weight, w_qkv, w_out, trace=False):
    from concourse.bass_utils import run_bass_kernel_spmd
    nc = get_program()
    in_maps = _prep_inputs(tokens, norm_weight, w_qkv, w_out)
    res = run_bass_kernel_spmd(nc, in_maps, core_ids=list(range(NCORES)), trace=trace)
    parts = [res.results[c]["out_part"] for c in range(NCORES)]
    out = np.empty((B, N, D), dtype=np.float32)
    for b in range(B):
        out[b] = parts[4 * b] + parts[4 * b + 1] + parts[4 * b + 2] + parts[4 * b + 3]
    return out, res


def kernel(tokens, norm_weight, w_qkv, w_out):
    out, _ = run(tokens, norm_weight, w_qkv, w_out, trace=False)
    return out


# revision 40
# speedup vs baseline: 1.3792x; 1.0275x over previous
"""TRN2 Bass kernel for nn_Attention (RMSNorm + QKV + softmax attention + out-proj).

Sharding: 8 cores = 2 batches x 4 head-pairs. Core c handles batch c//4 and
heads (2*(c%4), 2*(c%4)+1). Each core computes its partial out-projection
(contracting only its 128 rows of dim_inner); host sums the 4 partials per batch.

V2 design (cost-model driven):
  A) stream tokens [128,512] blocks: RMSNorm stats on DVE, rstd scale, PE
     transpose -> x^T (rotating [128,4,512] per 512-token chunk)
  B) QKV^T = w^T @ x^T (PE); V^T -> bf16 -> transposed back to V-natural with
     a ones column appended per head (softmax denominator for free)
  C) flash attention per (ib=512 queries, head): for each j-tile of 128 keys:
     S^T [128j, 512i] on PE (f32r, 512 cycles); exp split across THREE engines
     (ACT: exact table exp -> bf16; DVE/GPSIMD: Schraudolph bit-trick exp via
     tensor_scalar -> int16 bits of bf16). PV uses exp(S)^T tiles as the
     STATIONARY [128j, 128i] and bf16 V[128j,64+1] as MOVING: 65-cycle
     matmuls accumulate O-natural [128i, 65] in PSUM (col 64 = l).
  D) tail: linv=1/l (DVE), scale O by linv -> bf16 pair tile [i, 128dh both
     heads] (DVE), PE transpose [128,128], one 128-contraction out-proj matmul
     per i-tile vs bf16 w_out, ACT drains psum->sbuf, DMA out per ib.

Engine budget per core (cost model): PE ~206us (bottleneck), ACT ~150us,
DVE ~180us, Pool ~110us. exp assignment per j-tile keeps all three exp
engines concurrently busy so PE never starves.

Schraudolph exp: bf16_bits(p) = int16(S * (2^7/ln2) + C2B). Bias component
cancels through the softmax denominator; residual sawtooth ~1% std on the
offloaded fraction keeps total rel err well under the 2e-2 gate.

Engine discipline: walrus here allows only ONE semaphore wait per instruction
(fix_waits_nc post-pass splits/dedups), and no mixed 32/16-bit matmuls.
"""
import sys
sys.path.insert(0, "/opt/trn_rl_repo")
import numpy as np

B, N, D = 2, 4096, 512
H, DH = 8, 64
DI = H * DH
NCORES = 8
EPS = 1.1920929e-07  # float32 eps (torch nn.RMSNorm default)

# Schraudolph constants for bf16-bit exp: bits16(e^x) ~= x*C1B + C2B
C1B = 128.0 / 0.6931471805599453          # 2^7 / ln 2
C2B = float(127 * 128) - 366393.0 / 65536.0

_prog_cache = {}


def _patch_drain(tile_mod, mybir):
    """Split the multi-wait tail drain into a chain of single-wait drains
    (this walrus build rejects >1 sync wait per instruction)."""
    if getattr(tile_mod.TileContext, "_drain_patched", False):
        return

    def _patched(self, tick_clock, wait_clock):
        from concourse.vector_clock import ScopedClock
        nc = self.nc
        drain_inst = nc.sync.drain()
        wait_clock.add_sem_waits(drain_inst.ins, ScopedClock({None: tick_clock.global_clock}))
        si = drain_inst.ins.sync_info
        if si is not None and si.on_wait and len(si.on_wait) > 1:
            waits = list(si.on_wait)
            drain_inst.ins.sync_info = mybir.SyncInfo(
                on_wait=waits[:1], on_update=list(si.on_update or []))
            for w in waits[1:]:
                d2 = nc.sync.drain()
                d2.ins.sync_info = mybir.SyncInfo(on_wait=[w], on_update=[])
        nc.all_engine_barrier()
        assert self.sems is not None
        popped = nc._tile_sem_poison_stack.pop()
        assert popped is self._sem_poison
        nc.clear_and_free_semaphores(list(self.sems.allocated().values()))
        nc.all_engine_barrier()

    tile_mod.TileContext._drain_and_barrier = _patched
    tile_mod.TileContext._drain_patched = True


def build_program():
    import concourse.bass as bass
    import concourse.tile as tile
    from concourse import mybir
    from concourse.masks import make_identity

    _patch_drain(tile, mybir)

    F32 = mybir.dt.float32
    F32R = mybir.dt.float32r
    BF16 = mybir.dt.bfloat16
    I16 = mybir.dt.int16
    AF = mybir.ActivationFunctionType
    ALU = mybir.AluOpType

    NIC = N // 512          # 8 chunks of 512 tokens
    NJT = N // 128          # 32 key tiles of 128

    nc = bass.Bass(trn_type="TRN2", target_bir_lowering=False)

    tok = nc.dram_tensor("tok", [N, D], F32, kind="ExternalInput")
    wq = nc.dram_tensor("wq", [128, 4, 128], F32R, kind="ExternalInput")
    wk = nc.dram_tensor("wk", [128, 4, 128], F32R, kind="ExternalInput")
    wv = nc.dram_tensor("wv", [128, 4, 128], F32R, kind="ExternalInput")
    wo = nc.dram_tensor("wo", [128, 512], BF16, kind="ExternalInput")
    out_part = nc.dram_tensor("out_part", [N, D], F32, kind="ExternalOutput")

    tok_r = tok.rearrange("(ic t p) d -> ic p t d", t=4, p=128)
    out_r = out_part.rearrange("(ib t p) e -> ib p t e", t=4, p=128)

    # exp-engine assignment per j-tile PAIR within an iteration (A=ACT exact,
    # D=DVE Schraudolph), weighted so both engines finish together just
    # below the PE's per-iteration time. GPSIMD cannot read PSUM, so it
    # cannot join the exp crew; it takes phase-A SBUF work instead.
    NPAIR = NJT // 2
    def _mk_eng(w_a, w_d):
        pat, cnt = [], {"A": 0, "D": 0}
        wgt = {"A": w_a, "D": w_d}
        for _ in range(NPAIR):
            e = min(cnt, key=lambda k: (cnt[k] + 1) / wgt[k])
            pat.append(e)
            cnt[e] += 1
        return pat
    # alternate the split between iterations so ACT (which also does the
    # out-proj drains) keeps headroom below the PE's per-iteration pace
    ENG_EVEN = _mk_eng(9, 7)
    ENG_ODD = _mk_eng(8, 8)

    with tile.TileContext(nc) as tc:
        with tc.tile_pool(name="consts", bufs=1) as consts, \
             tc.tile_pool(name="big", bufs=1) as big, \
             tc.tile_pool(name="wpool", bufs=1) as wpool:

            # ---- constants ----
            ident_f = consts.tile([128, 128], F32)
            make_identity(nc, ident_f)
            ident = consts.tile([128, 128], F32R)
            nc.vector.tensor_copy(ident, ident_f)
            identb = consts.tile([128, 128], BF16)
            nc.vector.tensor_copy(identb, ident_f)
            eps_t = consts.tile([128, 1], F32)
            nc.vector.memset(eps_t, EPS)
            # preload the Exp ACT table during the otherwise-idle start window
            warm = consts.tile([1, 1], F32)
            nc.scalar.activation(warm, eps_t[0:1, :], AF.Exp)

            # ---- weights ----
            # first token chunk goes ahead of the weights in the DMA queue so
            # the RMSNorm pipeline starts as early as possible
            tok4_first = wpool.tile([128, 4, 512], F32)
            for t in range(4):
                nc.sync.dma_start(out=tok4_first[:, t, :], in_=tok_r[0, :, t, :])
            wq_sb = wpool.tile([128, 4, 128], F32R)
            wk_sb = wpool.tile([128, 4, 128], F32R)
            wv_sb = wpool.tile([128, 4, 128], F32R)
            wo_sb = wpool.tile([128, 512], BF16)
            nc.sync.dma_start(out=wq_sb, in_=wq[:, :, :])
            nc.sync.dma_start(out=wk_sb, in_=wk[:, :, :])
            nc.sync.dma_start(out=wv_sb, in_=wv[:, :, :])
            nc.sync.dma_start(out=wo_sb, in_=wo[:, :])

            # ---- persistent big buffers ----
            QT = big.tile([128, N], F32R)       # [2 heads x 64 qdims, n]
            KT = big.tile([128, N], F32R)
            Vb = big.tile([128, NJT, 2, 65], BF16)  # per j-tile: [v(64)|ones] per head
            ones_bf = consts.tile([128, NJT, 2, 1], BF16)
            nc.vector.memset(ones_bf, 1.0)
            nc.vector.tensor_copy(Vb[:, :, :, 64:65], ones_bf)

            # ---- phase A/B: RMSNorm + x^T + QKV^T + V natural ----
            with tc.tile_pool(name="ab_sbuf", bufs=3) as abp, \
                 tc.tile_pool(name="ab_stats", bufs=8) as stp, \
                 tc.tile_pool(name="ab_psum", bufs=4, space="PSUM") as abps, \
                 tc.tile_pool(name="qk_psum", bufs=3, space="PSUM") as qkps, \
                 tc.tile_pool(name="scr_psum", bufs=1, space="PSUM") as scrps:

                # PE joins: absorb each weight-DMA semaphore with a tiny bf16 matmul
                scr = scrps.tile([2, 2], F32, tag="scr", name="scrj")
                BF = mybir.dt.bfloat16
                for i, wtile in enumerate((wq_sb, wk_sb, wv_sb)):
                    src = wtile[0:1, 0:1, 0:2].bitcast(BF)[:, 0, 1::2]
                    nc.tensor.matmul(scr, src, src, start=(i == 0), stop=False)
                nc.tensor.matmul(scr, wo_sb[0:1, 0:2], wo_sb[0:1, 0:2],
                                 start=False, stop=True)

                def emit_stats_chain(tok4, t):
                    """RMSNorm rstd for one 128-token block."""
                    stats = stp.tile([128, 6], F32, tag="stats")
                    mv = stp.tile([128, 2], F32, tag="mv")
                    ms = stp.tile([128, 1], F32, tag="ms")
                    s_t = stp.tile([128, 1], F32, tag="s_t")
                    rstd = stp.tile([128, 1], F32, tag="rstd")
                    nc.vector.bn_stats(stats, tok4[:, t, :])
                    nc.vector.bn_aggr(mv, stats)
                    # E[x^2] = mean^2 + var
                    nc.vector.scalar_tensor_tensor(
                        ms, mv[:, 0:1], mv[:, 0:1], mv[:, 1:2],
                        op0=ALU.mult, op1=ALU.add)
                    nc.scalar.activation(s_t, ms, AF.Sqrt, bias=eps_t, scale=1.0)
                    nc.vector.reciprocal(rstd, s_t)
                    return rstd

                def emit_stage2_t(icp, tok4p, rstds, xtp, t):
                    """normalize + transpose one 128-token block of the
                    PREVIOUS chunk (its rstd is long since ready)."""
                    xn = stp.tile([128, 512], F32R, tag="xn")
                    eng = nc.vector if t % 2 == 0 else nc.gpsimd
                    eng.tensor_scalar_mul(xn, in0=tok4p[:, t, :], scalar1=rstds[t])
                    tp = abps.tile([128, 4, 128], F32R, tag="tp")
                    for c in range(4):
                        nc.tensor.transpose(tp[:, c, :], xn[:, c * 128:(c + 1) * 128], ident)
                    nc.scalar.copy(xtp[:, :, t * 128:(t + 1) * 128], tp)

                def emit_qkv_half(state, half):
                    # one 256-token half of QKV^T: the first half only needs
                    # token blocks t0/t1, so the PE starts projecting while
                    # t2/t3 are still normalizing. 256-free keeps f32r at
                    # 1 cycle/row.
                    xtp = state["xt"]
                    lo, hi = (0, 256) if half == 0 else (256, 512)
                    for idx, wtile in enumerate((wv_sb, wq_sb, wk_sb)):
                        if half == 0:
                            state["ps"].append(qkps.tile(
                                [128, 512], F32, tag="qk", name=f"qk{idx}"))
                        ps = state["ps"][idx]
                        for c in range(4):
                            # the bank-zeroing start is only on the first
                            # matmul of the bank (it zeroes both halves)
                            nc.tensor.matmul(
                                ps[:, lo:hi], wtile[:, c, :], xtp[:, c, lo:hi],
                                start=(half == 0 and c == 0),
                                stop=(half == 1 and c == 3),
                                skip_group_check=True)

                def emit_qkv_drain(icp, state):
                    vt = abp.tile([128, 512], BF16, tag="vt")
                    nc.scalar.copy(vt, state["ps"][0])
                    # ACT drains these: DVE is the phase-A critical path
                    nc.scalar.copy(QT[:, icp * 512:(icp + 1) * 512], state["ps"][1])
                    nc.scalar.copy(KT[:, icp * 512:(icp + 1) * 512], state["ps"][2])
                    # V^T -> V natural (bf16) into Vb (j on partitions)
                    vtp = abps.tile([128, 4, 128], BF16, tag="tp")
                    for jl in range(4):
                        nc.tensor.transpose(vtp[:, jl, :], vt[:, jl * 128:(jl + 1) * 128], identb)
                    nc.vector.tensor_copy(
                        Vb[:, icp * 4:(icp + 1) * 4, :, 0:64],
                        vtp.rearrange("p jl (h v) -> p jl h v", h=2))

                # one-chunk software pipeline: stats chains for chunk ic run
                # while chunk ic-1 is normalized/transposed/projected, so the
                # PE never waits on the DVE->ACT->DVE norm chain
                prev = None
                for ic in range(NIC + 1):
                    if ic < NIC:
                        if ic == 0:
                            tok4 = tok4_first
                        else:
                            tok4 = abp.tile([128, 4, 512], F32, tag="tok4")
                            nc.sync.dma_start(out=tok4, in_=tok_r[ic])
                        cur_rstds = []
                        cur_state = {"ic": ic, "tok4": tok4,
                                     "rstds": cur_rstds, "ps": [],
                                     "xt": abp.tile([128, 4, 512], F32R,
                                                    tag="xt", name=f"xt{ic}")}
                    for t in range(4):
                        if prev is not None:
                            emit_stage2_t(prev["ic"], prev["tok4"],
                                          prev["rstds"], prev["xt"], t)
                            if t == 1:
                                emit_qkv_half(prev, 0)
                        if ic < NIC:
                            cur_rstds.append(emit_stats_chain(tok4, t))
                            if ic == 0:
                                # chunk 0: normalize inline, right behind its
                                # own stats chain
                                emit_stage2_t(0, tok4, cur_rstds,
                                              cur_state["xt"], t)
                                if t == 1:
                                    emit_qkv_half(cur_state, 0)
                    if prev is not None:
                        emit_qkv_half(prev, 1)
                        emit_qkv_drain(prev["ic"], prev)
                        prev = None
                    if ic == 0:
                        emit_qkv_half(cur_state, 1)
                        emit_qkv_drain(0, cur_state)
                    elif ic < NIC:
                        prev = cur_state

            # ---- phase C: attention + out-proj (j-tile pairs) ----
            with tc.tile_pool(name="c_sbuf", bufs=3) as cp, \
                 tc.tile_pool(name="pt_pool", bufs=6) as ptp, \
                 tc.tile_pool(name="opair_pool", bufs=2) as oprp, \
                 tc.tile_pool(name="outsb_pool", bufs=2) as outsbp, \
                 tc.tile_pool(name="st_psum", bufs=3, space="PSUM") as stps, \
                 tc.tile_pool(name="o_psum", bufs=2, space="PSUM") as ops:

                iters = [(ib, hl) for ib in range(NIC) for hl in range(2)]
                NIT = len(iters)
                LOOK = 2  # S^T pairs emitted ahead of exp/PV

                def emit_st(k):
                    """S^T for j-tile pair p of iteration k//NPAIR."""
                    it_idx, p = divmod(k, NPAIR)
                    ib, hl = iters[it_idx]
                    h0 = hl * 64
                    st = stps.tile([128, 2, 512], F32, tag="st", name="stg")
                    for jl in range(2):
                        jt = 2 * p + jl
                        nc.tensor.matmul(
                            st[:, jl, :],
                            KT[h0:h0 + 64, jt * 128:(jt + 1) * 128],
                            QT[h0:h0 + 64, ib * 512:(ib + 1) * 512],
                            start=True, stop=True)
                    return st

                def emit_exp(st, p, it_idx):
                    pt = ptp.tile([128, 2, 512], BF16, tag="pt", name="ptg")
                    e = (ENG_EVEN if it_idx % 2 == 0 else ENG_ODD)[p]
                    if e == "A":
                        nc.scalar.activation(
                            pt.rearrange("a b c -> a (b c)"),
                            st.rearrange("a b c -> a (b c)"), AF.Exp)
                    else:
                        nc.vector.tensor_scalar(
                            pt.bitcast(I16).rearrange("a b c -> a (b c)"),
                            st.rearrange("a b c -> a (b c)"), C1B, C2B,
                            op0=ALU.mult, op1=ALU.add)
                    return pt

                # tail state
                o_accs = {}      # iter idx -> o_acc psum tile
                o_pairs = {}     # ib -> o_pair sbuf tile
                linvs = {}       # iter idx -> linv tile
                out_sbs = {}     # ib -> out_sb tile
                ot_tiles = {}    # ib -> (ot psum tile, ots sbuf tile)
                op_tiles = {}    # it -> out-proj psum tile (transient per ib)

                def emit_tail_step(k_prev, g):
                    """Interleave iter k_prev's tail into iter k_prev+1's
                    pair-step stream at slot g (16 slots per iteration)."""
                    ib, hl = iters[k_prev]
                    if g == 0:
                        o_acc = o_accs[k_prev]
                        linv = cp.tile([128, 4], F32, tag="linv", name=f"lv{hl}")
                        nc.vector.reciprocal(linv, o_acc[:, :, 64])
                        linvs[k_prev] = linv
                        if hl == 0:
                            o_pairs[ib] = oprp.tile(
                                [128, 4, 128], BF16, tag="opair", name=f"op{ib}")
                    elif 1 <= g <= 4:
                        it = g - 1
                        o_acc = o_accs[k_prev]
                        nc.vector.tensor_scalar_mul(
                            o_pairs[ib][:, it, hl * 64:(hl + 1) * 64],
                            in0=o_acc[:, it, 0:64],
                            scalar1=linvs[k_prev][:, it:it + 1])
                        if it == 3:
                            o_accs.pop(k_prev)
                            linvs.pop(k_prev)
                    elif hl == 1 and g == 5:
                        # transpose the 4 normalized [i,128] pair-tiles into a
                        # single psum bank (start only zeroes on the first);
                        # back-to-back to keep the ring slot held briefly
                        opr = o_pairs[ib]
                        ot = stps.tile([128, 4, 128], BF16, tag="st", name="otp")
                        ot_tiles[ib] = [ot, None]
                        for it in range(4):
                            nc.tensor.matmul(
                                ot[:, it, :], opr[:, it, :], identb,
                                is_transpose=True, start=(it == 0), stop=(it == 3),
                                skip_group_check=True)
                    elif hl == 1 and g == 6:
                        ots = cp.tile([128, 4, 128], BF16, tag="ots", name="otsb")
                        nc.vector.tensor_copy(ots, ot_tiles[ib][0])
                        ot_tiles[ib][1] = ots
                        o_pairs.pop(ib)
                    elif hl == 1 and 7 <= g <= 11:
                        # g=7..10: out-proj matmul it=g-7; g=8..11: drain it=g-8
                        if g >= 8:
                            it_d = g - 8
                            nc.scalar.copy(out_sbs[ib][:, it_d, :],
                                           op_tiles.pop(it_d))
                            # per-it DMA so the store overlaps later drains
                            nc.sync.dma_start(out=out_r[ib, :, it_d, :],
                                              in_=out_sbs[ib][:, it_d, :])
                            if it_d == 3:
                                out_sbs.pop(ib)
                                ot_tiles.pop(ib)
                        if g <= 10:
                            it = g - 7
                            if it == 0:
                                out_sbs[ib] = outsbp.tile(
                                    [128, 4, 512], F32, tag="outsb", name=f"ob{ib}")
                            op_ps = stps.tile([128, 512], F32, tag="st", name="opps")
                            nc.tensor.matmul(op_ps, ot_tiles[ib][1][:, it, :],
                                             wo_sb, start=True, stop=True)
                            op_tiles[it] = op_ps

                def emit_pv(it_idx, p, pt):
                    _, hl = iters[it_idx]
                    if p == 0:
                        o_accs[it_idx] = ops.tile(
                            [128, 4, 65], F32, tag="o", name=f"oacc{it_idx % 2}")
                    o_acc = o_accs[it_idx]
                    for jl in range(2):
                        jt = 2 * p + jl
                        for it in range(4):
                            # PSUM `start` zeroes the WHOLE bank: only the
                            # very first matmul of this o_acc bank may set it
                            nc.tensor.matmul(
                                o_acc[:, it, :],
                                pt[:, jl, it * 128:(it + 1) * 128],
                                Vb[:, jt, hl, :],
                                start=(jt == 0 and it == 0),
                                stop=(jt == NJT - 1 and it == 3),
                                skip_group_check=True)

                # ---- attention pipeline over (iteration, j-tile pair) steps:
                # PV lags exp by one step so every exp has an extra pair-step
                # of slack before the PE consumes its output ----
                TOT = NIT * NPAIR
                sts = {}
                pts = {}
                for k in range(LOOK):
                    sts[k] = emit_st(k)
                for k in range(TOT + 2):
                    if k < TOT:
                        it_idx, p = divmod(k, NPAIR)
                        pts[k] = emit_exp(sts.pop(k), p, it_idx)
                        if k + LOOK < TOT:
                            sts[k + LOOK] = emit_st(k + LOOK)
                    if k >= 2:
                        emit_pv(*divmod(k - 2, NPAIR), pts.pop(k - 2))
                    if k < TOT and it_idx > 0 and p >= 2:
                        # tail slots start at p==2: the previous iteration's
                        # final (2-step-lagged) PV lands at p==1, and the
                        # tail's linv must observe it
                        emit_tail_step(it_idx - 1, p - 2)

                # trailing tail for the final iteration
                for g in range(NPAIR):
                    emit_tail_step(NIT - 1, g)

    fix_waits_nc(nc, mybir)
    return nc


def fix_waits_nc(nc, mybir):
    """Post-pass over the scheduled program: (1) remove semaphore waits that
    are transitively implied by earlier waits (Tile emits per-proc-minimal,
    not transitively-minimal, waits), (2) split any instruction still
    carrying more than one wait by injecting single-wait NoOps in front of
    it — this walrus build rejects >1 sync wait per instruction.
    Mutates nc in place so CoreSim and hardware run identical sync."""
    nop_id = [0]

    def _is_ge(w):
        return w.sync_type == "semaphore" and w.wait_mode == "sem-ge-imm"

    for fn in nc.m.functions:
        for blk in fn.blocks:
            insts = list(blk.instructions)
            n = len(insts)

            producers = {}
            cum = {}
            nonmono = set()  # sems ever decremented: counter logic invalid
            for idx, inst in enumerate(insts):
                si = inst.sync_info
                for u in (si.on_update if si else []) or []:
                    if u.sync_type != "semaphore":
                        continue
                    sid = u.id
                    if u.update_mode != "sem-inc":
                        nonmono.add(sid)
                        continue
                    cum[sid] = cum.get(sid, 0) + int(u.update_value)
                    producers.setdefault(sid, []).append((cum[sid], idx))

            def producer_of(sid, val):
                for cv, idx in producers.get(sid, ()):
                    if cv >= val:
                        return idx
                return None

            prev_eng = [None] * n
            last = {}
            for idx, inst in enumerate(insts):
                e = inst.engine
                prev_eng[idx] = last.get(e)
                last[e] = idx

            def get_waits(inst):
                si = inst.sync_info
                return list(si.on_wait) if si and si.on_wait else []

            def is_ge(w):
                return _is_ge(w) and w.id not in nonmono

            know = [dict() for _ in range(n)]
            for _ in range(3):
                changed = False
                for idx, inst in enumerate(insts):
                    k = dict(know[prev_eng[idx]]) if prev_eng[idx] is not None else {}
                    for w in get_waits(inst):
                        if not is_ge(w):
                            continue
                        sid, val = w.id, int(w.wait_value)
                        if k.get(sid, -1) < val:
                            k[sid] = val
                        p = producer_of(sid, val)
                        if p is not None:
                            for s2, v2 in know[p].items():
                                if k.get(s2, -1) < v2:
                                    k[s2] = v2
                    if k != know[idx]:
                        know[idx] = k
                        changed = True
                if not changed:
                    break

            new_insts = []
            dirty = False
            for idx, inst in enumerate(insts):
                si = inst.sync_info
                waits = get_waits(inst)
                if si is not None and waits:
                    base = dict(know[prev_eng[idx]]) if prev_eng[idx] is not None else {}
                    kept = []
                    for w in waits:
                        if is_ge(w):
                            sid, val = w.id, int(w.wait_value)
                            if base.get(sid, -1) >= val:
                                continue
                            base[sid] = val
                            p = producer_of(sid, val)
                            if p is not None:
                                for s2, v2 in know[p].items():
                                    if base.get(s2, -1) < v2:
                                        base[s2] = v2
                        kept.append(w)
                    if len(kept) != len(waits) or len(kept) > 1:
                        dirty = True
                        for w in kept[:-1]:
                            nop_id[0] += 1
                            nop = mybir.InstNoOp(
                                name=f"I-waitfix-{nop_id[0]}", ins=[], outs=[])
                            nop.engine = inst.engine
                            nop.sync_info = mybir.SyncInfo(on_wait=[w], on_update=[])
                            nc.register_instruction(nop)
                            new_insts.append(nop)
                        inst.sync_info = mybir.SyncInfo(
                            on_wait=kept[-1:],
                            on_update=list(si.on_update or []))
                new_insts.append(inst)
            if dirty:
                blk.instructions = new_insts


def get_program():
    if "nc" not in _prog_cache:
        _prog_cache["nc"] = build_program()
    return _prog_cache["nc"]


def _prep_inputs(tokens, norm_weight, w_qkv, w_out):
    import ml_dtypes
    tokens = np.ascontiguousarray(np.asarray(tokens, dtype=np.float32))
    norm_weight = np.asarray(norm_weight, dtype=np.float32)
    w_qkv = np.asarray(w_qkv, dtype=np.float32)
    w_out = np.asarray(w_out, dtype=np.float32)

    wp = w_qkv * norm_weight[:, None]  # fold RMSNorm weight into qkv weights

    in_maps = []
    for c in range(NCORES):
        b = c // 4
        h0 = 2 * (c % 4)
        m = {}
        m["tok"] = tokens[b]
        for name, off in (("wq", 0), ("wk", DI), ("wv", 2 * DI)):
            w = wp[:, off + h0 * DH: off + (h0 + 2) * DH]       # [512, 128]
            m[name] = np.ascontiguousarray(
                w.reshape(4, 128, 128).transpose(1, 0, 2))       # [128, 4, 128]
        rows = w_out[h0 * DH:(h0 + 2) * DH, :]                   # [128, 512]
        m["wo"] = np.ascontiguousarray(rows.astype(ml_dtypes.bfloat16))
        in_maps.append(m)
    return in_maps


def run(tokens, norm_weight, w_qkv, w_out, trace=False):
    from concourse.bass_utils import run_bass_kernel_spmd
    nc = get_program()
    in_maps = _prep_inputs(tokens, norm_weight, w_qkv, w_out)
    res = run_bass_kernel_spmd(nc, in_maps, core_ids=list(range(NCORES)), trace=trace)
    parts = [res.results[c]["out_part"] for c in range(NCORES)]
    out = np.empty((B, N, D), dtype=np.float32)
    for b in range(B):
        out[b] = parts[4 * b] + parts[4 * b + 1] + parts[4 * b + 2] + parts[4 * b + 3]
    return out, res


def kernel(tokens, norm_weight, w_qkv, w_out):
    out, _ = run(tokens, norm_weight, w_qkv, w_out, trace=False)
    return out


# revision 43
# speedup vs baseline: 1.3810x; 1.0013x over previous
"""TRN2 Bass kernel for nn_Attention (RMSNorm + QKV + softmax attention + out-proj).

Sharding: 8 cores = 2 batches x 4 head-pairs. Core c handles batch c//4 and
heads (2*(c%4), 2*(c%4)+1). Each core computes its partial out-projection
(contracting only its 128 rows of dim_inner); host sums the 4 partials per batch.

V2 design (cost-model driven):
  A) stream tokens [128,512] blocks: RMSNorm stats on DVE, rstd scale, PE
     transpose -> x^T (rotating [128,4,512] per 512-token chunk)
  B) QKV^T = w^T @ x^T (PE); V^T -> bf16 -> transposed back to V-natural with
     a ones column appended per head (softmax denominator for free)
  C) flash attention per (ib=512 queries, head): for each j-tile of 128 keys:
     S^T [128j, 512i] on PE (f32r, 512 cycles); exp split across THREE engines
     (ACT: exact table exp -> bf16; DVE/GPSIMD: Schraudolph bit-trick exp via
     tensor_scalar -> int16 bits of bf16). PV uses exp(S)^T tiles as the
     STATIONARY [128j, 128i] and bf16 V[128j,64+1] as MOVING: 65-cycle
     matmuls accumulate O-natural [128i, 65] in PSUM (col 64 = l).
  D) tail: linv=1/l (DVE), scale O by linv -> bf16 pair tile [i, 128dh both
     heads] (DVE), PE transpose [128,128], one 128-contraction out-proj matmul
     per i-tile vs bf16 w_out, ACT drains psum->sbuf, DMA out per ib.

Engine budget per core (cost model): PE ~206us (bottleneck), ACT ~150us,
DVE ~180us, Pool ~110us. exp assignment per j-tile keeps all three exp
engines concurrently busy so PE never starves.

Schraudolph exp: bf16_bits(p) = int16(S * (2^7/ln2) + C2B). Bias component
cancels through the softmax denominator; residual sawtooth ~1% std on the
offloaded fraction keeps total rel err well under the 2e-2 gate.

Engine discipline: walrus here allows only ONE semaphore wait per instruction
(fix_waits_nc post-pass splits/dedups), and no mixed 32/16-bit matmuls.
"""
import sys
sys.path.insert(0, "/opt/trn_rl_repo")
import numpy as np

B, N, D = 2, 4096, 512
H, DH = 8, 64
DI = H * DH
NCORES = 8
EPS = 1.1920929e-07  # float32 eps (torch nn.RMSNorm default)

# Schraudolph constants for bf16-bit exp: bits16(e^x) ~= x*C1B + C2B
C1B = 128.0 / 0.6931471805599453          # 2^7 / ln 2
C2B = float(127 * 128) - 366393.0 / 65536.0

_prog_cache = {}


def _patch_drain(tile_mod, mybir):
    """Split the multi-wait tail drain into a chain of single-wait drains
    (this walrus build rejects >1 sync wait per instruction)."""
    if getattr(tile_mod.TileContext, "_drain_patched", False):
        return

    def _patched(self, tick_clock, wait_clock):
        from concourse.vector_clock import ScopedClock
        nc = self.nc
        drain_inst = nc.sync.drain()
        wait_clock.add_sem_waits(drain_inst.ins, ScopedClock({None: tick_clock.global_clock}))
        si = drain_inst.ins.sync_info
        if si is not None and si.on_wait and len(si.on_wait) > 1:
            waits = list(si.on_wait)
            drain_inst.ins.sync_info = mybir.SyncInfo(
                on_wait=waits[:1], on_update=list(si.on_update or []))
            for w in waits[1:]:
                d2 = nc.sync.drain()
                d2.ins.sync_info = mybir.SyncInfo(on_wait=[w], on_update=[])
        nc.all_engine_barrier()
        assert self.sems is not None
        popped = nc._tile_sem_poison_stack.pop()
        assert popped is self._sem_poison
        nc.clear_and_free_semaphores(list(self.sems.allocated().values()))
        nc.all_engine_barrier()

    tile_mod.TileContext._drain_and_barrier = _patched
    tile_mod.TileContext._drain_patched = True


def build_program():
    import concourse.bass as bass
    import concourse.tile as tile
    from concourse import mybir
    from concourse.masks import make_identity

    _patch_drain(tile, mybir)

    F32 = mybir.dt.float32
    F32R = mybir.dt.float32r
    BF16 = mybir.dt.bfloat16
    I16 = mybir.dt.int16
    AF = mybir.ActivationFunctionType
    ALU = mybir.AluOpType

    NIC = N // 512          # 8 chunks of 512 tokens
    NJT = N // 128          # 32 key tiles of 128

    nc = bass.Bass(trn_type="TRN2", target_bir_lowering=False)

    tok = nc.dram_tensor("tok", [N, D], F32, kind="ExternalInput")
    wq = nc.dram_tensor("wq", [128, 4, 128], F32R, kind="ExternalInput")
    wk = nc.dram_tensor("wk", [128, 4, 128], F32R, kind="ExternalInput")
    wv = nc.dram_tensor("wv", [128, 4, 128], F32R, kind="ExternalInput")
    wo = nc.dram_tensor("wo", [128, 512], BF16, kind="ExternalInput")
    out_part = nc.dram_tensor("out_part", [N, D], F32, kind="ExternalOutput")

    tok_r = tok.rearrange("(ic t p) d -> ic p t d", t=4, p=128)
    out_r = out_part.rearrange("(ib t p) e -> ib p t e", t=4, p=128)

    # exp-engine assignment per j-tile PAIR within an iteration (A=ACT exact,
    # D=DVE Schraudolph), weighted so both engines finish together just
    # below the PE's per-iteration time. GPSIMD cannot read PSUM, so it
    # cannot join the exp crew; it takes phase-A SBUF work instead.
    NPAIR = NJT // 2
    def _mk_eng(w_a, w_d):
        pat, cnt = [], {"A": 0, "D": 0}
        wgt = {"A": w_a, "D": w_d}
        for _ in range(NPAIR):
            e = min(cnt, key=lambda k: (cnt[k] + 1) / wgt[k])
            pat.append(e)
            cnt[e] += 1
        return pat
    # alternate the split between iterations so ACT (which also does the
    # out-proj drains) keeps headroom below the PE's per-iteration pace
    ENG_EVEN = _mk_eng(9, 7)
    ENG_ODD = _mk_eng(8, 8)

    with tile.TileContext(nc) as tc:
        with tc.tile_pool(name="consts", bufs=1) as consts, \
             tc.tile_pool(name="big", bufs=1) as big, \
             tc.tile_pool(name="wpool", bufs=1) as wpool:

            # ---- constants ----
            ident_f = consts.tile([128, 128], F32)
            make_identity(nc, ident_f)
            ident = consts.tile([128, 128], F32R)
            nc.vector.tensor_copy(ident, ident_f)
            identb = consts.tile([128, 128], BF16)
            nc.vector.tensor_copy(identb, ident_f)
            eps_t = consts.tile([128, 1], F32)
            nc.vector.memset(eps_t, EPS)
            # preload the Exp ACT table during the otherwise-idle start window
            warm = consts.tile([1, 1], F32)
            nc.scalar.activation(warm, eps_t[0:1, :], AF.Exp)

            # ---- weights ----
            # first token chunk goes ahead of the weights in the DMA queue so
            # the RMSNorm pipeline starts as early as possible
            tok4_first = wpool.tile([128, 4, 512], F32)
            for t in range(4):
                nc.sync.dma_start(out=tok4_first[:, t, :], in_=tok_r[0, :, t, :])
            wq_sb = wpool.tile([128, 4, 128], F32R)
            wk_sb = wpool.tile([128, 4, 128], F32R)
            wv_sb = wpool.tile([128, 4, 128], F32R)
            wo_sb = wpool.tile([128, 512], BF16)
            nc.sync.dma_start(out=wq_sb, in_=wq[:, :, :])
            nc.sync.dma_start(out=wk_sb, in_=wk[:, :, :])
            nc.sync.dma_start(out=wv_sb, in_=wv[:, :, :])
            nc.sync.dma_start(out=wo_sb, in_=wo[:, :])

            # ---- persistent big buffers ----
            QT = big.tile([128, N], F32R)       # [2 heads x 64 qdims, n]
            KT = big.tile([128, N], F32R)
            Vb = big.tile([128, NJT, 2, 65], BF16)  # per j-tile: [v(64)|ones] per head
            ones_bf = consts.tile([128, NJT, 2, 1], BF16)
            nc.vector.memset(ones_bf, 1.0)
            nc.vector.tensor_copy(Vb[:, :, :, 64:65], ones_bf)

            # ---- phase A/B: RMSNorm + x^T + QKV^T + V natural ----
            with tc.tile_pool(name="ab_sbuf", bufs=3) as abp, \
                 tc.tile_pool(name="ab_stats", bufs=8) as stp, \
                 tc.tile_pool(name="ab_psum", bufs=5, space="PSUM") as abps, \
                 tc.tile_pool(name="qk_psum", bufs=3, space="PSUM") as qkps:

                # PE joins: absorb each weight-DMA semaphore with a tiny bf16 matmul
                scr = qkps.tile([2, 2], F32, tag="qk", name="scrj")
                BF = mybir.dt.bfloat16
                for i, wtile in enumerate((wq_sb, wk_sb, wv_sb)):
                    src = wtile[0:1, 0:1, 0:2].bitcast(BF)[:, 0, 1::2]
                    nc.tensor.matmul(scr, src, src, start=(i == 0), stop=False)
                nc.tensor.matmul(scr, wo_sb[0:1, 0:2], wo_sb[0:1, 0:2],
                                 start=False, stop=True)

                def emit_stats_chain(tok4, t):
                    """RMSNorm rstd for one 128-token block."""
                    stats = stp.tile([128, 6], F32, tag="stats")
                    mv = stp.tile([128, 2], F32, tag="mv")
                    ms = stp.tile([128, 1], F32, tag="ms")
                    s_t = stp.tile([128, 1], F32, tag="s_t")
                    rstd = stp.tile([128, 1], F32, tag="rstd")
                    nc.vector.bn_stats(stats, tok4[:, t, :])
                    nc.vector.bn_aggr(mv, stats)
                    # E[x^2] = mean^2 + var
                    nc.vector.scalar_tensor_tensor(
                        ms, mv[:, 0:1], mv[:, 0:1], mv[:, 1:2],
                        op0=ALU.mult, op1=ALU.add)
                    nc.scalar.activation(s_t, ms, AF.Sqrt, bias=eps_t, scale=1.0)
                    nc.vector.reciprocal(rstd, s_t)
                    return rstd

                def emit_stage2_t(icp, tok4p, rstds, xtp, t):
                    """normalize + transpose one 128-token block of the
                    PREVIOUS chunk (its rstd is long since ready)."""
                    xn = stp.tile([128, 512], F32R, tag="xn")
                    eng = nc.vector if t % 2 == 0 else nc.gpsimd
                    eng.tensor_scalar_mul(xn, in0=tok4p[:, t, :], scalar1=rstds[t])
                    tp = abps.tile([128, 4, 128], F32R, tag="tp")
                    for c in range(4):
                        nc.tensor.transpose(tp[:, c, :], xn[:, c * 128:(c + 1) * 128], ident)
                    nc.scalar.copy(xtp[:, :, t * 128:(t + 1) * 128], tp)

                def emit_qkv_half(state, half):
                    # one 256-token half of QKV^T: the first half only needs
                    # token blocks t0/t1, so the PE starts projecting while
                    # t2/t3 are still normalizing. 256-free keeps f32r at
                    # 1 cycle/row.
                    xtp = state["xt"]
                    lo, hi = (0, 256) if half == 0 else (256, 512)
                    for idx, wtile in enumerate((wv_sb, wq_sb, wk_sb)):
                        if half == 0:
                            state["ps"].append(qkps.tile(
                                [128, 512], F32, tag="qk", name=f"qk{idx}"))
                        ps = state["ps"][idx]
                        for c in range(4):
                            # the bank-zeroing start is only on the first
                            # matmul of the bank (it zeroes both halves)
                            nc.tensor.matmul(
                                ps[:, lo:hi], wtile[:, c, :], xtp[:, c, lo:hi],
                                start=(half == 0 and c == 0),
                                stop=(half == 1 and c == 3),
                                skip_group_check=True)

                def emit_qkv_drain(icp, state):
                    vt = abp.tile([128, 512], BF16, tag="vt")
                    nc.scalar.copy(vt, state["ps"][0])
                    # ACT drains these: DVE is the phase-A critical path
                    nc.scalar.copy(QT[:, icp * 512:(icp + 1) * 512], state["ps"][1])
                    nc.scalar.copy(KT[:, icp * 512:(icp + 1) * 512], state["ps"][2])
                    # V^T -> V natural (bf16) into Vb (j on partitions)
                    vtp = abps.tile([128, 4, 128], BF16, tag="tp")
                    for jl in range(4):
                        nc.tensor.transpose(vtp[:, jl, :], vt[:, jl * 128:(jl + 1) * 128], identb)
                    nc.vector.tensor_copy(
                        Vb[:, icp * 4:(icp + 1) * 4, :, 0:64],
                        vtp.rearrange("p jl (h v) -> p jl h v", h=2))

                # one-chunk software pipeline: stats chains for chunk ic run
                # while chunk ic-1 is normalized/transposed/projected, so the
                # PE never waits on the DVE->ACT->DVE norm chain
                prev = None
                for ic in range(NIC + 1):
                    if ic < NIC:
                        if ic == 0:
                            tok4 = tok4_first
                        else:
                            tok4 = abp.tile([128, 4, 512], F32, tag="tok4")
                            nc.sync.dma_start(out=tok4, in_=tok_r[ic])
                        cur_rstds = []
                        cur_state = {"ic": ic, "tok4": tok4,
                                     "rstds": cur_rstds, "ps": [],
                                     "xt": abp.tile([128, 4, 512], F32R,
                                                    tag="xt", name=f"xt{ic}")}
                    for t in range(4):
                        if prev is not None:
                            emit_stage2_t(prev["ic"], prev["tok4"],
                                          prev["rstds"], prev["xt"], t)
                            if t == 1:
                                emit_qkv_half(prev, 0)
                        if ic < NIC:
                            cur_rstds.append(emit_stats_chain(tok4, t))
                            if ic == 0:
                                # chunk 0: normalize inline, right behind its
                                # own stats chain
                                emit_stage2_t(0, tok4, cur_rstds,
                                              cur_state["xt"], t)
                                if t == 1:
                                    emit_qkv_half(cur_state, 0)
                    if prev is not None:
                        emit_qkv_half(prev, 1)
                        emit_qkv_drain(prev["ic"], prev)
                        prev = None
                    if ic == 0:
                        emit_qkv_half(cur_state, 1)
                        emit_qkv_drain(0, cur_state)
                    elif ic < NIC:
                        prev = cur_state

            # ---- phase C: attention + out-proj (j-tile pairs) ----
            with tc.tile_pool(name="c_sbuf", bufs=3) as cp, \
                 tc.tile_pool(name="pt_pool", bufs=6) as ptp, \
                 tc.tile_pool(name="opair_pool", bufs=2) as oprp, \
                 tc.tile_pool(name="outsb_pool", bufs=2) as outsbp, \
                 tc.tile_pool(name="st_psum", bufs=3, space="PSUM") as stps, \
                 tc.tile_pool(name="o_psum", bufs=2, space="PSUM") as ops:

                iters = [(ib, hl) for ib in range(NIC) for hl in range(2)]
                NIT = len(iters)
                LOOK = 2  # S^T pairs emitted ahead of exp/PV

                def emit_st(k):
                    """S^T for j-tile pair p of iteration k//NPAIR."""
                    it_idx, p = divmod(k, NPAIR)
                    ib, hl = iters[it_idx]
                    h0 = hl * 64
                    st = stps.tile([128, 2, 512], F32, tag="st", name="stg")
                    for jl in range(2):
                        jt = 2 * p + jl
                        nc.tensor.matmul(
                            st[:, jl, :],
                            KT[h0:h0 + 64, jt * 128:(jt + 1) * 128],
                            QT[h0:h0 + 64, ib * 512:(ib + 1) * 512],
                            start=True, stop=True)
                    return st

                def emit_exp(st, p, it_idx):
                    pt = ptp.tile([128, 2, 512], BF16, tag="pt", name="ptg")
                    e = (ENG_EVEN if it_idx % 2 == 0 else ENG_ODD)[p]
                    if e == "A":
                        nc.scalar.activation(
                            pt.rearrange("a b c -> a (b c)"),
                            st.rearrange("a b c -> a (b c)"), AF.Exp)
                    else:
                        nc.vector.tensor_scalar(
                            pt.bitcast(I16).rearrange("a b c -> a (b c)"),
                            st.rearrange("a b c -> a (b c)"), C1B, C2B,
                            op0=ALU.mult, op1=ALU.add)
                    return pt

                # tail state
                o_accs = {}      # iter idx -> o_acc psum tile
                o_pairs = {}     # ib -> o_pair sbuf tile
                linvs = {}       # iter idx -> linv tile
                out_sbs = {}     # ib -> out_sb tile
                ot_tiles = {}    # ib -> (ot psum tile, ots sbuf tile)
                op_tiles = {}    # it -> out-proj psum tile (transient per ib)

                def emit_tail_step(k_prev, g):
                    """Interleave iter k_prev's tail into iter k_prev+1's
                    pair-step stream at slot g (14 usable slots)."""
                    ib, hl = iters[k_prev]
                    if g == 0:
                        o_acc = o_accs[k_prev]
                        linv = cp.tile([128, 4], F32, tag="linv", name=f"lv{hl}")
                        nc.vector.reciprocal(linv, o_acc[:, :, 64])
                        linvs[k_prev] = linv
                        if hl == 0:
                            o_pairs[ib] = oprp.tile(
                                [128, 4, 128], BF16, tag="opair", name=f"op{ib}")
                    elif 1 <= g <= 4:
                        it = g - 1
                        o_acc = o_accs[k_prev]
                        nc.vector.tensor_scalar_mul(
                            o_pairs[ib][:, it, hl * 64:(hl + 1) * 64],
                            in0=o_acc[:, it, 0:64],
                            scalar1=linvs[k_prev][:, it:it + 1])
                        if it == 3:
                            o_accs.pop(k_prev)
                            linvs.pop(k_prev)
                    elif hl == 1 and g == 5:
                        # transpose the 4 normalized [i,128] pair-tiles into a
                        # single psum bank (start only zeroes on the first);
                        # back-to-back to keep the ring slot held briefly
                        opr = o_pairs[ib]
                        ot = stps.tile([128, 4, 128], BF16, tag="st", name="otp")
                        ot_tiles[ib] = [ot, None]
                        for it in range(4):
                            nc.tensor.matmul(
                                ot[:, it, :], opr[:, it, :], identb,
                                is_transpose=True, start=(it == 0), stop=(it == 3),
                                skip_group_check=True)
                    elif hl == 1 and g == 6:
                        # ACT does this drain: a copy on DVE here delays its
                        # next exp pairs and stalls the PE
                        ots = cp.tile([128, 4, 128], BF16, tag="ots", name="otsb")
                        nc.scalar.copy(ots, ot_tiles[ib][0])
                        ot_tiles[ib][1] = ots
                        o_pairs.pop(ib)
                    elif hl == 1 and 7 <= g <= 11:
                        # g=7..10: out-proj matmul it=g-7; g=8..11: drain it=g-8
                        if g >= 8:
                            it_d = g - 8
                            nc.scalar.copy(out_sbs[ib][:, it_d, :],
                                           op_tiles.pop(it_d))
                            # per-it DMA so the store overlaps later drains
                            nc.sync.dma_start(out=out_r[ib, :, it_d, :],
                                              in_=out_sbs[ib][:, it_d, :])
                            if it_d == 3:
                                out_sbs.pop(ib)
                                ot_tiles.pop(ib)
                        if g <= 10:
                            it = g - 7
                            if it == 0:
                                out_sbs[ib] = outsbp.tile(
                                    [128, 4, 512], F32, tag="outsb", name=f"ob{ib}")
                            op_ps = stps.tile([128, 512], F32, tag="st", name="opps")
                            nc.tensor.matmul(op_ps, ot_tiles[ib][1][:, it, :],
                                             wo_sb, start=True, stop=True)
                            op_tiles[it] = op_ps

                def emit_pv(it_idx, p, pt):
                    _, hl = iters[it_idx]
                    if p == 0:
                        o_accs[it_idx] = ops.tile(
                            [128, 4, 65], F32, tag="o", name=f"oacc{it_idx % 2}")
                    o_acc = o_accs[it_idx]
                    for jl in range(2):
                        jt = 2 * p + jl
                        for it in range(4):
                            # PSUM `start` zeroes the WHOLE bank: only the
                            # very first matmul of this o_acc bank may set it
                            nc.tensor.matmul(
                                o_acc[:, it, :],
                                pt[:, jl, it * 128:(it + 1) * 128],
                                Vb[:, jt, hl, :],
                                start=(jt == 0 and it == 0),
                                stop=(jt == NJT - 1 and it == 3),
                                skip_group_check=True)

                # ---- attention pipeline over (iteration, j-tile pair) steps:
                # PV lags exp by one step so every exp has an extra pair-step
                # of slack before the PE consumes its output ----
                TOT = NIT * NPAIR
                sts = {}
                pts = {}
                for k in range(LOOK):
                    sts[k] = emit_st(k)
                for k in range(TOT + 2):
                    if k < TOT:
                        it_idx, p = divmod(k, NPAIR)
                        pts[k] = emit_exp(sts.pop(k), p, it_idx)
                        if k + LOOK < TOT:
                            sts[k + LOOK] = emit_st(k + LOOK)
                    if k >= 2:
                        emit_pv(*divmod(k - 2, NPAIR), pts.pop(k - 2))
                    if k < TOT and it_idx > 0 and p >= 2:
                        # tail slots start at p==2: the previous iteration's
                        # final (2-step-lagged) PV lands at p==1, and the
                        # tail's linv must observe it
                        emit_tail_step(it_idx - 1, p - 2)

                # trailing tail for the final iteration
                for g in range(NPAIR):
                    emit_tail_step(NIT - 1, g)

    fix_waits_nc(nc, mybir)
    return nc


def fix_waits_nc(nc, mybir):
    """Post-pass over the scheduled program: (1) remove semaphore waits that
    are transitively implied by earlier waits (Tile emits per-proc-minimal,
    not transitively-minimal, waits), (2) split any instruction still
    carrying more than one wait by injecting single-wait NoOps in front of
    it — this walrus build rejects >1 sync wait per instruction.
    Mutates nc in place so CoreSim and hardware run identical sync."""
    nop_id = [0]

    def _is_ge(w):
        return w.sync_type == "semaphore" and w.wait_mode == "sem-ge-imm"

    for fn in nc.m.functions:
        for blk in fn.blocks:
            insts = list(blk.instructions)
            n = len(insts)

            producers = {}
            cum = {}
            nonmono = set()  # sems ever decremented: counter logic invalid
            for idx, inst in enumerate(insts):
                si = inst.sync_info
                for u in (si.on_update if si else []) or []:
                    if u.sync_type != "semaphore":
                        continue
                    sid = u.id
                    if u.update_mode != "sem-inc":
                        nonmono.add(sid)
                        continue
                    cum[sid] = cum.get(sid, 0) + int(u.update_value)
                    producers.setdefault(sid, []).append((cum[sid], idx))

            def producer_of(sid, val):
                for cv, idx in producers.get(sid, ()):
                    if cv >= val:
                        return idx
                return None

            prev_eng = [None] * n
            last = {}
            for idx, inst in enumerate(insts):
                e = inst.engine
                prev_eng[idx] = last.get(e)
                last[e] = idx

            def get_waits(inst):
                si = inst.sync_info
                return list(si.on_wait) if si and si.on_wait else []

            def is_ge(w):
                return _is_ge(w) and w.id not in nonmono

            know = [dict() for _ in range(n)]
            for _ in range(3):
                changed = False
                for idx, inst in enumerate(insts):
                    k = dict(know[prev_eng[idx]]) if prev_eng[idx] is not None else {}
                    for w in get_waits(inst):
                        if not is_ge(w):
                            continue
                        sid, val = w.id, int(w.wait_value)
                        if k.get(sid, -1) < val:
                            k[sid] = val
                        p = producer_of(sid, val)
                        if p is not None:
                            for s2, v2 in know[p].items():
                                if k.get(s2, -1) < v2:
                                    k[s2] = v2
                    if k != know[idx]:
                        know[idx] = k
                        changed = True
                if not changed:
                    break

            new_insts = []
            dirty = False
            for idx, inst in enumerate(insts):
                si = inst.sync_info
                waits = get_waits(inst)
                if si is not None and waits:
                    base = dict(know[prev_eng[idx]]) if prev_eng[idx] is not None else {}
                    kept = []
                    for w in waits:
                        if is_ge(w):
                            sid, val = w.id, int(w.wait_value)
                            if base.get(sid, -1) >= val:
                                continue
                            base[sid] = val
                            p = producer_of(sid, val)
                            if p is not None:
                                for s2, v2 in know[p].items():
                                    if base.get(s2, -1) < v2:
                                        base[s2] = v2
                        kept.append(w)
                    if len(kept) != len(waits) or len(kept) > 1:
                        dirty = True
                        for w in kept[:-1]:
                            nop_id[0] += 1
                            nop = mybir.InstNoOp(
                                name=f"I-waitfix-{nop_id[0]}", ins=[], outs=[])
                            nop.engine = inst.engine
                            nop.sync_info = mybir.SyncInfo(on_wait=[w], on_update=[])
                            nc.register_instruction(nop)
                            new_insts.append(nop)
                        inst.sync_info = mybir.SyncInfo(
                            on_wait=kept[-1:],
                            on_update=list(si.on_update or []))
                new_insts.append(inst)
            if dirty:
                blk.instructions = new_insts


def get_program():
    if "nc" not in _prog_cache:
        _prog_cache["nc"] = build_program()
    return _prog_cache["nc"]


def _prep_inputs(tokens, norm_weight, w_qkv, w_out):
    import ml_dtypes
    tokens = np.ascontiguousarray(np.asarray(tokens, dtype=np.float32))
    norm_weight = np.asarray(norm_weight, dtype=np.float32)
    w_qkv = np.asarray(w_qkv, dtype=np.float32)
    w_out = np.asarray(w_out, dtype=np.float32)

    wp = w_qkv * norm_weight[:, None]  # fold RMSNorm weight into qkv weights

    in_maps = []
    for c in range(NCORES):
        b = c // 4
        h0 = 2 * (c % 4)
        m = {}
        m["tok"] = tokens[b]
        for name, off in (("wq", 0), ("wk", DI), ("wv", 2 * DI)):
            w = wp[:, off + h0 * DH: off + (h0 + 2) * DH]       # [512, 128]
            m[name] = np.ascontiguousarray(
                w.reshape(4, 128, 128).transpose(1, 0, 2))       # [128, 4, 128]
        rows = w_out[h0 * DH:(h0 + 2) * DH, :]                   # [128, 512]
        m["wo"] = np.ascontiguousarray(rows.astype(ml_dtypes.bfloat16))
        in_maps.append(m)
    return in_maps


def run(tokens, norm_weight, w_qkv, w_out, trace=False):
    from concourse.bass_utils import run_bass_kernel_spmd
    nc = get_program()
    in_maps = _prep_inputs(tokens, norm_weight, w_qkv, w_out)
    res = run_bass_kernel_spmd(nc, in_maps, core_ids=list(range(NCORES)), trace=trace)
    parts = [res.results[c]["out_part"] for c in range(NCORES)]
    out = np.empty((B, N, D), dtype=np.float32)
    for b in range(B):
        out[b] = parts[4 * b] + parts[4 * b + 1] + parts[4 * b + 2] + parts[4 * b + 3]
    return out, res


def kernel(tokens, norm_weight, w_qkv, w_out):
    out, _ = run(tokens, norm_weight, w_qkv, w_out, trace=False)
    return out


# revision 44
# speedup vs baseline: 1.3904x; 1.0067x over previous
"""TRN2 Bass kernel for nn_Attention (RMSNorm + QKV + softmax attention + out-proj).

Sharding: 8 cores = 2 batches x 4 head-pairs. Core c handles batch c//4 and
heads (2*(c%4), 2*(c%4)+1). Each core computes its partial out-projection
(contracting only its 128 rows of dim_inner); host sums the 4 partials per batch.

V2 design (cost-model driven):
  A) stream tokens [128,512] blocks: RMSNorm stats on DVE, rstd scale, PE
     transpose -> x^T (rotating [128,4,512] per 512-token chunk)
  B) QKV^T = w^T @ x^T (PE); V^T -> bf16 -> transposed back to V-natural with
     a ones column appended per head (softmax denominator for free)
  C) flash attention per (ib=512 queries, head): for each j-tile of 128 keys:
     S^T [128j, 512i] on PE (f32r, 512 cycles); exp split across THREE engines
     (ACT: exact table exp -> bf16; DVE/GPSIMD: Schraudolph bit-trick exp via
     tensor_scalar -> int16 bits of bf16). PV uses exp(S)^T tiles as the
     STATIONARY [128j, 128i] and bf16 V[128j,64+1] as MOVING: 65-cycle
     matmuls accumulate O-natural [128i, 65] in PSUM (col 64 = l).
  D) tail: linv=1/l (DVE), scale O by linv -> bf16 pair tile [i, 128dh both
     heads] (DVE), PE transpose [128,128], one 128-contraction out-proj matmul
     per i-tile vs bf16 w_out, ACT drains psum->sbuf, DMA out per ib.

Engine budget per core (cost model): PE ~206us (bottleneck), ACT ~150us,
DVE ~180us, Pool ~110us. exp assignment per j-tile keeps all three exp
engines concurrently busy so PE never starves.

Schraudolph exp: bf16_bits(p) = int16(S * (2^7/ln2) + C2B). Bias component
cancels through the softmax denominator; residual sawtooth ~1% std on the
offloaded fraction keeps total rel err well under the 2e-2 gate.

Engine discipline: walrus here allows only ONE semaphore wait per instruction
(fix_waits_nc post-pass splits/dedups), and no mixed 32/16-bit matmuls.
"""
import sys
sys.path.insert(0, "/opt/trn_rl_repo")
import numpy as np

B, N, D = 2, 4096, 512
H, DH = 8, 64
DI = H * DH
NCORES = 8
EPS = 1.1920929e-07  # float32 eps (torch nn.RMSNorm default)

# Schraudolph constants for bf16-bit exp: bits16(e^x) ~= x*C1B + C2B
C1B = 128.0 / 0.6931471805599453          # 2^7 / ln 2
C2B = float(127 * 128) - 366393.0 / 65536.0

_prog_cache = {}


def _patch_drain(tile_mod, mybir):
    """Split the multi-wait tail drain into a chain of single-wait drains
    (this walrus build rejects >1 sync wait per instruction)."""
    if getattr(tile_mod.TileContext, "_drain_patched", False):
        return

    def _patched(self, tick_clock, wait_clock):
        from concourse.vector_clock import ScopedClock
        nc = self.nc
        drain_inst = nc.sync.drain()
        wait_clock.add_sem_waits(drain_inst.ins, ScopedClock({None: tick_clock.global_clock}))
        si = drain_inst.ins.sync_info
        if si is not None and si.on_wait and len(si.on_wait) > 1:
            waits = list(si.on_wait)
            drain_inst.ins.sync_info = mybir.SyncInfo(
                on_wait=waits[:1], on_update=list(si.on_update or []))
            for w in waits[1:]:
                d2 = nc.sync.drain()
                d2.ins.sync_info = mybir.SyncInfo(on_wait=[w], on_update=[])
        nc.all_engine_barrier()
        assert self.sems is not None
        popped = nc._tile_sem_poison_stack.pop()
        assert popped is self._sem_poison
        nc.clear_and_free_semaphores(list(self.sems.allocated().values()))
        nc.all_engine_barrier()

    tile_mod.TileContext._drain_and_barrier = _patched
    tile_mod.TileContext._drain_patched = True


def build_program():
    import concourse.bass as bass
    import concourse.tile as tile
    from concourse import mybir
    from concourse.masks import make_identity

    _patch_drain(tile, mybir)

    F32 = mybir.dt.float32
    F32R = mybir.dt.float32r
    BF16 = mybir.dt.bfloat16
    I16 = mybir.dt.int16
    AF = mybir.ActivationFunctionType
    ALU = mybir.AluOpType

    NIC = N // 512          # 8 chunks of 512 tokens
    NJT = N // 128          # 32 key tiles of 128

    nc = bass.Bass(trn_type="TRN2", target_bir_lowering=False)

    tok = nc.dram_tensor("tok", [N, D], F32, kind="ExternalInput")
    wq = nc.dram_tensor("wq", [128, 4, 128], F32R, kind="ExternalInput")
    wk = nc.dram_tensor("wk", [128, 4, 128], F32R, kind="ExternalInput")
    wv = nc.dram_tensor("wv", [128, 4, 128], F32R, kind="ExternalInput")
    wo = nc.dram_tensor("wo", [128, 512], BF16, kind="ExternalInput")
    out_part = nc.dram_tensor("out_part", [N, D], F32, kind="ExternalOutput")

    tok_r = tok.rearrange("(ic t p) d -> ic p t d", t=4, p=128)
    out_r = out_part.rearrange("(ib t p) e -> ib p t e", t=4, p=128)

    # exp-engine assignment per j-tile PAIR within an iteration (A=ACT exact,
    # D=DVE Schraudolph), weighted so both engines finish together just
    # below the PE's per-iteration time. GPSIMD cannot read PSUM, so it
    # cannot join the exp crew; it takes phase-A SBUF work instead.
    NPAIR = NJT // 2
    def _mk_eng(w_a, w_d):
        pat, cnt = [], {"A": 0, "D": 0}
        wgt = {"A": w_a, "D": w_d}
        for _ in range(NPAIR):
            e = min(cnt, key=lambda k: (cnt[k] + 1) / wgt[k])
            pat.append(e)
            cnt[e] += 1
        return pat
    # alternate the split between iterations so ACT (which also does the
    # out-proj drains) keeps headroom below the PE's per-iteration pace
    ENG_EVEN = _mk_eng(9, 7)
    ENG_ODD = _mk_eng(8, 8)

    with tile.TileContext(nc) as tc:
        with tc.tile_pool(name="consts", bufs=1) as consts, \
             tc.tile_pool(name="big", bufs=1) as big, \
             tc.tile_pool(name="wpool", bufs=1) as wpool:

            # ---- constants ----
            ident_f = consts.tile([128, 128], F32)
            make_identity(nc, ident_f)
            ident = consts.tile([128, 128], F32R)
            nc.vector.tensor_copy(ident, ident_f)
            identb = consts.tile([128, 128], BF16)
            nc.vector.tensor_copy(identb, ident_f)
            eps_t = consts.tile([128, 1], F32)
            nc.vector.memset(eps_t, EPS)
            # preload the Exp ACT table during the otherwise-idle start window
            warm = consts.tile([1, 1], F32)
            nc.scalar.activation(warm, eps_t[0:1, :], AF.Exp)

            # ---- weights ----
            # first token chunk goes ahead of the weights in the DMA queue so
            # the RMSNorm pipeline starts as early as possible
            tok4_first = wpool.tile([128, 4, 512], F32)
            for t in range(4):
                nc.sync.dma_start(out=tok4_first[:, t, :], in_=tok_r[0, :, t, :])
            wq_sb = wpool.tile([128, 4, 128], F32R)
            wk_sb = wpool.tile([128, 4, 128], F32R)
            wv_sb = wpool.tile([128, 4, 128], F32R)
            wo_sb = wpool.tile([128, 512], BF16)
            nc.sync.dma_start(out=wq_sb, in_=wq[:, :, :])
            nc.sync.dma_start(out=wk_sb, in_=wk[:, :, :])
            nc.sync.dma_start(out=wv_sb, in_=wv[:, :, :])
            nc.sync.dma_start(out=wo_sb, in_=wo[:, :])

            # ---- persistent big buffers ----
            QT = big.tile([128, N], F32R)       # [2 heads x 64 qdims, n]
            KT = big.tile([128, N], F32R)
            Vb = big.tile([128, NJT, 2, 65], BF16)  # per j-tile: [v(64)|ones] per head
            ones_bf = consts.tile([128, NJT, 2, 1], BF16)
            nc.vector.memset(ones_bf, 1.0)
            nc.vector.tensor_copy(Vb[:, :, :, 64:65], ones_bf)

            # ---- phase A/B: RMSNorm + x^T + QKV^T + V natural ----
            with tc.tile_pool(name="ab_sbuf", bufs=3) as abp, \
                 tc.tile_pool(name="ab_stats", bufs=8) as stp, \
                 tc.tile_pool(name="ab_psum", bufs=5, space="PSUM") as abps, \
                 tc.tile_pool(name="qk_psum", bufs=3, space="PSUM") as qkps:

                # PE joins: absorb each weight-DMA semaphore with a tiny bf16 matmul
                scr = qkps.tile([2, 2], F32, tag="qk", name="scrj")
                BF = mybir.dt.bfloat16
                for i, wtile in enumerate((wq_sb, wk_sb, wv_sb)):
                    src = wtile[0:1, 0:1, 0:2].bitcast(BF)[:, 0, 1::2]
                    nc.tensor.matmul(scr, src, src, start=(i == 0), stop=False)
                nc.tensor.matmul(scr, wo_sb[0:1, 0:2], wo_sb[0:1, 0:2],
                                 start=False, stop=True)

                def emit_stats_chain(tok4, t):
                    """RMSNorm rstd for one 128-token block."""
                    stats = stp.tile([128, 6], F32, tag="stats")
                    mv = stp.tile([128, 2], F32, tag="mv")
                    ms = stp.tile([128, 1], F32, tag="ms")
                    s_t = stp.tile([128, 1], F32, tag="s_t")
                    rstd = stp.tile([128, 1], F32, tag="rstd")
                    nc.vector.bn_stats(stats, tok4[:, t, :])
                    nc.vector.bn_aggr(mv, stats)
                    # E[x^2] = mean^2 + var
                    nc.vector.scalar_tensor_tensor(
                        ms, mv[:, 0:1], mv[:, 0:1], mv[:, 1:2],
                        op0=ALU.mult, op1=ALU.add)
                    nc.scalar.activation(s_t, ms, AF.Sqrt, bias=eps_t, scale=1.0)
                    nc.vector.reciprocal(rstd, s_t)
                    return rstd

                def emit_stage2_t(icp, tok4p, rstds, xtp, t):
                    """normalize + transpose one 128-token block of the
                    PREVIOUS chunk (its rstd is long since ready)."""
                    xn = stp.tile([128, 512], F32R, tag="xn")
                    eng = nc.vector if t % 2 == 0 else nc.gpsimd
                    eng.tensor_scalar_mul(xn, in0=tok4p[:, t, :], scalar1=rstds[t])
                    tp = abps.tile([128, 4, 128], F32R, tag="tp")
                    for c in range(4):
                        nc.tensor.transpose(tp[:, c, :], xn[:, c * 128:(c + 1) * 128], ident)
                    nc.scalar.copy(xtp[:, :, t * 128:(t + 1) * 128], tp)

                def emit_qkv_half(state, half):
                    # one 256-token half of QKV^T: the first half only needs
                    # token blocks t0/t1, so the PE starts projecting while
                    # t2/t3 are still normalizing. 256-free keeps f32r at
                    # 1 cycle/row.
                    xtp = state["xt"]
                    lo, hi = (0, 256) if half == 0 else (256, 512)
                    for idx, wtile in enumerate((wv_sb, wq_sb, wk_sb)):
                        if half == 0:
                            state["ps"].append(qkps.tile(
                                [128, 512], F32, tag="qk", name=f"qk{idx}"))
                        ps = state["ps"][idx]
                        for c in range(4):
                            # the bank-zeroing start is only on the first
                            # matmul of the bank (it zeroes both halves)
                            nc.tensor.matmul(
                                ps[:, lo:hi], wtile[:, c, :], xtp[:, c, lo:hi],
                                start=(half == 0 and c == 0),
                                stop=(half == 1 and c == 3),
                                skip_group_check=True)

                def emit_qkv_drain(icp, state):
                    vt = abp.tile([128, 512], BF16, tag="vt")
                    nc.scalar.copy(vt, state["ps"][0])
                    # ACT drains these: DVE is the phase-A critical path
                    nc.scalar.copy(QT[:, icp * 512:(icp + 1) * 512], state["ps"][1])
                    nc.scalar.copy(KT[:, icp * 512:(icp + 1) * 512], state["ps"][2])
                    state["vt"] = vt

                def emit_vb(icp, state):
                    # V^T -> V natural (bf16) into Vb (j on partitions);
                    # deferred into the next chunk's stream so the PE doesn't
                    # stall on the vt drain at the chunk boundary
                    vt = state["vt"]
                    vtp = abps.tile([128, 4, 128], BF16, tag="tp")
                    for jl in range(4):
                        nc.tensor.transpose(vtp[:, jl, :], vt[:, jl * 128:(jl + 1) * 128], identb)
                    nc.vector.tensor_copy(
                        Vb[:, icp * 4:(icp + 1) * 4, :, 0:64],
                        vtp.rearrange("p jl (h v) -> p jl h v", h=2))

                # one-chunk software pipeline: stats chains for chunk ic run
                # while chunk ic-1 is normalized/transposed/projected, so the
                # PE never waits on the DVE->ACT->DVE norm chain
                prev = None
                done = None
                for ic in range(NIC + 1):
                    if ic < NIC:
                        if ic == 0:
                            tok4 = tok4_first
                        else:
                            tok4 = abp.tile([128, 4, 512], F32, tag="tok4")
                            nc.sync.dma_start(out=tok4, in_=tok_r[ic])
                        cur_rstds = []
                        cur_state = {"ic": ic, "tok4": tok4,
                                     "rstds": cur_rstds, "ps": [],
                                     "xt": abp.tile([128, 4, 512], F32R,
                                                    tag="xt", name=f"xt{ic}")}
                    for t in range(4):
                        if prev is not None:
                            emit_stage2_t(prev["ic"], prev["tok4"],
                                          prev["rstds"], prev["xt"], t)
                            if t == 1:
                                emit_qkv_half(prev, 0)
                        if ic < NIC:
                            cur_rstds.append(emit_stats_chain(tok4, t))
                            if ic == 0:
                                # chunk 0: normalize inline, right behind its
                                # own stats chain
                                emit_stage2_t(0, tok4, cur_rstds,
                                              cur_state["xt"], t)
                                if t == 1:
                                    emit_qkv_half(cur_state, 0)
                        if t == 2 and done is not None:
                            emit_vb(done["ic"], done)
                            done = None
                    if prev is not None:
                        emit_qkv_half(prev, 1)
                        emit_qkv_drain(prev["ic"], prev)
                        done = prev
                        prev = None
                    if ic == 0:
                        emit_qkv_half(cur_state, 1)
                        emit_qkv_drain(0, cur_state)
                        done = cur_state
                    elif ic < NIC:
                        prev = cur_state
                if done is not None:
                    emit_vb(done["ic"], done)
                    done = None

            # ---- phase C: attention + out-proj (j-tile pairs) ----
            with tc.tile_pool(name="c_sbuf", bufs=3) as cp, \
                 tc.tile_pool(name="pt_pool", bufs=6) as ptp, \
                 tc.tile_pool(name="opair_pool", bufs=2) as oprp, \
                 tc.tile_pool(name="outsb_pool", bufs=2) as outsbp, \
                 tc.tile_pool(name="st_psum", bufs=3, space="PSUM") as stps, \
                 tc.tile_pool(name="o_psum", bufs=2, space="PSUM") as ops:

                iters = [(ib, hl) for ib in range(NIC) for hl in range(2)]
                NIT = len(iters)
                LOOK = 2  # S^T pairs emitted ahead of exp/PV

                def emit_st(k):
                    """S^T for j-tile pair p of iteration k//NPAIR."""
                    it_idx, p = divmod(k, NPAIR)
                    ib, hl = iters[it_idx]
                    h0 = hl * 64
                    st = stps.tile([128, 2, 512], F32, tag="st", name="stg")
                    for jl in range(2):
                        jt = 2 * p + jl
                        nc.tensor.matmul(
                            st[:, jl, :],
                            KT[h0:h0 + 64, jt * 128:(jt + 1) * 128],
                            QT[h0:h0 + 64, ib * 512:(ib + 1) * 512],
                            start=True, stop=True)
                    return st

                def emit_exp(st, p, it_idx):
                    pt = ptp.tile([128, 2, 512], BF16, tag="pt", name="ptg")
                    e = (ENG_EVEN if it_idx % 2 == 0 else ENG_ODD)[p]
                    if e == "A":
                        nc.scalar.activation(
                            pt.rearrange("a b c -> a (b c)"),
                            st.rearrange("a b c -> a (b c)"), AF.Exp)
                    else:
                        nc.vector.tensor_scalar(
                            pt.bitcast(I16).rearrange("a b c -> a (b c)"),
                            st.rearrange("a b c -> a (b c)"), C1B, C2B,
                            op0=ALU.mult, op1=ALU.add)
                    return pt

                # tail state
                o_accs = {}      # iter idx -> o_acc psum tile
                o_pairs = {}     # ib -> o_pair sbuf tile
                linvs = {}       # iter idx -> linv tile
                out_sbs = {}     # ib -> out_sb tile
                ot_tiles = {}    # ib -> (ot psum tile, ots sbuf tile)
                op_tiles = {}    # it -> out-proj psum tile (transient per ib)

                def emit_tail_step(k_prev, g):
                    """Interleave iter k_prev's tail into iter k_prev+1's
                    pair-step stream at slot g (14 usable slots)."""
                    ib, hl = iters[k_prev]
                    if g == 0:
                        o_acc = o_accs[k_prev]
                        linv = cp.tile([128, 4], F32, tag="linv", name=f"lv{hl}")
                        nc.vector.reciprocal(linv, o_acc[:, :, 64])
                        linvs[k_prev] = linv
                        if hl == 0:
                            o_pairs[ib] = oprp.tile(
                                [128, 4, 128], BF16, tag="opair", name=f"op{ib}")
                    elif 1 <= g <= 4:
                        it = g - 1
                        o_acc = o_accs[k_prev]
                        nc.vector.tensor_scalar_mul(
                            o_pairs[ib][:, it, hl * 64:(hl + 1) * 64],
                            in0=o_acc[:, it, 0:64],
                            scalar1=linvs[k_prev][:, it:it + 1])
                        if it == 3:
                            o_accs.pop(k_prev)
                            linvs.pop(k_prev)
                    elif hl == 1 and g == 5:
                        # transpose the 4 normalized [i,128] pair-tiles into a
                        # single psum bank (start only zeroes on the first);
                        # back-to-back to keep the ring slot held briefly
                        opr = o_pairs[ib]
                        ot = stps.tile([128, 4, 128], BF16, tag="st", name="otp")
                        ot_tiles[ib] = [ot, None]
                        for it in range(4):
                            nc.tensor.matmul(
                                ot[:, it, :], opr[:, it, :], identb,
                                is_transpose=True, start=(it == 0), stop=(it == 3),
                                skip_group_check=True)
                    elif hl == 1 and g == 6:
                        # ACT does this drain: a copy on DVE here delays its
                        # next exp pairs and stalls the PE
                        ots = cp.tile([128, 4, 128], BF16, tag="ots", name="otsb")
                        nc.scalar.copy(ots, ot_tiles[ib][0])
                        ot_tiles[ib][1] = ots
                        o_pairs.pop(ib)
                    elif hl == 1 and 7 <= g <= 11:
                        # g=7..10: out-proj matmul it=g-7; g=8..11: drain it=g-8
                        if g >= 8:
                            it_d = g - 8
                            nc.scalar.copy(out_sbs[ib][:, it_d, :],
                                           op_tiles.pop(it_d))
                            # per-it DMA so the store overlaps later drains
                            nc.sync.dma_start(out=out_r[ib, :, it_d, :],
                                              in_=out_sbs[ib][:, it_d, :])
                            if it_d == 3:
                                out_sbs.pop(ib)
                                ot_tiles.pop(ib)
                        if g <= 10:
                            it = g - 7
                            if it == 0:
                                out_sbs[ib] = outsbp.tile(
                                    [128, 4, 512], F32, tag="outsb", name=f"ob{ib}")
                            op_ps = stps.tile([128, 512], F32, tag="st", name="opps")
                            nc.tensor.matmul(op_ps, ot_tiles[ib][1][:, it, :],
                                             wo_sb, start=True, stop=True)
                            op_tiles[it] = op_ps

                def emit_pv(it_idx, p, pt):
                    _, hl = iters[it_idx]
                    if p == 0:
                        o_accs[it_idx] = ops.tile(
                            [128, 4, 65], F32, tag="o", name=f"oacc{it_idx % 2}")
                    o_acc = o_accs[it_idx]
                    for jl in range(2):
                        jt = 2 * p + jl
                        for it in range(4):
                            # PSUM `start` zeroes the WHOLE bank: only the
                            # very first matmul of this o_acc bank may set it
                            nc.tensor.matmul(
                                o_acc[:, it, :],
                                pt[:, jl, it * 128:(it + 1) * 128],
                                Vb[:, jt, hl, :],
                                start=(jt == 0 and it == 0),
                                stop=(jt == NJT - 1 and it == 3),
                                skip_group_check=True)

                # ---- attention pipeline over (iteration, j-tile pair) steps:
                # PV lags exp by one step so every exp has an extra pair-step
                # of slack before the PE consumes its output ----
                TOT = NIT * NPAIR
                sts = {}
                pts = {}
                for k in range(LOOK):
                    sts[k] = emit_st(k)
                for k in range(TOT + 2):
                    if k < TOT:
                        it_idx, p = divmod(k, NPAIR)
                        pts[k] = emit_exp(sts.pop(k), p, it_idx)
                        if k + LOOK < TOT:
                            sts[k + LOOK] = emit_st(k + LOOK)
                    if k >= 2:
                        emit_pv(*divmod(k - 2, NPAIR), pts.pop(k - 2))
                    if k < TOT and it_idx > 0 and p >= 2:
                        # tail slots start at p==2: the previous iteration's
                        # final (2-step-lagged) PV lands at p==1, and the
                        # tail's linv must observe it
                        emit_tail_step(it_idx - 1, p - 2)

                # trailing tail for the final iteration: no exp traffic to
                # pace against, so run a tight per-it pipeline instead
                kl = NIT - 1
                ibl, _ = iters[kl]
                o_acc = o_accs.pop(kl)
                linv = cp.tile([128, 4], F32, tag="linv", name="lvf")
                nc.vector.reciprocal(linv, o_acc[:, :, 64])
                oprl = o_pairs[ibl]
                otl = stps.tile([128, 4, 128], BF16, tag="st", name="otpf")
                out_sbl = outsbp.tile([128, 4, 512], F32, tag="outsb", name="obf")
                opsl = {}
                for it in range(4):
                    nc.vector.tensor_scalar_mul(
                        oprl[:, it, 64:128], in0=o_acc[:, it, 0:64],
                        scalar1=linv[:, it:it + 1])
                    nc.tensor.matmul(
                        otl[:, it, :], oprl[:, it, :], identb,
                        is_transpose=True, start=(it == 0), stop=(it == 3),
                        skip_group_check=True)
                    # per-it ots copy so each out-proj starts immediately
                    otsl = cp.tile([128, 128], BF16, tag="ots", name=f"otsf{it}")
                    nc.scalar.copy(otsl, otl[:, it, :])
                    op_ps = stps.tile([128, 512], F32, tag="st", name=f"opf{it}")
                    nc.tensor.matmul(op_ps, otsl, wo_sb, start=True, stop=True)
                    opsl[it] = op_ps
                    if it >= 1:
                        nc.scalar.copy(out_sbl[:, it - 1, :], opsl.pop(it - 1))
                        nc.sync.dma_start(out=out_r[ibl, :, it - 1, :],
                                          in_=out_sbl[:, it - 1, :])
                nc.scalar.copy(out_sbl[:, 3, :], opsl.pop(3))
                nc.sync.dma_start(out=out_r[ibl, :, 3, :], in_=out_sbl[:, 3, :])

    fix_waits_nc(nc, mybir)
    return nc


def fix_waits_nc(nc, mybir):
    """Post-pass over the scheduled program: (1) remove semaphore waits that
    are transitively implied by earlier waits (Tile emits per-proc-minimal,
    not transitively-minimal, waits), (2) split any instruction still
    carrying more than one wait by injecting single-wait NoOps in front of
    it — this walrus build rejects >1 sync wait per instruction.
    Mutates nc in place so CoreSim and hardware run identical sync."""
    nop_id = [0]

    def _is_ge(w):
        return w.sync_type == "semaphore" and w.wait_mode == "sem-ge-imm"

    for fn in nc.m.functions:
        for blk in fn.blocks:
            insts = list(blk.instructions)
            n = len(insts)

            producers = {}
            cum = {}
            nonmono = set()  # sems ever decremented: counter logic invalid
            for idx, inst in enumerate(insts):
                si = inst.sync_info
                for u in (si.on_update if si else []) or []:
                    if u.sync_type != "semaphore":
                        continue
                    sid = u.id
                    if u.update_mode != "sem-inc":
                        nonmono.add(sid)
                        continue
                    cum[sid] = cum.get(sid, 0) + int(u.update_value)
                    producers.setdefault(sid, []).append((cum[sid], idx))

            def producer_of(sid, val):
                for cv, idx in producers.get(sid, ()):
                    if cv >= val:
                        return idx
                return None

            prev_eng = [None] * n
            last = {}
            for idx, inst in enumerate(insts):
                e = inst.engine
                prev_eng[idx] = last.get(e)
                last[e] = idx

            def get_waits(inst):
                si = inst.sync_info
                return list(si.on_wait) if si and si.on_wait else []

            def is_ge(w):
                return _is_ge(w) and w.id not in nonmono

            know = [dict() for _ in range(n)]
            for _ in range(3):
                changed = False
                for idx, inst in enumerate(insts):
                    k = dict(know[prev_eng[idx]]) if prev_eng[idx] is not None else {}
                    for w in get_waits(inst):
                        if not is_ge(w):
                            continue
                        sid, val = w.id, int(w.wait_value)
                        if k.get(sid, -1) < val:
                            k[sid] = val
                        p = producer_of(sid, val)
                        if p is not None:
                            for s2, v2 in know[p].items():
                                if k.get(s2, -1) < v2:
                                    k[s2] = v2
                    if k != know[idx]:
                        know[idx] = k
                        changed = True
                if not changed:
                    break

            new_insts = []
            dirty = False
            for idx, inst in enumerate(insts):
                si = inst.sync_info
                waits = get_waits(inst)
                if si is not None and waits:
                    base = dict(know[prev_eng[idx]]) if prev_eng[idx] is not None else {}
                    kept = []
                    for w in waits:
                        if is_ge(w):
                            sid, val = w.id, int(w.wait_value)
                            if base.get(sid, -1) >= val:
                                continue
                            base[sid] = val
                            p = producer_of(sid, val)
                            if p is not None:
                                for s2, v2 in know[p].items():
                                    if base.get(s2, -1) < v2:
                                        base[s2] = v2
                        kept.append(w)
                    if len(kept) != len(waits) or len(kept) > 1:
                        dirty = True
                        for w in kept[:-1]:
                            nop_id[0] += 1
                            nop = mybir.InstNoOp(
                                name=f"I-waitfix-{nop_id[0]}", ins=[], outs=[])
                            nop.engine = inst.engine
                            nop.sync_info = mybir.SyncInfo(on_wait=[w], on_update=[])
                            nc.register_instruction(nop)
                            new_insts.append(nop)
                        inst.sync_info = mybir.SyncInfo(
                            on_wait=kept[-1:],
                            on_update=list(si.on_update or []))
                new_insts.append(inst)
            if dirty:
                blk.instructions = new_insts


def get_program():
    if "nc" not in _prog_cache:
        _prog_cache["nc"] = build_program()
    return _prog_cache["nc"]


def _prep_inputs(tokens, norm_weight, w_qkv, w_out):
    import ml_dtypes
    tokens = np.ascontiguousarray(np.asarray(tokens, dtype=np.float32))
    norm_weight = np.asarray(norm_weight, dtype=np.float32)
    w_qkv = np.asarray(w_qkv, dtype=np.float32)
    w_out = np.asarray(w_out, dtype=np.float32)

    wp = w_qkv * norm_weight[:, None]  # fold RMSNorm weight into qkv weights

    in_maps = []
    for c in range(NCORES):
        b = c // 4
        h0 = 2 * (c % 4)
        m = {}
        m["tok"] = tokens[b]
        for name, off in (("wq", 0), ("wk", DI), ("wv", 2 * DI)):
            w = wp[:, off + h0 * DH: off + (h0 + 2) * DH]       # [512, 128]
            m[name] = np.ascontiguousarray(
                w.reshape(4, 128, 128).transpose(1, 0, 2))       # [128, 4, 128]
        rows = w_out[h0 * DH:(h0 + 2) * DH, :]                   # [128, 512]
        m["wo"] = np.ascontiguousarray(rows.astype(ml_dtypes.bfloat16))
        in_maps.append(m)
    return in_maps


def run(tokens, norm_weight, w_qkv, w_out, trace=False):
    from concourse.bass_utils import run_bass_kernel_spmd
    nc = get_program()
    in_maps = _prep_inputs(tokens, norm_weight, w_qkv, w_out)
    res = run_bass_kernel_spmd(nc, in_maps, core_ids=list(range(NCORES)), trace=trace)
    parts = [res.results[c]["out_part"] for c in range(NCORES)]
    out = np.empty((B, N, D), dtype=np.float32)
    for b in range(B):
        out[b] = parts[4 * b] + parts[4 * b + 1] + parts[4 * b + 2] + parts[4 * b + 3]
    return out, res


def kernel(tokens, norm_weight, w_qkv, w_out):
    out, _ = run(tokens, norm_weight, w_qkv, w_out, trace=False)
    return out


# revision 45
# speedup vs baseline: 1.4064x; 1.0116x over previous
"""TRN2 Bass kernel for nn_Attention (RMSNorm + QKV + softmax attention + out-proj).

Sharding: 8 cores = 2 batches x 4 head-pairs. Core c handles batch c//4 and
heads (2*(c%4), 2*(c%4)+1). Each core computes its partial out-projection
(contracting only its 128 rows of dim_inner); host sums the 4 partials per batch.

V2 design (cost-model driven):
  A) stream tokens [128,512] blocks: RMSNorm stats on DVE, rstd scale, PE
     transpose -> x^T (rotating [128,4,512] per 512-token chunk)
  B) QKV^T = w^T @ x^T (PE); V^T -> bf16 -> transposed back to V-natural with
     a ones column appended per head (softmax denominator for free)
  C) flash attention per (ib=512 queries, head): for each j-tile of 128 keys:
     S^T [128j, 512i] on PE (f32r, 512 cycles); exp split across THREE engines
     (ACT: exact table exp -> bf16; DVE/GPSIMD: Schraudolph bit-trick exp via
     tensor_scalar -> int16 bits of bf16). PV uses exp(S)^T tiles as the
     STATIONARY [128j, 128i] and bf16 V[128j,64+1] as MOVING: 65-cycle
     matmuls accumulate O-natural [128i, 65] in PSUM (col 64 = l).
  D) tail: linv=1/l (DVE), scale O by linv -> bf16 pair tile [i, 128dh both
     heads] (DVE), PE transpose [128,128], one 128-contraction out-proj matmul
     per i-tile vs bf16 w_out, ACT drains psum->sbuf, DMA out per ib.

Engine budget per core (cost model): PE ~206us (bottleneck), ACT ~150us,
DVE ~180us, Pool ~110us. exp assignment per j-tile keeps all three exp
engines concurrently busy so PE never starves.

Schraudolph exp: bf16_bits(p) = int16(S * (2^7/ln2) + C2B). Bias component
cancels through the softmax denominator; residual sawtooth ~1% std on the
offloaded fraction keeps total rel err well under the 2e-2 gate.

Engine discipline: walrus here allows only ONE semaphore wait per instruction
(fix_waits_nc post-pass splits/dedups), and no mixed 32/16-bit matmuls.
"""
import sys
sys.path.insert(0, "/opt/trn_rl_repo")
import numpy as np

B, N, D = 2, 4096, 512
H, DH = 8, 64
DI = H * DH
NCORES = 8
EPS = 1.1920929e-07  # float32 eps (torch nn.RMSNorm default)

# Schraudolph constants for bf16-bit exp: bits16(e^x) ~= x*C1B + C2B
C1B = 128.0 / 0.6931471805599453          # 2^7 / ln 2
C2B = float(127 * 128) - 366393.0 / 65536.0

_prog_cache = {}


def _patch_drain(tile_mod, mybir):
    """Split the multi-wait tail drain into a chain of single-wait drains
    (this walrus build rejects >1 sync wait per instruction)."""
    if getattr(tile_mod.TileContext, "_drain_patched", False):
        return

    def _patched(self, tick_clock, wait_clock):
        from concourse.vector_clock import ScopedClock
        nc = self.nc
        drain_inst = nc.sync.drain()
        wait_clock.add_sem_waits(drain_inst.ins, ScopedClock({None: tick_clock.global_clock}))
        si = drain_inst.ins.sync_info
        if si is not None and si.on_wait and len(si.on_wait) > 1:
            waits = list(si.on_wait)
            drain_inst.ins.sync_info = mybir.SyncInfo(
                on_wait=waits[:1], on_update=list(si.on_update or []))
            for w in waits[1:]:
                d2 = nc.sync.drain()
                d2.ins.sync_info = mybir.SyncInfo(on_wait=[w], on_update=[])
        nc.all_engine_barrier()
        assert self.sems is not None
        popped = nc._tile_sem_poison_stack.pop()
        assert popped is self._sem_poison
        nc.clear_and_free_semaphores(list(self.sems.allocated().values()))
        nc.all_engine_barrier()

    tile_mod.TileContext._drain_and_barrier = _patched
    tile_mod.TileContext._drain_patched = True


def build_program():
    import concourse.bass as bass
    import concourse.tile as tile
    from concourse import mybir
    from concourse.masks import make_identity

    _patch_drain(tile, mybir)

    F32 = mybir.dt.float32
    F32R = mybir.dt.float32r
    BF16 = mybir.dt.bfloat16
    I16 = mybir.dt.int16
    AF = mybir.ActivationFunctionType
    ALU = mybir.AluOpType

    NIC = N // 512          # 8 chunks of 512 tokens
    NJT = N // 128          # 32 key tiles of 128

    nc = bass.Bass(trn_type="TRN2", target_bir_lowering=False)

    tok = nc.dram_tensor("tok", [N, D], F32, kind="ExternalInput")
    wq = nc.dram_tensor("wq", [128, 4, 128], F32R, kind="ExternalInput")
    wk = nc.dram_tensor("wk", [128, 4, 128], F32R, kind="ExternalInput")
    wv = nc.dram_tensor("wv", [128, 4, 128], F32R, kind="ExternalInput")
    wo = nc.dram_tensor("wo", [128, 512], BF16, kind="ExternalInput")
    out_part = nc.dram_tensor("out_part", [N, D], F32, kind="ExternalOutput")

    tok_r = tok.rearrange("(ic t p) d -> ic p t d", t=4, p=128)
    out_r = out_part.rearrange("(ib t p) e -> ib p t e", t=4, p=128)

    # exp-engine assignment per j-tile PAIR within an iteration (A=ACT exact,
    # D=DVE Schraudolph), weighted so both engines finish together just
    # below the PE's per-iteration time. GPSIMD cannot read PSUM, so it
    # cannot join the exp crew; it takes phase-A SBUF work instead.
    NPAIR = NJT // 2
    def _mk_eng(w_a, w_d):
        pat, cnt = [], {"A": 0, "D": 0}
        wgt = {"A": w_a, "D": w_d}
        for _ in range(NPAIR):
            e = min(cnt, key=lambda k: (cnt[k] + 1) / wgt[k])
            pat.append(e)
            cnt[e] += 1
        return pat
    # alternate the split between iterations so ACT (which also does the
    # out-proj drains) keeps headroom below the PE's per-iteration pace
    ENG_EVEN = _mk_eng(9, 7)
    ENG_ODD = _mk_eng(8, 8)

    with tile.TileContext(nc) as tc:
        with tc.tile_pool(name="consts", bufs=1) as consts, \
             tc.tile_pool(name="big", bufs=1) as big, \
             tc.tile_pool(name="wpool", bufs=1) as wpool:

            # ---- constants ----
            ident_f = consts.tile([128, 128], F32)
            make_identity(nc, ident_f)
            ident = consts.tile([128, 128], F32R)
            nc.vector.tensor_copy(ident, ident_f)
            identb = consts.tile([128, 128], BF16)
            nc.vector.tensor_copy(identb, ident_f)
            eps_t = consts.tile([128, 1], F32)
            nc.vector.memset(eps_t, EPS)
            # preload the Exp ACT table during the otherwise-idle start window
            warm = consts.tile([1, 1], F32)
            nc.scalar.activation(warm, eps_t[0:1, :], AF.Exp)

            # ---- weights ----
            # first token chunk goes ahead of the weights in the DMA queue so
            # the RMSNorm pipeline starts as early as possible
            tok4_first = wpool.tile([128, 4, 512], F32)
            for t in range(4):
                nc.sync.dma_start(out=tok4_first[:, t, :], in_=tok_r[0, :, t, :])
            wq_sb = wpool.tile([128, 4, 128], F32R)
            wk_sb = wpool.tile([128, 4, 128], F32R)
            wv_sb = wpool.tile([128, 4, 128], F32R)
            wo_sb = wpool.tile([128, 512], BF16)
            nc.sync.dma_start(out=wq_sb, in_=wq[:, :, :])
            nc.sync.dma_start(out=wk_sb, in_=wk[:, :, :])
            nc.sync.dma_start(out=wv_sb, in_=wv[:, :, :])
            nc.sync.dma_start(out=wo_sb, in_=wo[:, :])

            # ---- persistent big buffers ----
            QT = big.tile([128, N], F32R)       # [2 heads x 64 qdims, n]
            KT = big.tile([128, N], F32R)
            Vb = big.tile([128, NJT, 2, 65], BF16)  # per j-tile: [v(64)|ones] per head
            ones_bf = consts.tile([128, NJT, 2, 1], BF16)
            nc.vector.memset(ones_bf, 1.0)
            nc.vector.tensor_copy(Vb[:, :, :, 64:65], ones_bf)

            # ---- phase A/B: RMSNorm + x^T + QKV^T + V natural ----
            with tc.tile_pool(name="ab_sbuf", bufs=3) as abp, \
                 tc.tile_pool(name="ab_stats", bufs=8) as stp, \
                 tc.tile_pool(name="ab_psum", bufs=5, space="PSUM") as abps, \
                 tc.tile_pool(name="qk_psum", bufs=3, space="PSUM") as qkps:

                # PE joins: absorb each weight-DMA semaphore with a tiny bf16 matmul
                scr = qkps.tile([2, 2], F32, tag="qk", name="scrj")
                BF = mybir.dt.bfloat16
                for i, wtile in enumerate((wq_sb, wk_sb, wv_sb)):
                    src = wtile[0:1, 0:1, 0:2].bitcast(BF)[:, 0, 1::2]
                    nc.tensor.matmul(scr, src, src, start=(i == 0), stop=False)
                nc.tensor.matmul(scr, wo_sb[0:1, 0:2], wo_sb[0:1, 0:2],
                                 start=False, stop=True)

                def emit_stats_chain(tok4, t):
                    """RMSNorm rstd for one 128-token block."""
                    stats = stp.tile([128, 6], F32, tag="stats")
                    mv = stp.tile([128, 2], F32, tag="mv")
                    ms = stp.tile([128, 1], F32, tag="ms")
                    s_t = stp.tile([128, 1], F32, tag="s_t")
                    rstd = stp.tile([128, 1], F32, tag="rstd")
                    nc.vector.bn_stats(stats, tok4[:, t, :])
                    nc.vector.bn_aggr(mv, stats)
                    # E[x^2] = mean^2 + var
                    nc.vector.scalar_tensor_tensor(
                        ms, mv[:, 0:1], mv[:, 0:1], mv[:, 1:2],
                        op0=ALU.mult, op1=ALU.add)
                    nc.scalar.activation(s_t, ms, AF.Sqrt, bias=eps_t, scale=1.0)
                    nc.vector.reciprocal(rstd, s_t)
                    return rstd

                def emit_stage2_t(icp, tok4p, rstds, xtp, t):
                    """normalize + transpose one 128-token block of the
                    PREVIOUS chunk (its rstd is long since ready)."""
                    xn = stp.tile([128, 512], F32R, tag="xn")
                    nc.gpsimd.tensor_scalar_mul(xn, in0=tok4p[:, t, :], scalar1=rstds[t])
                    tp = abps.tile([128, 4, 128], F32R, tag="tp")
                    for c in range(4):
                        nc.tensor.transpose(tp[:, c, :], xn[:, c * 128:(c + 1) * 128], ident)
                    nc.scalar.copy(xtp[:, :, t * 128:(t + 1) * 128], tp)

                def emit_qkv_half(state, half):
                    # one 256-token half of QKV^T: the first half only needs
                    # token blocks t0/t1, so the PE starts projecting while
                    # t2/t3 are still normalizing. 256-free keeps f32r at
                    # 1 cycle/row.
                    xtp = state["xt"]
                    lo, hi = (0, 256) if half == 0 else (256, 512)
                    for idx, wtile in enumerate((wv_sb, wq_sb, wk_sb)):
                        if half == 0:
                            state["ps"].append(qkps.tile(
                                [128, 512], F32, tag="qk", name=f"qk{idx}"))
                        ps = state["ps"][idx]
                        for c in range(4):
                            # the bank-zeroing start is only on the first
                            # matmul of the bank (it zeroes both halves)
                            nc.tensor.matmul(
                                ps[:, lo:hi], wtile[:, c, :], xtp[:, c, lo:hi],
                                start=(half == 0 and c == 0),
                                stop=(half == 1 and c == 3),
                                skip_group_check=True)

                def emit_qkv_drain(icp, state):
                    vt = abp.tile([128, 512], BF16, tag="vt")
                    nc.scalar.copy(vt, state["ps"][0])
                    # ACT drains these: DVE is the phase-A critical path
                    nc.vector.tensor_copy(QT[:, icp * 512:(icp + 1) * 512], state["ps"][1])
                    nc.scalar.copy(KT[:, icp * 512:(icp + 1) * 512], state["ps"][2])
                    state["vt"] = vt

                def emit_vb(icp, state):
                    # V^T -> V natural (bf16) into Vb (j on partitions);
                    # deferred into the next chunk's stream so the PE doesn't
                    # stall on the vt drain at the chunk boundary
                    vt = state["vt"]
                    vtp = abps.tile([128, 4, 128], BF16, tag="tp")
                    for jl in range(4):
                        nc.tensor.transpose(vtp[:, jl, :], vt[:, jl * 128:(jl + 1) * 128], identb)
                    nc.vector.tensor_copy(
                        Vb[:, icp * 4:(icp + 1) * 4, :, 0:64],
                        vtp.rearrange("p jl (h v) -> p jl h v", h=2))

                # one-chunk software pipeline: stats chains for chunk ic run
                # while chunk ic-1 is normalized/transposed/projected, so the
                # PE never waits on the DVE->ACT->DVE norm chain
                prev = None
                done = None
                for ic in range(NIC + 1):
                    if ic < NIC:
                        if ic == 0:
                            tok4 = tok4_first
                        else:
                            tok4 = abp.tile([128, 4, 512], F32, tag="tok4")
                            nc.sync.dma_start(out=tok4, in_=tok_r[ic])
                        cur_rstds = []
                        cur_state = {"ic": ic, "tok4": tok4,
                                     "rstds": cur_rstds, "ps": [],
                                     "xt": abp.tile([128, 4, 512], F32R,
                                                    tag="xt", name=f"xt{ic}")}
                    for t in range(4):
                        if prev is not None:
                            emit_stage2_t(prev["ic"], prev["tok4"],
                                          prev["rstds"], prev["xt"], t)
                            if t == 1:
                                emit_qkv_half(prev, 0)
                        if ic < NIC:
                            cur_rstds.append(emit_stats_chain(tok4, t))
                            if ic == 0:
                                # chunk 0: normalize inline, right behind its
                                # own stats chain
                                emit_stage2_t(0, tok4, cur_rstds,
                                              cur_state["xt"], t)
                                if t == 1:
                                    emit_qkv_half(cur_state, 0)
                        if t == 2 and done is not None:
                            emit_vb(done["ic"], done)
                            done = None
                    if prev is not None:
                        emit_qkv_half(prev, 1)
                        emit_qkv_drain(prev["ic"], prev)
                        done = prev
                        prev = None
                    if ic == 0:
                        emit_qkv_half(cur_state, 1)
                        emit_qkv_drain(0, cur_state)
                        done = cur_state
                    elif ic < NIC:
                        prev = cur_state
                if done is not None:
                    emit_vb(done["ic"], done)
                    done = None

            # ---- phase C: attention + out-proj (j-tile pairs) ----
            with tc.tile_pool(name="c_sbuf", bufs=3) as cp, \
                 tc.tile_pool(name="pt_pool", bufs=6) as ptp, \
                 tc.tile_pool(name="opair_pool", bufs=2) as oprp, \
                 tc.tile_pool(name="outsb_pool", bufs=2) as outsbp, \
                 tc.tile_pool(name="st_psum", bufs=3, space="PSUM") as stps, \
                 tc.tile_pool(name="o_psum", bufs=2, space="PSUM") as ops:

                iters = [(ib, hl) for ib in range(NIC) for hl in range(2)]
                NIT = len(iters)
                LOOK = 2  # S^T pairs emitted ahead of exp/PV

                def emit_st(k):
                    """S^T for j-tile pair p of iteration k//NPAIR."""
                    it_idx, p = divmod(k, NPAIR)
                    ib, hl = iters[it_idx]
                    h0 = hl * 64
                    st = stps.tile([128, 2, 512], F32, tag="st", name="stg")
                    for jl in range(2):
                        jt = 2 * p + jl
                        nc.tensor.matmul(
                            st[:, jl, :],
                            KT[h0:h0 + 64, jt * 128:(jt + 1) * 128],
                            QT[h0:h0 + 64, ib * 512:(ib + 1) * 512],
                            start=True, stop=True)
                    return st

                def emit_exp(st, p, it_idx):
                    pt = ptp.tile([128, 2, 512], BF16, tag="pt", name="ptg")
                    e = (ENG_EVEN if it_idx % 2 == 0 else ENG_ODD)[p]
                    if e == "A":
                        nc.scalar.activation(
                            pt.rearrange("a b c -> a (b c)"),
                            st.rearrange("a b c -> a (b c)"), AF.Exp)
                    else:
                        nc.vector.tensor_scalar(
                            pt.bitcast(I16).rearrange("a b c -> a (b c)"),
                            st.rearrange("a b c -> a (b c)"), C1B, C2B,
                            op0=ALU.mult, op1=ALU.add)
                    return pt

                # tail state
                o_accs = {}      # iter idx -> o_acc psum tile
                o_pairs = {}     # ib -> o_pair sbuf tile
                linvs = {}       # iter idx -> linv tile
                out_sbs = {}     # ib -> out_sb tile
                ot_tiles = {}    # ib -> (ot psum tile, ots sbuf tile)
                op_tiles = {}    # it -> out-proj psum tile (transient per ib)

                def emit_tail_step(k_prev, g):
                    """Interleave iter k_prev's tail into iter k_prev+1's
                    pair-step stream at slot g (14 usable slots)."""
                    ib, hl = iters[k_prev]
                    if g == 0:
                        o_acc = o_accs[k_prev]
                        linv = cp.tile([128, 4], F32, tag="linv", name=f"lv{hl}")
                        nc.vector.reciprocal(linv, o_acc[:, :, 64])
                        linvs[k_prev] = linv
                        if hl == 0:
                            o_pairs[ib] = oprp.tile(
                                [128, 4, 128], BF16, tag="opair", name=f"op{ib}")
                    elif 1 <= g <= 4:
                        it = g - 1
                        o_acc = o_accs[k_prev]
                        nc.vector.tensor_scalar_mul(
                            o_pairs[ib][:, it, hl * 64:(hl + 1) * 64],
                            in0=o_acc[:, it, 0:64],
                            scalar1=linvs[k_prev][:, it:it + 1])
                        if it == 3:
                            o_accs.pop(k_prev)
                            linvs.pop(k_prev)
                    elif hl == 1 and g == 5:
                        # transpose the 4 normalized [i,128] pair-tiles into a
                        # single psum bank (start only zeroes on the first);
                        # back-to-back to keep the ring slot held briefly
                        opr = o_pairs[ib]
                        ot = stps.tile([128, 4, 128], BF16, tag="st", name="otp")
                        ot_tiles[ib] = [ot, None]
                        for it in range(4):
                            nc.tensor.matmul(
                                ot[:, it, :], opr[:, it, :], identb,
                                is_transpose=True, start=(it == 0), stop=(it == 3),
                                skip_group_check=True)
                    elif hl == 1 and g == 6:
                        # ACT does this drain: a copy on DVE here delays its
                        # next exp pairs and stalls the PE
                        ots = cp.tile([128, 4, 128], BF16, tag="ots", name="otsb")
                        nc.scalar.copy(ots, ot_tiles[ib][0])
                        ot_tiles[ib][1] = ots
                        o_pairs.pop(ib)
                    elif hl == 1 and 7 <= g <= 11:
                        # g=7..10: out-proj matmul it=g-7; g=8..11: drain it=g-8
                        if g >= 8:
                            it_d = g - 8
                            nc.scalar.copy(out_sbs[ib][:, it_d, :],
                                           op_tiles.pop(it_d))
                            # per-it DMA so the store overlaps later drains
                            nc.sync.dma_start(out=out_r[ib, :, it_d, :],
                                              in_=out_sbs[ib][:, it_d, :])
                            if it_d == 3:
                                out_sbs.pop(ib)
                                ot_tiles.pop(ib)
                        if g <= 10:
                            it = g - 7
                            if it == 0:
                                out_sbs[ib] = outsbp.tile(
                                    [128, 4, 512], F32, tag="outsb", name=f"ob{ib}")
                            op_ps = stps.tile([128, 512], F32, tag="st", name="opps")
                            nc.tensor.matmul(op_ps, ot_tiles[ib][1][:, it, :],
                                             wo_sb, start=True, stop=True)
                            op_tiles[it] = op_ps

                def emit_pv(it_idx, p, pt):
                    _, hl = iters[it_idx]
                    if p == 0:
                        o_accs[it_idx] = ops.tile(
                            [128, 4, 65], F32, tag="o", name=f"oacc{it_idx % 2}")
                    o_acc = o_accs[it_idx]
                    for jl in range(2):
                        jt = 2 * p + jl
                        for it in range(4):
                            # PSUM `start` zeroes the WHOLE bank: only the
                            # very first matmul of this o_acc bank may set it
                            nc.tensor.matmul(
                                o_acc[:, it, :],
                                pt[:, jl, it * 128:(it + 1) * 128],
                                Vb[:, jt, hl, :],
                                start=(jt == 0 and it == 0),
                                stop=(jt == NJT - 1 and it == 3),
                                skip_group_check=True)

                # ---- attention pipeline over (iteration, j-tile pair) steps:
                # PV lags exp by one step so every exp has an extra pair-step
                # of slack before the PE consumes its output ----
                TOT = NIT * NPAIR
                sts = {}
                pts = {}
                for k in range(LOOK):
                    sts[k] = emit_st(k)
                for k in range(TOT + 2):
                    if k < TOT:
                        it_idx, p = divmod(k, NPAIR)
                        pts[k] = emit_exp(sts.pop(k), p, it_idx)
                        if k + LOOK < TOT:
                            sts[k + LOOK] = emit_st(k + LOOK)
                    if k >= 2:
                        emit_pv(*divmod(k - 2, NPAIR), pts.pop(k - 2))
                    if k < TOT and it_idx > 0 and p >= 2:
                        # tail slots start at p==2: the previous iteration's
                        # final (2-step-lagged) PV lands at p==1, and the
                        # tail's linv must observe it
                        emit_tail_step(it_idx - 1, p - 2)

                # trailing tail for the final iteration: no exp traffic to
                # pace against, so run a tight per-it pipeline instead
                kl = NIT - 1
                ibl, _ = iters[kl]
                o_acc = o_accs.pop(kl)
                linv = cp.tile([128, 4], F32, tag="linv", name="lvf")
                nc.vector.reciprocal(linv, o_acc[:, :, 64])
                oprl = o_pairs[ibl]
                otl = stps.tile([128, 4, 128], BF16, tag="st", name="otpf")
                out_sbl = outsbp.tile([128, 4, 512], F32, tag="outsb", name="obf")
                opsl = {}
                for it in range(4):
                    nc.vector.tensor_scalar_mul(
                        oprl[:, it, 64:128], in0=o_acc[:, it, 0:64],
                        scalar1=linv[:, it:it + 1])
                    nc.tensor.matmul(
                        otl[:, it, :], oprl[:, it, :], identb,
                        is_transpose=True, start=(it == 0), stop=(it == 3),
                        skip_group_check=True)
                    # per-it ots copy so each out-proj starts immediately
                    otsl = cp.tile([128, 128], BF16, tag="ots", name=f"otsf{it}")
                    nc.scalar.copy(otsl, otl[:, it, :])
                    op_ps = stps.tile([128, 512], F32, tag="st", name=f"opf{it}")
                    nc.tensor.matmul(op_ps, otsl, wo_sb, start=True, stop=True)
                    opsl[it] = op_ps
                    if it >= 1:
                        nc.scalar.copy(out_sbl[:, it - 1, :], opsl.pop(it - 1))
                        nc.sync.dma_start(out=out_r[ibl, :, it - 1, :],
                                          in_=out_sbl[:, it - 1, :])
                nc.scalar.copy(out_sbl[:, 3, :], opsl.pop(3))
                nc.sync.dma_start(out=out_r[ibl, :, 3, :], in_=out_sbl[:, 3, :])

    fix_waits_nc(nc, mybir)
    return nc


def fix_waits_nc(nc, mybir):
    """Post-pass over the scheduled program: (1) remove semaphore waits that
    are transitively implied by earlier waits (Tile emits per-proc-minimal,
    not transitively-minimal, waits), (2) split any instruction still
    carrying more than one wait by injecting single-wait NoOps in front of
    it — this walrus build rejects >1 sync wait per instruction.
    Mutates nc in place so CoreSim and hardware run identical sync."""
    nop_id = [0]

    def _is_ge(w):
        return w.sync_type == "semaphore" and w.wait_mode == "sem-ge-imm"

    for fn in nc.m.functions:
        for blk in fn.blocks:
            insts = list(blk.instructions)
            n = len(insts)

            producers = {}
            cum = {}
            nonmono = set()  # sems ever decremented: counter logic invalid
            for idx, inst in enumerate(insts):
                si = inst.sync_info
                for u in (si.on_update if si else []) or []:
                    if u.sync_type != "semaphore":
                        continue
                    sid = u.id
                    if u.update_mode != "sem-inc":
                        nonmono.add(sid)
                        continue
                    cum[sid] = cum.get(sid, 0) + int(u.update_value)
                    producers.setdefault(sid, []).append((cum[sid], idx))

            def producer_of(sid, val):
                for cv, idx in producers.get(sid, ()):
                    if cv >= val:
                        return idx
                return None

            prev_eng = [None] * n
            last = {}
            for idx, inst in enumerate(insts):
                e = inst.engine
                prev_eng[idx] = last.get(e)
                last[e] = idx

            def get_waits(inst):
                si = inst.sync_info
                return list(si.on_wait) if si and si.on_wait else []

            def is_ge(w):
                return _is_ge(w) and w.id not in nonmono

            know = [dict() for _ in range(n)]
            for _ in range(3):
                changed = False
                for idx, inst in enumerate(insts):
                    k = dict(know[prev_eng[idx]]) if prev_eng[idx] is not None else {}
                    for w in get_waits(inst):
                        if not is_ge(w):
                            continue
                        sid, val = w.id, int(w.wait_value)
                        if k.get(sid, -1) < val:
                            k[sid] = val
                        p = producer_of(sid, val)
                        if p is not None:
                            for s2, v2 in know[p].items():
                                if k.get(s2, -1) < v2:
                                    k[s2] = v2
                    if k != know[idx]:
                        know[idx] = k
                        changed = True
                if not changed:
                    break

            new_insts = []
            dirty = False
            for idx, inst in enumerate(insts):
                si = inst.sync_info
                waits = get_waits(inst)
                if si is not None and waits:
                    base = dict(know[prev_eng[idx]]) if prev_eng[idx] is not None else {}
                    kept = []
                    for w in waits:
                        if is_ge(w):
                            sid, val = w.id, int(w.wait_value)
                            if base.get(sid, -1) >= val:
                                continue
                            base[sid] = val
                            p = producer_of(sid, val)
                            if p is not None:
                                for s2, v2 in know[p].items():
                                    if base.get(s2, -1) < v2:
                                        base[s2] = v2
                        kept.append(w)
                    if len(kept) != len(waits) or len(kept) > 1:
                        dirty = True
                        for w in kept[:-1]:
                            nop_id[0] += 1
                            nop = mybir.InstNoOp(
                                name=f"I-waitfix-{nop_id[0]}", ins=[], outs=[])
                            nop.engine = inst.engine
                            nop.sync_info = mybir.SyncInfo(on_wait=[w], on_update=[])
                            nc.register_instruction(nop)
                            new_insts.append(nop)
                        inst.sync_info = mybir.SyncInfo(
                            on_wait=kept[-1:],
                            on_update=list(si.on_update or []))
                new_insts.append(inst)
            if dirty:
                blk.instructions = new_insts


def get_program():
    if "nc" not in _prog_cache:
        _prog_cache["nc"] = build_program()
    return _prog_cache["nc"]


def _prep_inputs(tokens, norm_weight, w_qkv, w_out):
    import ml_dtypes
    tokens = np.ascontiguousarray(np.asarray(tokens, dtype=np.float32))
    norm_weight = np.asarray(norm_weight, dtype=np.float32)
    w_qkv = np.asarray(w_qkv, dtype=np.float32)
    w_out = np.asarray(w_out, dtype=np.float32)

    wp = w_qkv * norm_weight[:, None]  # fold RMSNorm weight into qkv weights

    in_maps = []
    for c in range(NCORES):
        b = c // 4
        h0 = 2 * (c % 4)
        m = {}
        m["tok"] = tokens[b]
        for name, off in (("wq", 0), ("wk", DI), ("wv", 2 * DI)):
            w = wp[:, off + h0 * DH: off + (h0 + 2) * DH]       # [512, 128]
            m[name] = np.ascontiguousarray(
                w.reshape(4, 128, 128).transpose(1, 0, 2))       # [128, 4, 128]
        rows = w_out[h0 * DH:(h0 + 2) * DH, :]                   # [128, 512]
        m["wo"] = np.ascontiguousarray(rows.astype(ml_dtypes.bfloat16))
        in_maps.append(m)
    return in_maps


def run(tokens, norm_weight, w_qkv, w_out, trace=False):
    from concourse.bass_utils import run_bass_kernel_spmd
    nc = get_program()
    in_maps = _prep_inputs(tokens, norm_weight, w_qkv, w_out)
    res = run_bass_kernel_spmd(nc, in_maps, core_ids=list(range(NCORES)), trace=trace)
    parts = [res.results[c]["out_part"] for c in range(NCORES)]
    out = np.empty((B, N, D), dtype=np.float32)
    for b in range(B):
        out[b] = parts[4 * b] + parts[4 * b + 1] + parts[4 * b + 2] + parts[4 * b + 3]
    return out, res


def kernel(tokens, norm_weight, w_qkv, w_out):
    out, _ = run(tokens, norm_weight, w_qkv, w_out, trace=False)
    return out


# revision 46
# speedup vs baseline: 1.4151x; 1.0062x over previous
"""TRN2 Bass kernel for nn_Attention (RMSNorm + QKV + softmax attention + out-proj).

Sharding: 8 cores = 2 batches x 4 head-pairs. Core c handles batch c//4 and
heads (2*(c%4), 2*(c%4)+1). Each core computes its partial out-projection
(contracting only its 128 rows of dim_inner); host sums the 4 partials per batch.

V2 design (cost-model driven):
  A) stream tokens [128,512] blocks: RMSNorm stats on DVE, rstd scale, PE
     transpose -> x^T (rotating [128,4,512] per 512-token chunk)
  B) QKV^T = w^T @ x^T (PE); V^T -> bf16 -> transposed back to V-natural with
     a ones column appended per head (softmax denominator for free)
  C) flash attention per (ib=512 queries, head): for each j-tile of 128 keys:
     S^T [128j, 512i] on PE (f32r, 512 cycles); exp split across THREE engines
     (ACT: exact table exp -> bf16; DVE/GPSIMD: Schraudolph bit-trick exp via
     tensor_scalar -> int16 bits of bf16). PV uses exp(S)^T tiles as the
     STATIONARY [128j, 128i] and bf16 V[128j,64+1] as MOVING: 65-cycle
     matmuls accumulate O-natural [128i, 65] in PSUM (col 64 = l).
  D) tail: linv=1/l (DVE), scale O by linv -> bf16 pair tile [i, 128dh both
     heads] (DVE), PE transpose [128,128], one 128-contraction out-proj matmul
     per i-tile vs bf16 w_out, ACT drains psum->sbuf, DMA out per ib.

Engine budget per core (cost model): PE ~206us (bottleneck), ACT ~150us,
DVE ~180us, Pool ~110us. exp assignment per j-tile keeps all three exp
engines concurrently busy so PE never starves.

Schraudolph exp: bf16_bits(p) = int16(S * (2^7/ln2) + C2B). Bias component
cancels through the softmax denominator; residual sawtooth ~1% std on the
offloaded fraction keeps total rel err well under the 2e-2 gate.

Engine discipline: walrus here allows only ONE semaphore wait per instruction
(fix_waits_nc post-pass splits/dedups), and no mixed 32/16-bit matmuls.
"""
import sys
sys.path.insert(0, "/opt/trn_rl_repo")
import numpy as np

B, N, D = 2, 4096, 512
H, DH = 8, 64
DI = H * DH
NCORES = 8
EPS = 1.1920929e-07  # float32 eps (torch nn.RMSNorm default)

# Schraudolph constants for bf16-bit exp: bits16(e^x) ~= x*C1B + C2B
C1B = 128.0 / 0.6931471805599453          # 2^7 / ln 2
C2B = float(127 * 128) - 366393.0 / 65536.0

_prog_cache = {}


def _patch_drain(tile_mod, mybir):
    """Split the multi-wait tail drain into a chain of single-wait drains
    (this walrus build rejects >1 sync wait per instruction)."""
    if getattr(tile_mod.TileContext, "_drain_patched", False):
        return

    def _patched(self, tick_clock, wait_clock):
        from concourse.vector_clock import ScopedClock
        nc = self.nc
        drain_inst = nc.sync.drain()
        wait_clock.add_sem_waits(drain_inst.ins, ScopedClock({None: tick_clock.global_clock}))
        si = drain_inst.ins.sync_info
        if si is not None and si.on_wait and len(si.on_wait) > 1:
            waits = list(si.on_wait)
            drain_inst.ins.sync_info = mybir.SyncInfo(
                on_wait=waits[:1], on_update=list(si.on_update or []))
            for w in waits[1:]:
                d2 = nc.sync.drain()
                d2.ins.sync_info = mybir.SyncInfo(on_wait=[w], on_update=[])
        nc.all_engine_barrier()
        assert self.sems is not None
        popped = nc._tile_sem_poison_stack.pop()
        assert popped is self._sem_poison
        nc.clear_and_free_semaphores(list(self.sems.allocated().values()))
        nc.all_engine_barrier()

    tile_mod.TileContext._drain_and_barrier = _patched
    tile_mod.TileContext._drain_patched = True


def build_program():
    import concourse.bass as bass
    import concourse.tile as tile
    from concourse import mybir
    from concourse.masks import make_identity

    _patch_drain(tile, mybir)

    F32 = mybir.dt.float32
    F32R = mybir.dt.float32r
    BF16 = mybir.dt.bfloat16
    I16 = mybir.dt.int16
    AF = mybir.ActivationFunctionType
    ALU = mybir.AluOpType

    NIC = N // 512          # 8 chunks of 512 tokens
    NJT = N // 128          # 32 key tiles of 128

    nc = bass.Bass(trn_type="TRN2", target_bir_lowering=False)

    tok = nc.dram_tensor("tok", [N, D], F32, kind="ExternalInput")
    wq = nc.dram_tensor("wq", [128, 4, 128], F32R, kind="ExternalInput")
    wk = nc.dram_tensor("wk", [128, 4, 128], F32R, kind="ExternalInput")
    wv = nc.dram_tensor("wv", [128, 4, 128], F32R, kind="ExternalInput")
    wo = nc.dram_tensor("wo", [128, 512], BF16, kind="ExternalInput")
    out_part = nc.dram_tensor("out_part", [N, D], F32, kind="ExternalOutput")

    tok_r = tok.rearrange("(ic t p) d -> ic p t d", t=4, p=128)
    out_r = out_part.rearrange("(ib t p) e -> ib p t e", t=4, p=128)

    # exp-engine assignment per j-tile PAIR within an iteration (A=ACT exact,
    # D=DVE Schraudolph), weighted so both engines finish together just
    # below the PE's per-iteration time. GPSIMD cannot read PSUM, so it
    # cannot join the exp crew; it takes phase-A SBUF work instead.
    NPAIR = NJT // 2
    def _mk_eng(w_a, w_d):
        pat, cnt = [], {"A": 0, "D": 0}
        wgt = {"A": w_a, "D": w_d}
        for _ in range(NPAIR):
            e = min(cnt, key=lambda k: (cnt[k] + 1) / wgt[k])
            pat.append(e)
            cnt[e] += 1
        return pat
    # alternate the split between iterations so ACT (which also does the
    # out-proj drains) keeps headroom below the PE's per-iteration pace
    ENG_EVEN = _mk_eng(9, 7)
    ENG_ODD = _mk_eng(8, 8)

    with tile.TileContext(nc) as tc:
        with tc.tile_pool(name="consts", bufs=1) as consts, \
             tc.tile_pool(name="big", bufs=1) as big, \
             tc.tile_pool(name="wpool", bufs=1) as wpool:

            # ---- constants ----
            ident_f = consts.tile([128, 128], F32)
            make_identity(nc, ident_f)
            ident = consts.tile([128, 128], F32R)
            nc.vector.tensor_copy(ident, ident_f)
            identb = consts.tile([128, 128], BF16)
            nc.vector.tensor_copy(identb, ident_f)
            eps_t = consts.tile([128, 1], F32)
            nc.vector.memset(eps_t, EPS)
            # preload the Exp ACT table during the otherwise-idle start window
            warm = consts.tile([1, 1], F32)
            nc.scalar.activation(warm, eps_t[0:1, :], AF.Exp)

            # ---- weights ----
            # first token chunk goes ahead of the weights in the DMA queue so
            # the RMSNorm pipeline starts as early as possible
            tok4_first = wpool.tile([128, 4, 512], F32)
            for t in range(4):
                nc.sync.dma_start(out=tok4_first[:, t, :], in_=tok_r[0, :, t, :])
            wq_sb = wpool.tile([128, 4, 128], F32R)
            wk_sb = wpool.tile([128, 4, 128], F32R)
            wv_sb = wpool.tile([128, 4, 128], F32R)
            wo_sb = wpool.tile([128, 512], BF16)
            nc.sync.dma_start(out=wq_sb, in_=wq[:, :, :])
            nc.sync.dma_start(out=wk_sb, in_=wk[:, :, :])
            nc.sync.dma_start(out=wv_sb, in_=wv[:, :, :])
            nc.sync.dma_start(out=wo_sb, in_=wo[:, :])

            # ---- persistent big buffers ----
            QT = big.tile([128, N], F32R)       # [2 heads x 64 qdims, n]
            KT = big.tile([128, N], F32R)
            Vb = big.tile([128, NJT, 2, 65], BF16)  # per j-tile: [v(64)|ones] per head
            ones_bf = consts.tile([128, NJT, 2, 1], BF16)
            nc.vector.memset(ones_bf, 1.0)
            nc.vector.tensor_copy(Vb[:, :, :, 64:65], ones_bf)

            # ---- phase A/B: RMSNorm + x^T + QKV^T + V natural ----
            with tc.tile_pool(name="ab_sbuf", bufs=3) as abp, \
                 tc.tile_pool(name="ab_stats", bufs=8) as stp, \
                 tc.tile_pool(name="ab_psum", bufs=5, space="PSUM") as abps, \
                 tc.tile_pool(name="qk_psum", bufs=3, space="PSUM") as qkps:

                # PE joins: absorb each weight-DMA semaphore with a tiny bf16 matmul
                scr = qkps.tile([2, 2], F32, tag="qk", name="scrj")
                BF = mybir.dt.bfloat16
                for i, wtile in enumerate((wq_sb, wk_sb, wv_sb)):
                    src = wtile[0:1, 0:1, 0:2].bitcast(BF)[:, 0, 1::2]
                    nc.tensor.matmul(scr, src, src, start=(i == 0), stop=False)
                nc.tensor.matmul(scr, wo_sb[0:1, 0:2], wo_sb[0:1, 0:2],
                                 start=False, stop=True)

                def emit_stats_chain(tok4, t):
                    """RMSNorm rstd for one 128-token block."""
                    stats = stp.tile([128, 6], F32, tag="stats")
                    mv = stp.tile([128, 2], F32, tag="mv")
                    ms = stp.tile([128, 1], F32, tag="ms")
                    s_t = stp.tile([128, 1], F32, tag="s_t")
                    rstd = stp.tile([128, 1], F32, tag="rstd")
                    nc.vector.bn_stats(stats, tok4[:, t, :])
                    nc.vector.bn_aggr(mv, stats)
                    # E[x^2] = mean^2 + var
                    nc.vector.scalar_tensor_tensor(
                        ms, mv[:, 0:1], mv[:, 0:1], mv[:, 1:2],
                        op0=ALU.mult, op1=ALU.add)
                    nc.scalar.activation(s_t, ms, AF.Sqrt, bias=eps_t, scale=1.0)
                    nc.vector.reciprocal(rstd, s_t)
                    return rstd

                def emit_stage2_t(icp, tok4p, rstds, xtp, t):
                    """normalize + transpose one 128-token block of the
                    PREVIOUS chunk (its rstd is long since ready)."""
                    xn = stp.tile([128, 512], F32R, tag="xn")
                    nc.gpsimd.tensor_scalar_mul(xn, in0=tok4p[:, t, :], scalar1=rstds[t])
                    tp = abps.tile([128, 4, 128], F32R, tag="tp")
                    for c in range(4):
                        nc.tensor.transpose(tp[:, c, :], xn[:, c * 128:(c + 1) * 128], ident)
                    nc.scalar.copy(xtp[:, :, t * 128:(t + 1) * 128], tp)

                def emit_qkv_half(state, half):
                    # one 256-token half of QKV^T: the first half only needs
                    # token blocks t0/t1, so the PE starts projecting while
                    # t2/t3 are still normalizing. 256-free keeps f32r at
                    # 1 cycle/row.
                    xtp = state["xt"]
                    lo, hi = (0, 256) if half == 0 else (256, 512)
                    for idx, wtile in enumerate((wv_sb, wq_sb, wk_sb)):
                        if half == 0:
                            state["ps"].append(qkps.tile(
                                [128, 512], F32, tag="qk", name=f"qk{idx}"))
                        ps = state["ps"][idx]
                        for c in range(4):
                            # the bank-zeroing start is only on the first
                            # matmul of the bank (it zeroes both halves)
                            nc.tensor.matmul(
                                ps[:, lo:hi], wtile[:, c, :], xtp[:, c, lo:hi],
                                start=(half == 0 and c == 0),
                                stop=(half == 1 and c == 3),
                                skip_group_check=True)

                def emit_qkv_drain(icp, state):
                    vt = abp.tile([128, 512], BF16, tag="vt")
                    nc.scalar.copy(vt, state["ps"][0])
                    # ACT drains these: DVE is the phase-A critical path
                    nc.vector.tensor_copy(QT[:, icp * 512:(icp + 1) * 512], state["ps"][1])
                    nc.scalar.copy(KT[:, icp * 512:(icp + 1) * 512], state["ps"][2])
                    state["vt"] = vt

                def emit_vb(icp, state):
                    # V^T -> V natural (bf16) into Vb (j on partitions);
                    # deferred into the next chunk's stream so the PE doesn't
                    # stall on the vt drain at the chunk boundary
                    vt = state["vt"]
                    vtp = abps.tile([128, 4, 128], BF16, tag="tp")
                    for jl in range(4):
                        nc.tensor.transpose(vtp[:, jl, :], vt[:, jl * 128:(jl + 1) * 128], identb)
                    nc.vector.tensor_copy(
                        Vb[:, icp * 4:(icp + 1) * 4, :, 0:64],
                        vtp.rearrange("p jl (h v) -> p jl h v", h=2))

                # one-chunk software pipeline: stats chains for chunk ic run
                # while chunk ic-1 is normalized/transposed/projected, so the
                # PE never waits on the DVE->ACT->DVE norm chain
                prev = None
                done = None
                for ic in range(NIC + 1):
                    if ic < NIC:
                        if ic == 0:
                            tok4 = tok4_first
                        else:
                            tok4 = abp.tile([128, 4, 512], F32, tag="tok4")
                            nc.sync.dma_start(out=tok4, in_=tok_r[ic])
                        cur_rstds = []
                        cur_state = {"ic": ic, "tok4": tok4,
                                     "rstds": cur_rstds, "ps": [],
                                     "xt": abp.tile([128, 4, 512], F32R,
                                                    tag="xt", name=f"xt{ic}")}
                    for t in range(4):
                        if prev is not None:
                            emit_stage2_t(prev["ic"], prev["tok4"],
                                          prev["rstds"], prev["xt"], t)
                            if t == 1:
                                emit_qkv_half(prev, 0)
                        if ic < NIC:
                            cur_rstds.append(emit_stats_chain(tok4, t))
                            if ic == 0:
                                # chunk 0: normalize inline, right behind its
                                # own stats chain
                                emit_stage2_t(0, tok4, cur_rstds,
                                              cur_state["xt"], t)
                                if t == 1:
                                    emit_qkv_half(cur_state, 0)
                        if t == 2 and done is not None:
                            emit_vb(done["ic"], done)
                            done = None
                    if prev is not None:
                        emit_qkv_half(prev, 1)
                        emit_qkv_drain(prev["ic"], prev)
                        done = prev
                        prev = None
                    if ic == 0:
                        emit_qkv_half(cur_state, 1)
                        emit_qkv_drain(0, cur_state)
                        done = cur_state
                    elif ic < NIC:
                        prev = cur_state
                if done is not None:
                    emit_vb(done["ic"], done)
                    done = None

            # ---- phase C: attention + out-proj (j-tile pairs) ----
            with tc.tile_pool(name="c_sbuf", bufs=3) as cp, \
                 tc.tile_pool(name="pt_pool", bufs=6) as ptp, \
                 tc.tile_pool(name="opair_pool", bufs=2) as oprp, \
                 tc.tile_pool(name="outsb_pool", bufs=2) as outsbp, \
                 tc.tile_pool(name="st_psum", bufs=3, space="PSUM") as stps, \
                 tc.tile_pool(name="o_psum", bufs=2, space="PSUM") as ops:

                iters = [(ib, hl) for ib in range(NIC) for hl in range(2)]
                NIT = len(iters)
                LOOK = 2  # S^T pairs emitted ahead of exp/PV

                def emit_st(k):
                    """S^T for j-tile pair p of iteration k//NPAIR."""
                    it_idx, p = divmod(k, NPAIR)
                    ib, hl = iters[it_idx]
                    h0 = hl * 64
                    st = stps.tile([128, 2, 512], F32, tag="st", name="stg")
                    for jl in range(2):
                        jt = 2 * p + jl
                        nc.tensor.matmul(
                            st[:, jl, :],
                            KT[h0:h0 + 64, jt * 128:(jt + 1) * 128],
                            QT[h0:h0 + 64, ib * 512:(ib + 1) * 512],
                            start=True, stop=True)
                    return st

                def emit_exp(st, p, it_idx):
                    pt = ptp.tile([128, 2, 512], BF16, tag="pt", name="ptg")
                    e = (ENG_EVEN if it_idx % 2 == 0 else ENG_ODD)[p]
                    if e == "A":
                        nc.scalar.activation(
                            pt.rearrange("a b c -> a (b c)"),
                            st.rearrange("a b c -> a (b c)"), AF.Exp)
                    else:
                        nc.vector.tensor_scalar(
                            pt.bitcast(I16).rearrange("a b c -> a (b c)"),
                            st.rearrange("a b c -> a (b c)"), C1B, C2B,
                            op0=ALU.mult, op1=ALU.add)
                    return pt

                # tail state
                o_accs = {}      # iter idx -> o_acc psum tile
                o_pairs = {}     # ib -> o_pair sbuf tile
                linvs = {}       # iter idx -> linv tile
                out_sbs = {}     # ib -> out_sb tile
                ot_tiles = {}    # ib -> (ot psum tile, ots sbuf tile)
                op_tiles = {}    # it -> out-proj psum tile (transient per ib)

                def emit_tail_step(k_prev, g):
                    """Interleave iter k_prev's tail into iter k_prev+1's
                    pair-step stream at slot g (14 usable slots)."""
                    ib, hl = iters[k_prev]
                    if g == 0:
                        o_acc = o_accs[k_prev]
                        linv = cp.tile([128, 4], F32, tag="linv", name=f"lv{hl}")
                        nc.vector.reciprocal(linv, o_acc[:, :, 64])
                        linvs[k_prev] = linv
                        if hl == 0:
                            o_pairs[ib] = oprp.tile(
                                [128, 4, 128], BF16, tag="opair", name=f"op{ib}")
                    elif 1 <= g <= 4:
                        it = g - 1
                        o_acc = o_accs[k_prev]
                        nc.vector.tensor_scalar_mul(
                            o_pairs[ib][:, it, hl * 64:(hl + 1) * 64],
                            in0=o_acc[:, it, 0:64],
                            scalar1=linvs[k_prev][:, it:it + 1])
                        if it == 3:
                            o_accs.pop(k_prev)
                            linvs.pop(k_prev)
                    elif hl == 1 and g == 5:
                        # ONE 2-bank ring slot serves the whole out-proj tail:
                        # bank A holds the transposed pair-tiles (bf16 view),
                        # bank B is reused by all 4 out-proj matmuls. This
                        # steals 1 ring slot per tail window instead of 5.
                        opr = o_pairs[ib]
                        tail_t = stps.tile([128, 2, 512], F32, tag="st",
                                           name="tailt")
                        ot = tail_t[:, 0, 0:256].bitcast(BF16).rearrange(
                            "p (a b) -> p a b", a=4)
                        ot_tiles[ib] = [tail_t, ot, None]
                        for it in range(4):
                            nc.tensor.matmul(
                                ot[:, it, :], opr[:, it, :], identb,
                                is_transpose=True, start=(it == 0), stop=(it == 3),
                                skip_group_check=True)
                    elif hl == 1 and g == 6:
                        # ACT does this drain: a copy on DVE here delays its
                        # next exp pairs and stalls the PE
                        ots = cp.tile([128, 4, 128], BF16, tag="ots", name="otsb")
                        nc.scalar.copy(ots, ot_tiles[ib][1])
                        ot_tiles[ib][2] = ots
                        o_pairs.pop(ib)
                    elif hl == 1 and 7 <= g <= 11:
                        # g=7..10: out-proj matmul it=g-7; g=8..11: drain it=g-8
                        if g >= 8:
                            it_d = g - 8
                            nc.scalar.copy(out_sbs[ib][:, it_d, :],
                                           op_tiles.pop(it_d))
                            # per-it DMA so the store overlaps later drains
                            nc.sync.dma_start(out=out_r[ib, :, it_d, :],
                                              in_=out_sbs[ib][:, it_d, :])
                            if it_d == 3:
                                out_sbs.pop(ib)
                                ot_tiles.pop(ib)
                        if g <= 10:
                            it = g - 7
                            if it == 0:
                                out_sbs[ib] = outsbp.tile(
                                    [128, 4, 512], F32, tag="outsb", name=f"ob{ib}")
                            # bank B of the shared tail tile, reused per it
                            op_ps = ot_tiles[ib][0][:, 1, :]
                            nc.tensor.matmul(op_ps, ot_tiles[ib][2][:, it, :],
                                             wo_sb, start=True, stop=True)
                            op_tiles[it] = op_ps

                def emit_pv(it_idx, p, pt):
                    _, hl = iters[it_idx]
                    if p == 0:
                        o_accs[it_idx] = ops.tile(
                            [128, 4, 65], F32, tag="o", name=f"oacc{it_idx % 2}")
                    o_acc = o_accs[it_idx]
                    for jl in range(2):
                        jt = 2 * p + jl
                        for it in range(4):
                            # PSUM `start` zeroes the WHOLE bank: only the
                            # very first matmul of this o_acc bank may set it
                            nc.tensor.matmul(
                                o_acc[:, it, :],
                                pt[:, jl, it * 128:(it + 1) * 128],
                                Vb[:, jt, hl, :],
                                start=(jt == 0 and it == 0),
                                stop=(jt == NJT - 1 and it == 3),
                                skip_group_check=True)

                # ---- attention pipeline over (iteration, j-tile pair) steps:
                # PV lags exp by one step so every exp has an extra pair-step
                # of slack before the PE consumes its output ----
                TOT = NIT * NPAIR
                sts = {}
                pts = {}
                for k in range(LOOK):
                    sts[k] = emit_st(k)
                for k in range(TOT + 2):
                    if k < TOT:
                        it_idx, p = divmod(k, NPAIR)
                        pts[k] = emit_exp(sts.pop(k), p, it_idx)
                        if k + LOOK < TOT:
                            sts[k + LOOK] = emit_st(k + LOOK)
                    if k >= 2:
                        emit_pv(*divmod(k - 2, NPAIR), pts.pop(k - 2))
                    if k < TOT and it_idx > 0 and p >= 2:
                        # tail slots start at p==2: the previous iteration's
                        # final (2-step-lagged) PV lands at p==1, and the
                        # tail's linv must observe it
                        emit_tail_step(it_idx - 1, p - 2)

                # trailing tail for the final iteration: no exp traffic to
                # pace against, so run a tight per-it pipeline instead
                kl = NIT - 1
                ibl, _ = iters[kl]
                o_acc = o_accs.pop(kl)
                linv = cp.tile([128, 4], F32, tag="linv", name="lvf")
                nc.vector.reciprocal(linv, o_acc[:, :, 64])
                oprl = o_pairs[ibl]
                otl = stps.tile([128, 4, 128], BF16, tag="st", name="otpf")
                out_sbl = outsbp.tile([128, 4, 512], F32, tag="outsb", name="obf")
                opsl = {}
                for it in range(4):
                    nc.vector.tensor_scalar_mul(
                        oprl[:, it, 64:128], in0=o_acc[:, it, 0:64],
                        scalar1=linv[:, it:it + 1])
                    nc.tensor.matmul(
                        otl[:, it, :], oprl[:, it, :], identb,
                        is_transpose=True, start=(it == 0), stop=(it == 3),
                        skip_group_check=True)
                    # per-it ots copy so each out-proj starts immediately
                    otsl = cp.tile([128, 128], BF16, tag="ots", name=f"otsf{it}")
                    nc.scalar.copy(otsl, otl[:, it, :])
                    op_ps = stps.tile([128, 512], F32, tag="st", name=f"opf{it}")
                    nc.tensor.matmul(op_ps, otsl, wo_sb, start=True, stop=True)
                    opsl[it] = op_ps
                    if it >= 1:
                        nc.scalar.copy(out_sbl[:, it - 1, :], opsl.pop(it - 1))
                        nc.sync.dma_start(out=out_r[ibl, :, it - 1, :],
                                          in_=out_sbl[:, it - 1, :])
                nc.scalar.copy(out_sbl[:, 3, :], opsl.pop(3))
                nc.sync.dma_start(out=out_r[ibl, :, 3, :], in_=out_sbl[:, 3, :])

    fix_waits_nc(nc, mybir)
    return nc


def fix_waits_nc(nc, mybir):
    """Post-pass over the scheduled program: (1) remove semaphore waits that
    are transitively implied by earlier waits (Tile emits per-proc-minimal,
    not transitively-minimal, waits), (2) split any instruction still
    carrying more than one wait by injecting single-wait NoOps in front of
    it — this walrus build rejects >1 sync wait per instruction.
    Mutates nc in place so CoreSim and hardware run identical sync."""
    nop_id = [0]

    def _is_ge(w):
        return w.sync_type == "semaphore" and w.wait_mode == "sem-ge-imm"

    for fn in nc.m.functions:
        for blk in fn.blocks:
            insts = list(blk.instructions)
            n = len(insts)

            producers = {}
            cum = {}
            nonmono = set()  # sems ever decremented: counter logic invalid
            for idx, inst in enumerate(insts):
                si = inst.sync_info
                for u in (si.on_update if si else []) or []:
                    if u.sync_type != "semaphore":
                        continue
                    sid = u.id
                    if u.update_mode != "sem-inc":
                        nonmono.add(sid)
                        continue
                    cum[sid] = cum.get(sid, 0) + int(u.update_value)
                    producers.setdefault(sid, []).append((cum[sid], idx))

            def producer_of(sid, val):
                for cv, idx in producers.get(sid, ()):
                    if cv >= val:
                        return idx
                return None

            prev_eng = [None] * n
            last = {}
            for idx, inst in enumerate(insts):
                e = inst.engine
                prev_eng[idx] = last.get(e)
                last[e] = idx

            def get_waits(inst):
                si = inst.sync_info
                return list(si.on_wait) if si and si.on_wait else []

            def is_ge(w):
                return _is_ge(w) and w.id not in nonmono

            know = [dict() for _ in range(n)]
            for _ in range(3):
                changed = False
                for idx, inst in enumerate(insts):
                    k = dict(know[prev_eng[idx]]) if prev_eng[idx] is not None else {}
                    for w in get_waits(inst):
                        if not is_ge(w):
                            continue
                        sid, val = w.id, int(w.wait_value)
                        if k.get(sid, -1) < val:
                            k[sid] = val
                        p = producer_of(sid, val)
                        if p is not None:
                            for s2, v2 in know[p].items():
                                if k.get(s2, -1) < v2:
                                    k[s2] = v2
                    if k != know[idx]:
                        know[idx] = k
                        changed = True
                if not changed:
                    break

            new_insts = []
            dirty = False
            for idx, inst in enumerate(insts):
                si = inst.sync_info
                waits = get_waits(inst)
                if si is not None and waits:
                    base = dict(know[prev_eng[idx]]) if prev_eng[idx] is not None else {}
                    kept = []
                    for w in waits:
                        if is_ge(w):
                            sid, val = w.id, int(w.wait_value)
                            if base.get(sid, -1) >= val:
                                continue
                            base[sid] = val
                            p = producer_of(sid, val)
                            if p is not None:
                                for s2, v2 in know[p].items():
                                    if base.get(s2, -1) < v2:
                                        base[s2] = v2
                        kept.append(w)
                    if len(kept) != len(waits) or len(kept) > 1:
                        dirty = True
                        for w in kept[:-1]:
                            nop_id[0] += 1
                            nop = mybir.InstNoOp(
                                name=f"I-waitfix-{nop_id[0]}", ins=[], outs=[])
                            nop.engine = inst.engine
                            nop.sync_info = mybir.SyncInfo(on_wait=[w], on_update=[])
                            nc.register_instruction(nop)
                            new_insts.append(nop)
                        inst.sync_info = mybir.SyncInfo(
                            on_wait=kept[-1:],
                            on_update=list(si.on_update or []))
                new_insts.append(inst)
            if dirty:
                blk.instructions = new_insts


def get_program():
    if "nc" not in _prog_cache:
        _prog_cache["nc"] = build_program()
    return _prog_cache["nc"]


def _prep_inputs(tokens, norm_weight, w_qkv, w_out):
    import ml_dtypes
    tokens = np.ascontiguousarray(np.asarray(tokens, dtype=np.float32))
    norm_weight = np.asarray(norm_weight, dtype=np.float32)
    w_qkv = np.asarray(w_qkv, dtype=np.float32)
    w_out = np.asarray(w_out, dtype=np.float32)

    wp = w_qkv * norm_weight[:, None]  # fold RMSNorm weight into qkv weights

    in_maps = []
    for c in range(NCORES):
        b = c // 4
        h0 = 2 * (c % 4)
        m = {}
        m["tok"] = tokens[b]
        for name, off in (("wq", 0), ("wk", DI), ("wv", 2 * DI)):
            w = wp[:, off + h0 * DH: off + (h0 + 2) * DH]       # [512, 128]
            m[name] = np.ascontiguousarray(
                w.reshape(4, 128, 128).transpose(1, 0, 2))       # [128, 4, 128]
        rows = w_out[h0 * DH:(h0 + 2) * DH, :]                   # [128, 512]
        m["wo"] = np.ascontiguousarray(rows.astype(ml_dtypes.bfloat16))
        in_maps.append(m)
    return in_maps


def run(tokens, norm_weight, w_qkv, w_out, trace=False):
    from concourse.bass_utils import run_bass_kernel_spmd
    nc = get_program()
    in_maps = _prep_inputs(tokens, norm_weight, w_qkv, w_out)
    res = run_bass_kernel_spmd(nc, in_maps, core_ids=list(range(NCORES)), trace=trace)
    parts = [res.results[c]["out_part"] for c in range(NCORES)]
    out = np.empty((B, N, D), dtype=np.float32)
    for b in range(B):
        out[b] = parts[4 * b] + parts[4 * b + 1] + parts[4 * b + 2] + parts[4 * b + 3]
    return out, res


def kernel(tokens, norm_weight, w_qkv, w_out):
    out, _ = run(tokens, norm_weight, w_qkv, w_out, trace=False)
    return out


# revision 49
# speedup vs baseline: 1.4181x; 1.0022x over previous
"""TRN2 Bass kernel for nn_Attention (RMSNorm + QKV + softmax attention + out-proj).

Sharding: 8 cores = 2 batches x 4 head-pairs. Core c handles batch c//4 and
heads (2*(c%4), 2*(c%4)+1). Each core computes its partial out-projection
(contracting only its 128 rows of dim_inner); host sums the 4 partials per batch.

V2 design (cost-model driven):
  A) stream tokens [128,512] blocks: RMSNorm stats on DVE, rstd scale, PE
     transpose -> x^T (rotating [128,4,512] per 512-token chunk)
  B) QKV^T = w^T @ x^T (PE); V^T -> bf16 -> transposed back to V-natural with
     a ones column appended per head (softmax denominator for free)
  C) flash attention per (ib=512 queries, head): for each j-tile of 128 keys:
     S^T [128j, 512i] on PE (f32r, 512 cycles); exp split across THREE engines
     (ACT: exact table exp -> bf16; DVE/GPSIMD: Schraudolph bit-trick exp via
     tensor_scalar -> int16 bits of bf16). PV uses exp(S)^T tiles as the
     STATIONARY [128j, 128i] and bf16 V[128j,64+1] as MOVING: 65-cycle
     matmuls accumulate O-natural [128i, 65] in PSUM (col 64 = l).
  D) tail: linv=1/l (DVE), scale O by linv -> bf16 pair tile [i, 128dh both
     heads] (DVE), PE transpose [128,128], one 128-contraction out-proj matmul
     per i-tile vs bf16 w_out, ACT drains psum->sbuf, DMA out per ib.

Engine budget per core (cost model): PE ~206us (bottleneck), ACT ~150us,
DVE ~180us, Pool ~110us. exp assignment per j-tile keeps all three exp
engines concurrently busy so PE never starves.

Schraudolph exp: bf16_bits(p) = int16(S * (2^7/ln2) + C2B). Bias component
cancels through the softmax denominator; residual sawtooth ~1% std on the
offloaded fraction keeps total rel err well under the 2e-2 gate.

Engine discipline: walrus here allows only ONE semaphore wait per instruction
(fix_waits_nc post-pass splits/dedups), and no mixed 32/16-bit matmuls.
"""
import sys
sys.path.insert(0, "/opt/trn_rl_repo")
import numpy as np

B, N, D = 2, 4096, 512
H, DH = 8, 64
DI = H * DH
NCORES = 8
EPS = 1.1920929e-07  # float32 eps (torch nn.RMSNorm default)

# Schraudolph constants for bf16-bit exp: bits16(e^x) ~= x*C1B + C2B
C1B = 128.0 / 0.6931471805599453          # 2^7 / ln 2
C2B = float(127 * 128) - 366393.0 / 65536.0

_prog_cache = {}


def _patch_drain(tile_mod, mybir):
    """Split the multi-wait tail drain into a chain of single-wait drains
    (this walrus build rejects >1 sync wait per instruction)."""
    if getattr(tile_mod.TileContext, "_drain_patched", False):
        return

    def _patched(self, tick_clock, wait_clock):
        from concourse.vector_clock import ScopedClock
        nc = self.nc
        drain_inst = nc.sync.drain()
        wait_clock.add_sem_waits(drain_inst.ins, ScopedClock({None: tick_clock.global_clock}))
        si = drain_inst.ins.sync_info
        if si is not None and si.on_wait and len(si.on_wait) > 1:
            waits = list(si.on_wait)
            drain_inst.ins.sync_info = mybir.SyncInfo(
                on_wait=waits[:1], on_update=list(si.on_update or []))
            for w in waits[1:]:
                d2 = nc.sync.drain()
                d2.ins.sync_info = mybir.SyncInfo(on_wait=[w], on_update=[])
        nc.all_engine_barrier()
        assert self.sems is not None
        popped = nc._tile_sem_poison_stack.pop()
        assert popped is self._sem_poison
        nc.clear_and_free_semaphores(list(self.sems.allocated().values()))
        nc.all_engine_barrier()

    tile_mod.TileContext._drain_and_barrier = _patched
    tile_mod.TileContext._drain_patched = True


def build_program():
    import concourse.bass as bass
    import concourse.tile as tile
    from concourse import mybir
    from concourse.masks import make_identity

    _patch_drain(tile, mybir)

    F32 = mybir.dt.float32
    F32R = mybir.dt.float32r
    BF16 = mybir.dt.bfloat16
    I16 = mybir.dt.int16
    AF = mybir.ActivationFunctionType
    ALU = mybir.AluOpType

    NIC = N // 512          # 8 chunks of 512 tokens
    NJT = N // 128          # 32 key tiles of 128

    nc = bass.Bass(trn_type="TRN2", target_bir_lowering=False)

    tok = nc.dram_tensor("tok", [N, D], F32, kind="ExternalInput")
    wq = nc.dram_tensor("wq", [128, 4, 128], F32R, kind="ExternalInput")
    wk = nc.dram_tensor("wk", [128, 4, 128], F32R, kind="ExternalInput")
    wv = nc.dram_tensor("wv", [128, 4, 128], F32R, kind="ExternalInput")
    wo = nc.dram_tensor("wo", [128, 512], BF16, kind="ExternalInput")
    out_part = nc.dram_tensor("out_part", [N, D], F32, kind="ExternalOutput")

    tok_r = tok.rearrange("(ic t p) d -> ic p t d", t=4, p=128)
    out_r = out_part.rearrange("(ib t p) e -> ib p t e", t=4, p=128)

    # exp-engine assignment per j-tile PAIR within an iteration (A=ACT exact,
    # D=DVE Schraudolph), weighted so both engines finish together just
    # below the PE's per-iteration time. GPSIMD cannot read PSUM, so it
    # cannot join the exp crew; it takes phase-A SBUF work instead.
    NPAIR = NJT // 2
    def _mk_eng(w_a, w_d):
        pat, cnt = [], {"A": 0, "D": 0}
        wgt = {"A": w_a, "D": w_d}
        for _ in range(NPAIR):
            e = min(cnt, key=lambda k: (cnt[k] + 1) / wgt[k])
            pat.append(e)
            cnt[e] += 1
        return pat
    # alternate the split between iterations so ACT (which also does the
    # out-proj drains) keeps headroom below the PE's per-iteration pace
    ENG_EVEN = _mk_eng(9, 7)
    ENG_ODD = _mk_eng(8, 8)

    with tile.TileContext(nc) as tc:
        with tc.tile_pool(name="consts", bufs=1) as consts, \
             tc.tile_pool(name="big", bufs=1) as big, \
             tc.tile_pool(name="wpool", bufs=1) as wpool:

            # ---- constants ----
            ident_f = consts.tile([128, 128], F32)
            make_identity(nc, ident_f)
            ident = consts.tile([128, 128], F32R)
            nc.vector.tensor_copy(ident, ident_f)
            identb = consts.tile([128, 128], BF16)
            nc.vector.tensor_copy(identb, ident_f)
            eps_t = consts.tile([128, 1], F32)
            nc.vector.memset(eps_t, EPS)
            # preload the Exp ACT table during the otherwise-idle start window
            warm = consts.tile([1, 1], F32)
            nc.scalar.activation(warm, eps_t[0:1, :], AF.Exp)

            # ---- weights ----
            # first token chunk goes ahead of the weights in the DMA queue so
            # the RMSNorm pipeline starts as early as possible
            tok4_first = wpool.tile([128, 4, 512], F32)
            for t in range(4):
                nc.sync.dma_start(out=tok4_first[:, t, :], in_=tok_r[0, :, t, :])
            wq_sb = wpool.tile([128, 4, 128], F32R)
            wk_sb = wpool.tile([128, 4, 128], F32R)
            wv_sb = wpool.tile([128, 4, 128], F32R)
            wo_sb = wpool.tile([128, 512], BF16)
            nc.sync.dma_start(out=wq_sb, in_=wq[:, :, :])
            nc.sync.dma_start(out=wk_sb, in_=wk[:, :, :])
            nc.sync.dma_start(out=wv_sb, in_=wv[:, :, :])
            nc.sync.dma_start(out=wo_sb, in_=wo[:, :])

            # ---- persistent big buffers ----
            QT = big.tile([128, N], F32R)       # [2 heads x 64 qdims, n]
            KT = big.tile([128, N], F32R)
            Vb = big.tile([128, NJT, 2, 65], BF16)  # per j-tile: [v(64)|ones] per head
            ones_bf = consts.tile([128, NJT, 2, 1], BF16)
            nc.vector.memset(ones_bf, 1.0)
            nc.vector.tensor_copy(Vb[:, :, :, 64:65], ones_bf)

            # ---- phase A/B: RMSNorm + x^T + QKV^T + V natural ----
            with tc.tile_pool(name="ab_sbuf", bufs=3) as abp, \
                 tc.tile_pool(name="ab_stats", bufs=8) as stp, \
                 tc.tile_pool(name="ab_psum", bufs=5, space="PSUM") as abps, \
                 tc.tile_pool(name="qk_psum", bufs=3, space="PSUM") as qkps:

                # PE joins: absorb each weight-DMA semaphore with a tiny bf16 matmul
                scr = qkps.tile([2, 2], F32, tag="qk", name="scrj")
                BF = mybir.dt.bfloat16
                for i, wtile in enumerate((wq_sb, wk_sb, wv_sb)):
                    src = wtile[0:1, 0:1, 0:2].bitcast(BF)[:, 0, 1::2]
                    nc.tensor.matmul(scr, src, src, start=(i == 0), stop=False)
                nc.tensor.matmul(scr, wo_sb[0:1, 0:2], wo_sb[0:1, 0:2],
                                 start=False, stop=True)

                def emit_stats_chain(tok4, t):
                    """RMSNorm rstd for one 128-token block."""
                    stats = stp.tile([128, 6], F32, tag="stats")
                    mv = stp.tile([128, 2], F32, tag="mv")
                    ms = stp.tile([128, 1], F32, tag="ms")
                    s_t = stp.tile([128, 1], F32, tag="s_t")
                    rstd = stp.tile([128, 1], F32, tag="rstd")
                    nc.vector.bn_stats(stats, tok4[:, t, :])
                    nc.vector.bn_aggr(mv, stats)
                    # E[x^2] = mean^2 + var
                    nc.vector.scalar_tensor_tensor(
                        ms, mv[:, 0:1], mv[:, 0:1], mv[:, 1:2],
                        op0=ALU.mult, op1=ALU.add)
                    nc.scalar.activation(s_t, ms, AF.Sqrt, bias=eps_t, scale=1.0)
                    nc.vector.reciprocal(rstd, s_t)
                    return rstd

                def emit_stage2_t(icp, tok4p, rstds, xtp, t):
                    """normalize + transpose one 128-token block of the
                    PREVIOUS chunk (its rstd is long since ready)."""
                    xn = stp.tile([128, 512], F32R, tag="xn")
                    nc.gpsimd.tensor_scalar_mul(xn, in0=tok4p[:, t, :], scalar1=rstds[t])
                    tp = abps.tile([128, 4, 128], F32R, tag="tp")
                    for c in range(4):
                        nc.tensor.transpose(tp[:, c, :], xn[:, c * 128:(c + 1) * 128], ident)
                    nc.scalar.copy(xtp[:, :, t * 128:(t + 1) * 128], tp)

                def emit_qkv_half(state, half):
                    # one 256-token half of QKV^T: the first half only needs
                    # token blocks t0/t1, so the PE starts projecting while
                    # t2/t3 are still normalizing. 256-free keeps f32r at
                    # 1 cycle/row.
                    xtp = state["xt"]
                    lo, hi = (0, 256) if half == 0 else (256, 512)
                    for idx, wtile in enumerate((wv_sb, wq_sb, wk_sb)):
                        if half == 0:
                            state["ps"].append(qkps.tile(
                                [128, 512], F32, tag="qk", name=f"qk{idx}"))
                        ps = state["ps"][idx]
                        for c in range(4):
                            # the bank-zeroing start is only on the first
                            # matmul of the bank (it zeroes both halves)
                            nc.tensor.matmul(
                                ps[:, lo:hi], wtile[:, c, :], xtp[:, c, lo:hi],
                                start=(half == 0 and c == 0),
                                stop=(half == 1 and c == 3),
                                skip_group_check=True)

                def emit_qkv_drain(icp, state):
                    vt = abp.tile([128, 512], BF16, tag="vt")
                    nc.scalar.copy(vt, state["ps"][0])
                    # ACT drains these: DVE is the phase-A critical path
                    nc.vector.tensor_copy(QT[:, icp * 512:(icp + 1) * 512], state["ps"][1])
                    nc.scalar.copy(KT[:, icp * 512:(icp + 1) * 512], state["ps"][2])
                    state["vt"] = vt

                def emit_vb(icp, state):
                    # V^T -> V natural (bf16) into Vb (j on partitions);
                    # deferred into the next chunk's stream so the PE doesn't
                    # stall on the vt drain at the chunk boundary
                    vt = state["vt"]
                    vtp = abps.tile([128, 4, 128], BF16, tag="tp")
                    for jl in range(4):
                        nc.tensor.transpose(vtp[:, jl, :], vt[:, jl * 128:(jl + 1) * 128], identb)
                    nc.vector.tensor_copy(
                        Vb[:, icp * 4:(icp + 1) * 4, :, 0:64],
                        vtp.rearrange("p jl (h v) -> p jl h v", h=2))

                # one-chunk software pipeline: stats chains for chunk ic run
                # while chunk ic-1 is normalized/transposed/projected, so the
                # PE never waits on the DVE->ACT->DVE norm chain
                prev = None
                done = None
                for ic in range(NIC + 1):
                    if ic < NIC:
                        if ic == 0:
                            tok4 = tok4_first
                        else:
                            tok4 = abp.tile([128, 4, 512], F32, tag="tok4")
                            nc.sync.dma_start(out=tok4, in_=tok_r[ic])
                        cur_rstds = []
                        cur_state = {"ic": ic, "tok4": tok4,
                                     "rstds": cur_rstds, "ps": [],
                                     "xt": abp.tile([128, 4, 512], F32R,
                                                    tag="xt", name=f"xt{ic}")}
                    for t in range(4):
                        if prev is not None:
                            emit_stage2_t(prev["ic"], prev["tok4"],
                                          prev["rstds"], prev["xt"], t)
                            if t == 1:
                                emit_qkv_half(prev, 0)
                        if ic < NIC:
                            cur_rstds.append(emit_stats_chain(tok4, t))
                            if ic == 0:
                                # chunk 0: normalize inline, right behind its
                                # own stats chain
                                emit_stage2_t(0, tok4, cur_rstds,
                                              cur_state["xt"], t)
                                if t == 1:
                                    emit_qkv_half(cur_state, 0)
                        if t == 2 and done is not None:
                            emit_vb(done["ic"], done)
                            done = None
                    if prev is not None:
                        emit_qkv_half(prev, 1)
                        emit_qkv_drain(prev["ic"], prev)
                        done = prev
                        prev = None
                    if ic == 0:
                        emit_qkv_half(cur_state, 1)
                        emit_qkv_drain(0, cur_state)
                        done = cur_state
                    elif ic < NIC:
                        prev = cur_state
                if done is not None:
                    emit_vb(done["ic"], done)
                    done = None

            # ---- phase C: attention + out-proj (j-tile pairs) ----
            with tc.tile_pool(name="c_sbuf", bufs=3) as cp, \
                 tc.tile_pool(name="pt_pool", bufs=6) as ptp, \
                 tc.tile_pool(name="opair_pool", bufs=2) as oprp, \
                 tc.tile_pool(name="outsb_pool", bufs=2) as outsbp, \
                 tc.tile_pool(name="st_psum", bufs=3, space="PSUM") as stps, \
                 tc.tile_pool(name="o_psum", bufs=2, space="PSUM") as ops:

                iters = [(ib, hl) for ib in range(NIC) for hl in range(2)]
                NIT = len(iters)
                LOOK = 2  # S^T pairs emitted ahead of exp/PV

                def emit_st(k):
                    """S^T for j-tile pair p of iteration k//NPAIR."""
                    it_idx, p = divmod(k, NPAIR)
                    ib, hl = iters[it_idx]
                    h0 = hl * 64
                    st = stps.tile([128, 2, 512], F32, tag="st", name="stg")
                    for jl in range(2):
                        jt = 2 * p + jl
                        nc.tensor.matmul(
                            st[:, jl, :],
                            KT[h0:h0 + 64, jt * 128:(jt + 1) * 128],
                            QT[h0:h0 + 64, ib * 512:(ib + 1) * 512],
                            start=True, stop=True)
                    return st

                def emit_exp(st, p, it_idx):
                    pt = ptp.tile([128, 2, 512], BF16, tag="pt", name="ptg")
                    e = (ENG_EVEN if it_idx % 2 == 0 else ENG_ODD)[p]
                    if e == "A":
                        nc.scalar.activation(
                            pt.rearrange("a b c -> a (b c)"),
                            st.rearrange("a b c -> a (b c)"), AF.Exp)
                    else:
                        nc.vector.tensor_scalar(
                            pt.bitcast(I16).rearrange("a b c -> a (b c)"),
                            st.rearrange("a b c -> a (b c)"), C1B, C2B,
                            op0=ALU.mult, op1=ALU.add)
                    return pt

                # tail state
                o_accs = {}      # iter idx -> o_acc psum tile
                o_pairs = {}     # ib -> o_pair sbuf tile
                linvs = {}       # iter idx -> linv tile
                out_sbs = {}     # ib -> out_sb tile
                ot_tiles = {}    # ib -> (ot psum tile, ots sbuf tile)
                op_tiles = {}    # it -> out-proj psum tile (transient per ib)

                def emit_tail_step(k_prev, g):
                    """Interleave iter k_prev's tail into iter k_prev+1's
                    pair-step stream at slot g (14 usable slots)."""
                    ib, hl = iters[k_prev]
                    if g == 0:
                        o_acc = o_accs[k_prev]
                        linv = cp.tile([128, 4], F32, tag="linv", name=f"lv{hl}")
                        nc.vector.reciprocal(linv, o_acc[:, :, 64])
                        linvs[k_prev] = linv
                        if hl == 0:
                            o_pairs[ib] = oprp.tile(
                                [128, 4, 128], BF16, tag="opair", name=f"op{ib}")
                    elif 1 <= g <= 4:
                        it = g - 1
                        o_acc = o_accs[k_prev]
                        nc.vector.tensor_scalar_mul(
                            o_pairs[ib][:, it, hl * 64:(hl + 1) * 64],
                            in0=o_acc[:, it, 0:64],
                            scalar1=linvs[k_prev][:, it:it + 1])
                        if it == 3:
                            o_accs.pop(k_prev)
                            linvs.pop(k_prev)
                    elif hl == 1 and g == 5:
                        # ONE 2-bank ring slot serves the whole out-proj tail:
                        # bank A holds the transposed pair-tiles (bf16 view),
                        # bank B is reused by all 4 out-proj matmuls. This
                        # steals 1 ring slot per tail window instead of 5.
                        opr = o_pairs[ib]
                        tail_t = stps.tile([128, 2, 512], F32, tag="st",
                                           name="tailt")
                        ot = tail_t[:, 0, 0:256].bitcast(BF16).rearrange(
                            "p (a b) -> p a b", a=4)
                        ot_tiles[ib] = [tail_t, ot, None]
                        for it in range(4):
                            nc.tensor.matmul(
                                ot[:, it, :], opr[:, it, :], identb,
                                is_transpose=True, start=(it == 0), stop=(it == 3),
                                skip_group_check=True)
                    elif hl == 1 and g == 6:
                        # ACT does this drain: a copy on DVE here delays its
                        # next exp pairs and stalls the PE
                        ots = cp.tile([128, 4, 128], BF16, tag="ots", name="otsb")
                        nc.scalar.copy(ots, ot_tiles[ib][1])
                        ot_tiles[ib][2] = ots
                        o_pairs.pop(ib)
                    elif hl == 1 and 7 <= g <= 11:
                        # g=7..10: out-proj matmul it=g-7; g=8..11: drain it=g-8
                        if g >= 8:
                            it_d = g - 8
                            nc.scalar.copy(out_sbs[ib][:, it_d, :],
                                           op_tiles.pop(it_d))
                            # per-it DMA so the store overlaps later drains
                            nc.sync.dma_start(out=out_r[ib, :, it_d, :],
                                              in_=out_sbs[ib][:, it_d, :])
                            if it_d == 3:
                                out_sbs.pop(ib)
                                ot_tiles.pop(ib)
                        if g <= 10:
                            it = g - 7
                            if it == 0:
                                out_sbs[ib] = outsbp.tile(
                                    [128, 4, 512], F32, tag="outsb", name=f"ob{ib}")
                            # bank B of the shared tail tile, reused per it
                            op_ps = ot_tiles[ib][0][:, 1, :]
                            nc.tensor.matmul(op_ps, ot_tiles[ib][2][:, it, :],
                                             wo_sb, start=True, stop=True)
                            op_tiles[it] = op_ps

                def emit_pv(it_idx, p, pt):
                    _, hl = iters[it_idx]
                    if p == 0:
                        o_accs[it_idx] = ops.tile(
                            [128, 4, 65], F32, tag="o", name=f"oacc{it_idx % 2}")
                    o_acc = o_accs[it_idx]
                    for jl in range(2):
                        jt = 2 * p + jl
                        for it in range(4):
                            # PSUM `start` zeroes the WHOLE bank: only the
                            # very first matmul of this o_acc bank may set it
                            nc.tensor.matmul(
                                o_acc[:, it, :],
                                pt[:, jl, it * 128:(it + 1) * 128],
                                Vb[:, jt, hl, :],
                                start=(jt == 0 and it == 0),
                                stop=(jt == NJT - 1 and it == 3),
                                skip_group_check=True)

                # ---- attention pipeline over (iteration, j-tile pair) steps:
                # PV lags exp by one step so every exp has an extra pair-step
                # of slack before the PE consumes its output ----
                TOT = NIT * NPAIR
                sts = {}
                pts = {}
                for k in range(LOOK):
                    sts[k] = emit_st(k)
                for k in range(TOT + 3):
                    if k < TOT:
                        it_idx, p = divmod(k, NPAIR)
                        pts[k] = emit_exp(sts.pop(k), p, it_idx)
                        if k + LOOK < TOT:
                            sts[k + LOOK] = emit_st(k + LOOK)
                    if k >= 3:
                        emit_pv(*divmod(k - 3, NPAIR), pts.pop(k - 3))
                    if k < TOT and it_idx > 0 and p >= 3:
                        # tail slots start at p==3: the previous iteration's
                        # final (3-step-lagged) PV lands at p==2, and the
                        # tail's linv must observe it
                        emit_tail_step(it_idx - 1, p - 3)

                # trailing tail for the final iteration: no exp traffic to
                # pace against, so run a tight per-it pipeline instead
                kl = NIT - 1
                ibl, _ = iters[kl]
                o_acc = o_accs.pop(kl)
                linv = cp.tile([128, 4], F32, tag="linv", name="lvf")
                nc.vector.reciprocal(linv, o_acc[:, :, 64])
                oprl = o_pairs[ibl]
                otl = stps.tile([128, 4, 128], BF16, tag="st", name="otpf")
                out_sbl = outsbp.tile([128, 4, 512], F32, tag="outsb", name="obf")
                opsl = {}
                for it in range(4):
                    nc.vector.tensor_scalar_mul(
                        oprl[:, it, 64:128], in0=o_acc[:, it, 0:64],
                        scalar1=linv[:, it:it + 1])
                    nc.tensor.matmul(
                        otl[:, it, :], oprl[:, it, :], identb,
                        is_transpose=True, start=(it == 0), stop=(it == 3),
                        skip_group_check=True)
                    # per-it ots copy so each out-proj starts immediately
                    otsl = cp.tile([128, 128], BF16, tag="ots", name=f"otsf{it}")
                    nc.scalar.copy(otsl, otl[:, it, :])
                    op_ps = stps.tile([128, 512], F32, tag="st", name=f"opf{it}")
                    nc.tensor.matmul(op_ps, otsl, wo_sb, start=True, stop=True)
                    opsl[it] = op_ps
                    if it >= 1:
                        nc.scalar.copy(out_sbl[:, it - 1, :], opsl.pop(it - 1))
                        nc.sync.dma_start(out=out_r[ibl, :, it - 1, :],
                                          in_=out_sbl[:, it - 1, :])
                nc.scalar.copy(out_sbl[:, 3, :], opsl.pop(3))
                nc.sync.dma_start(out=out_r[ibl, :, 3, :], in_=out_sbl[:, 3, :])

    fix_waits_nc(nc, mybir)
    return nc


def fix_waits_nc(nc, mybir):
    """Post-pass over the scheduled program: (1) remove semaphore waits that
    are transitively implied by earlier waits (Tile emits per-proc-minimal,
    not transitively-minimal, waits), (2) split any instruction still
    carrying more than one wait by injecting single-wait NoOps in front of
    it — this walrus build rejects >1 sync wait per instruction.
    Mutates nc in place so CoreSim and hardware run identical sync."""
    nop_id = [0]

    def _is_ge(w):
        return w.sync_type == "semaphore" and w.wait_mode == "sem-ge-imm"

    for fn in nc.m.functions:
        for blk in fn.blocks:
            insts = list(blk.instructions)
            n = len(insts)

            producers = {}
            cum = {}
            nonmono = set()  # sems ever decremented: counter logic invalid
            for idx, inst in enumerate(insts):
                si = inst.sync_info
                for u in (si.on_update if si else []) or []:
                    if u.sync_type != "semaphore":
                        continue
                    sid = u.id
                    if u.update_mode != "sem-inc":
                        nonmono.add(sid)
                        continue
                    cum[sid] = cum.get(sid, 0) + int(u.update_value)
                    producers.setdefault(sid, []).append((cum[sid], idx))

            def producer_of(sid, val):
                for cv, idx in producers.get(sid, ()):
                    if cv >= val:
                        return idx
                return None

            prev_eng = [None] * n
            last = {}
            for idx, inst in enumerate(insts):
                e = inst.engine
                prev_eng[idx] = last.get(e)
                last[e] = idx

            def get_waits(inst):
                si = inst.sync_info
                return list(si.on_wait) if si and si.on_wait else []

            def is_ge(w):
                return _is_ge(w) and w.id not in nonmono

            know = [dict() for _ in range(n)]
            for _ in range(3):
                changed = False
                for idx, inst in enumerate(insts):
                    k = dict(know[prev_eng[idx]]) if prev_eng[idx] is not None else {}
                    for w in get_waits(inst):
                        if not is_ge(w):
                            continue
                        sid, val = w.id, int(w.wait_value)
                        if k.get(sid, -1) < val:
                            k[sid] = val
                        p = producer_of(sid, val)
                        if p is not None:
                            for s2, v2 in know[p].items():
                                if k.get(s2, -1) < v2:
                                    k[s2] = v2
                    if k != know[idx]:
                        know[idx] = k
                        changed = True
                if not changed:
                    break

            new_insts = []
            dirty = False
            for idx, inst in enumerate(insts):
                si = inst.sync_info
                waits = get_waits(inst)
                if si is not None and waits:
                    base = dict(know[prev_eng[idx]]) if prev_eng[idx] is not None else {}
                    kept = []
                    for w in waits:
                        if is_ge(w):
                            sid, val = w.id, int(w.wait_value)
                            if base.get(sid, -1) >= val:
                                continue
                            base[sid] = val
                            p = producer_of(sid, val)
                            if p is not None:
                                for s2, v2 in know[p].items():
                                    if base.get(s2, -1) < v2:
                                        base[s2] = v2
                        kept.append(w)
                    if len(kept) != len(waits) or len(kept) > 1:
                        dirty = True
                        for w in kept[:-1]:
                            nop_id[0] += 1
                            nop = mybir.InstNoOp(
                                name=f"I-waitfix-{nop_id[0]}", ins=[], outs=[])
                            nop.engine = inst.engine
                            nop.sync_info = mybir.SyncInfo(on_wait=[w], on_update=[])
                            nc.register_instruction(nop)
                            new_insts.append(nop)
                        inst.sync_info = mybir.SyncInfo(
                            on_wait=kept[-1:],
                            on_update=list(si.on_update or []))
                new_insts.append(inst)
            if dirty:
                blk.instructions = new_insts


def get_program():
    if "nc" not in _prog_cache:
        _prog_cache["nc"] = build_program()
    return _prog_cache["nc"]


def _prep_inputs(tokens, norm_weight, w_qkv, w_out):
    import ml_dtypes
    tokens = np.ascontiguousarray(np.asarray(tokens, dtype=np.float32))
    norm_weight = np.asarray(norm_weight, dtype=np.float32)
    w_qkv = np.asarray(w_qkv, dtype=np.float32)
    w_out = np.asarray(w_out, dtype=np.float32)

    wp = w_qkv * norm_weight[:, None]  # fold RMSNorm weight into qkv weights

    in_maps = []
    for c in range(NCORES):
        b = c // 4
        h0 = 2 * (c % 4)
        m = {}
        m["tok"] = tokens[b]
        for name, off in (("wq", 0), ("wk", DI), ("wv", 2 * DI)):
            w = wp[:, off + h0 * DH: off + (h0 + 2) * DH]       # [512, 128]
            m[name] = np.ascontiguousarray(
                w.reshape(4, 128, 128).transpose(1, 0, 2))       # [128, 4, 128]
        rows = w_out[h0 * DH:(h0 + 2) * DH, :]                   # [128, 512]
        m["wo"] = np.ascontiguousarray(rows.astype(ml_dtypes.bfloat16))
        in_maps.append(m)
    return in_maps


def run(tokens, norm_weight, w_qkv, w_out, trace=False):
    from concourse.bass_utils import run_bass_kernel_spmd
    nc = get_program()
    in_maps = _prep_inputs(tokens, norm_weight, w_qkv, w_out)
    res = run_bass_kernel_spmd(nc, in_maps, core_ids=list(range(NCORES)), trace=trace)
    parts = [res.results[c]["out_part"] for c in range(NCORES)]
    out = np.empty((B, N, D), dtype=np.float32)
    for b in range(B):
        out[b] = parts[4 * b] + parts[4 * b + 1] + parts[4 * b + 2] + parts[4 * b + 3]
    return out, res


def kernel(tokens, norm_weight, w_qkv, w_out):
    out, _ = run(tokens, norm_weight, w_qkv, w_out, trace=False)
    return out


# revision 52
# speedup vs baseline: 1.4222x; 1.0029x over previous
"""TRN2 Bass kernel for nn_Attention (RMSNorm + QKV + softmax attention + out-proj).

Sharding: 8 cores = 2 batches x 4 head-pairs. Core c handles batch c//4 and
heads (2*(c%4), 2*(c%4)+1). Each core computes its partial out-projection
(contracting only its 128 rows of dim_inner); host sums the 4 partials per batch.

V2 design (cost-model driven):
  A) stream tokens [128,512] blocks: RMSNorm stats on DVE, rstd scale, PE
     transpose -> x^T (rotating [128,4,512] per 512-token chunk)
  B) QKV^T = w^T @ x^T (PE); V^T -> bf16 -> transposed back to V-natural with
     a ones column appended per head (softmax denominator for free)
  C) flash attention per (ib=512 queries, head): for each j-tile of 128 keys:
     S^T [128j, 512i] on PE (f32r, 512 cycles); exp split across THREE engines
     (ACT: exact table exp -> bf16; DVE/GPSIMD: Schraudolph bit-trick exp via
     tensor_scalar -> int16 bits of bf16). PV uses exp(S)^T tiles as the
     STATIONARY [128j, 128i] and bf16 V[128j,64+1] as MOVING: 65-cycle
     matmuls accumulate O-natural [128i, 65] in PSUM (col 64 = l).
  D) tail: linv=1/l (DVE), scale O by linv -> bf16 pair tile [i, 128dh both
     heads] (DVE), PE transpose [128,128], one 128-contraction out-proj matmul
     per i-tile vs bf16 w_out, ACT drains psum->sbuf, DMA out per ib.

Engine budget per core (cost model): PE ~206us (bottleneck), ACT ~150us,
DVE ~180us, Pool ~110us. exp assignment per j-tile keeps all three exp
engines concurrently busy so PE never starves.

Schraudolph exp: bf16_bits(p) = int16(S * (2^7/ln2) + C2B). Bias component
cancels through the softmax denominator; residual sawtooth ~1% std on the
offloaded fraction keeps total rel err well under the 2e-2 gate.

Engine discipline: walrus here allows only ONE semaphore wait per instruction
(fix_waits_nc post-pass splits/dedups), and no mixed 32/16-bit matmuls.
"""
import sys
sys.path.insert(0, "/opt/trn_rl_repo")
import numpy as np

B, N, D = 2, 4096, 512
H, DH = 8, 64
DI = H * DH
NCORES = 8
EPS = 1.1920929e-07  # float32 eps (torch nn.RMSNorm default)

# Schraudolph constants for bf16-bit exp: bits16(e^x) ~= x*C1B + C2B
C1B = 128.0 / 0.6931471805599453          # 2^7 / ln 2
C2B = float(127 * 128) - 366393.0 / 65536.0

_prog_cache = {}


def _patch_drain(tile_mod, mybir):
    """Split the multi-wait tail drain into a chain of single-wait drains
    (this walrus build rejects >1 sync wait per instruction)."""
    if getattr(tile_mod.TileContext, "_drain_patched", False):
        return

    def _patched(self, tick_clock, wait_clock):
        from concourse.vector_clock import ScopedClock
        nc = self.nc
        drain_inst = nc.sync.drain()
        wait_clock.add_sem_waits(drain_inst.ins, ScopedClock({None: tick_clock.global_clock}))
        si = drain_inst.ins.sync_info
        if si is not None and si.on_wait and len(si.on_wait) > 1:
            waits = list(si.on_wait)
            drain_inst.ins.sync_info = mybir.SyncInfo(
                on_wait=waits[:1], on_update=list(si.on_update or []))
            for w in waits[1:]:
                d2 = nc.sync.drain()
                d2.ins.sync_info = mybir.SyncInfo(on_wait=[w], on_update=[])
        nc.all_engine_barrier()
        assert self.sems is not None
        popped = nc._tile_sem_poison_stack.pop()
        assert popped is self._sem_poison
        nc.clear_and_free_semaphores(list(self.sems.allocated().values()))
        nc.all_engine_barrier()

    tile_mod.TileContext._drain_and_barrier = _patched
    tile_mod.TileContext._drain_patched = True


def build_program():
    import concourse.bass as bass
    import concourse.tile as tile
    from concourse import mybir
    from concourse.masks import make_identity

    _patch_drain(tile, mybir)

    F32 = mybir.dt.float32
    F32R = mybir.dt.float32r
    BF16 = mybir.dt.bfloat16
    I16 = mybir.dt.int16
    AF = mybir.ActivationFunctionType
    ALU = mybir.AluOpType

    NIC = N // 512          # 8 chunks of 512 tokens
    NJT = N // 128          # 32 key tiles of 128

    nc = bass.Bass(trn_type="TRN2", target_bir_lowering=False)

    tok = nc.dram_tensor("tok", [N, D], F32, kind="ExternalInput")
    wq = nc.dram_tensor("wq", [128, 4, 128], F32R, kind="ExternalInput")
    wk = nc.dram_tensor("wk", [128, 4, 128], F32R, kind="ExternalInput")
    wv = nc.dram_tensor("wv", [128, 4, 128], F32R, kind="ExternalInput")
    wo = nc.dram_tensor("wo", [128, 512], BF16, kind="ExternalInput")
    out_part = nc.dram_tensor("out_part", [N, D], F32, kind="ExternalOutput")

    tok_r = tok.rearrange("(ic t p) d -> ic p t d", t=4, p=128)
    out_r = out_part.rearrange("(ib t p) e -> ib p t e", t=4, p=128)

    # exp-engine assignment per j-tile PAIR within an iteration (A=ACT exact,
    # D=DVE Schraudolph), weighted so both engines finish together just
    # below the PE's per-iteration time. GPSIMD cannot read PSUM, so it
    # cannot join the exp crew; it takes phase-A SBUF work instead.
    NPAIR = NJT // 2
    def _mk_eng(w_a, w_d):
        pat, cnt = [], {"A": 0, "D": 0}
        wgt = {"A": w_a, "D": w_d}
        for _ in range(NPAIR):
            e = min(cnt, key=lambda k: (cnt[k] + 1) / wgt[k])
            pat.append(e)
            cnt[e] += 1
        return pat
    # alternate the split between iterations so ACT (which also does the
    # out-proj drains) keeps headroom below the PE's per-iteration pace
    ENG_EVEN = _mk_eng(9, 7)
    ENG_ODD = _mk_eng(8, 8)

    with tile.TileContext(nc) as tc:
        with tc.tile_pool(name="consts", bufs=1) as consts, \
             tc.tile_pool(name="big", bufs=1) as big, \
             tc.tile_pool(name="wpool", bufs=1) as wpool:

            # ---- constants ----
            ident_f = consts.tile([128, 128], F32)
            make_identity(nc, ident_f)
            ident = consts.tile([128, 128], F32R)
            nc.vector.tensor_copy(ident, ident_f)
            identb = consts.tile([128, 128], BF16)
            nc.vector.tensor_copy(identb, ident_f)
            eps_t = consts.tile([128, 1], F32)
            nc.vector.memset(eps_t, EPS)
            # preload the Exp ACT table during the otherwise-idle start window
            warm = consts.tile([1, 1], F32)
            nc.scalar.activation(warm, eps_t[0:1, :], AF.Exp)

            # ---- weights ----
            # first token chunk goes ahead of the weights in the DMA queue so
            # the RMSNorm pipeline starts as early as possible
            tok4_first = wpool.tile([128, 4, 512], F32)
            for t in range(4):
                nc.sync.dma_start(out=tok4_first[:, t, :], in_=tok_r[0, :, t, :])
            wq_sb = wpool.tile([128, 4, 128], F32R)
            wk_sb = wpool.tile([128, 4, 128], F32R)
            wv_sb = wpool.tile([128, 4, 128], F32R)
            wo_sb = wpool.tile([128, 512], BF16)
            nc.sync.dma_start(out=wq_sb, in_=wq[:, :, :])
            nc.sync.dma_start(out=wk_sb, in_=wk[:, :, :])
            nc.sync.dma_start(out=wv_sb, in_=wv[:, :, :])
            nc.sync.dma_start(out=wo_sb, in_=wo[:, :])

            # ---- persistent big buffers ----
            QT = big.tile([128, N], F32R)       # [2 heads x 64 qdims, n]
            KT = big.tile([128, N], F32R)
            Vb = big.tile([128, NJT, 2, 65], BF16)  # per j-tile: [v(64)|ones] per head
            ones_bf = consts.tile([128, NJT, 2, 1], BF16)
            nc.vector.memset(ones_bf, 1.0)
            nc.vector.tensor_copy(Vb[:, :, :, 64:65], ones_bf)

            # ---- phase A/B: RMSNorm + x^T + QKV^T + V natural ----
            with tc.tile_pool(name="ab_sbuf", bufs=4) as abp, \
                 tc.tile_pool(name="ab_stats", bufs=8) as stp, \
                 tc.tile_pool(name="ab_psum", bufs=5, space="PSUM") as abps, \
                 tc.tile_pool(name="qk_psum", bufs=3, space="PSUM") as qkps:

                # PE joins: absorb each weight-DMA semaphore with a tiny bf16 matmul
                scr = qkps.tile([2, 2], F32, tag="qk", name="scrj")
                BF = mybir.dt.bfloat16
                for i, wtile in enumerate((wq_sb, wk_sb, wv_sb)):
                    src = wtile[0:1, 0:1, 0:2].bitcast(BF)[:, 0, 1::2]
                    nc.tensor.matmul(scr, src, src, start=(i == 0), stop=False)
                nc.tensor.matmul(scr, wo_sb[0:1, 0:2], wo_sb[0:1, 0:2],
                                 start=False, stop=True)

                def emit_stats_chain(tok4, t):
                    """RMSNorm rstd for one 128-token block."""
                    stats = stp.tile([128, 6], F32, tag="stats")
                    mv = stp.tile([128, 2], F32, tag="mv")
                    ms = stp.tile([128, 1], F32, tag="ms")
                    s_t = stp.tile([128, 1], F32, tag="s_t")
                    rstd = stp.tile([128, 1], F32, tag="rstd")
                    nc.vector.bn_stats(stats, tok4[:, t, :])
                    nc.vector.bn_aggr(mv, stats)
                    # E[x^2] = mean^2 + var
                    nc.vector.scalar_tensor_tensor(
                        ms, mv[:, 0:1], mv[:, 0:1], mv[:, 1:2],
                        op0=ALU.mult, op1=ALU.add)
                    nc.scalar.activation(s_t, ms, AF.Sqrt, bias=eps_t, scale=1.0)
                    nc.vector.reciprocal(rstd, s_t)
                    return rstd

                def emit_stage2_t(icp, tok4p, rstds, xtp, t):
                    """normalize + transpose one 128-token block of the
                    PREVIOUS chunk (its rstd is long since ready)."""
                    xn = stp.tile([128, 512], F32R, tag="xn")
                    nc.gpsimd.tensor_scalar_mul(xn, in0=tok4p[:, t, :], scalar1=rstds[t])
                    tp = abps.tile([128, 4, 128], F32R, tag="tp")
                    for c in range(4):
                        nc.tensor.transpose(tp[:, c, :], xn[:, c * 128:(c + 1) * 128], ident)
                    nc.scalar.copy(xtp[:, :, t * 128:(t + 1) * 128], tp)

                def emit_qkv_half(state, half):
                    # one 256-token half of QKV^T: the first half only needs
                    # token blocks t0/t1, so the PE starts projecting while
                    # t2/t3 are still normalizing. 256-free keeps f32r at
                    # 1 cycle/row.
                    xtp = state["xt"]
                    lo, hi = (0, 256) if half == 0 else (256, 512)
                    for idx, wtile in enumerate((wv_sb, wq_sb, wk_sb)):
                        if half == 0:
                            state["ps"].append(qkps.tile(
                                [128, 512], F32, tag="qk", name=f"qk{idx}"))
                        ps = state["ps"][idx]
                        for c in range(4):
                            # the bank-zeroing start is only on the first
                            # matmul of the bank (it zeroes both halves)
                            nc.tensor.matmul(
                                ps[:, lo:hi], wtile[:, c, :], xtp[:, c, lo:hi],
                                start=(half == 0 and c == 0),
                                stop=(half == 1 and c == 3),
                                skip_group_check=True)

                def emit_qkv_drain(icp, state):
                    vt = abp.tile([128, 512], BF16, tag="vt")
                    nc.scalar.copy(vt, state["ps"][0])
                    # ACT drains these: DVE is the phase-A critical path
                    nc.vector.tensor_copy(QT[:, icp * 512:(icp + 1) * 512], state["ps"][1])
                    nc.scalar.copy(KT[:, icp * 512:(icp + 1) * 512], state["ps"][2])
                    state["vt"] = vt

                def emit_vb(icp, state):
                    # V^T -> V natural (bf16) into Vb (j on partitions);
                    # deferred into the next chunk's stream so the PE doesn't
                    # stall on the vt drain at the chunk boundary
                    vt = state["vt"]
                    vtp = abps.tile([128, 4, 128], BF16, tag="tp")
                    for jl in range(4):
                        nc.tensor.transpose(vtp[:, jl, :], vt[:, jl * 128:(jl + 1) * 128], identb)
                    nc.vector.tensor_copy(
                        Vb[:, icp * 4:(icp + 1) * 4, :, 0:64],
                        vtp.rearrange("p jl (h v) -> p jl h v", h=2))

                # one-chunk software pipeline: stats chains for chunk ic run
                # while chunk ic-1 is normalized/transposed/projected, so the
                # PE never waits on the DVE->ACT->DVE norm chain
                prev = None
                done = None
                for ic in range(NIC + 1):
                    if ic < NIC:
                        if ic == 0:
                            tok4 = tok4_first
                        else:
                            tok4 = abp.tile([128, 4, 512], F32, tag="tok4")
                            nc.sync.dma_start(out=tok4, in_=tok_r[ic])
                        cur_rstds = []
                        cur_state = {"ic": ic, "tok4": tok4,
                                     "rstds": cur_rstds, "ps": [],
                                     "xt": abp.tile([128, 4, 512], F32R,
                                                    tag="xt", name=f"xt{ic}")}
                    for t in range(4):
                        if prev is not None:
                            emit_stage2_t(prev["ic"], prev["tok4"],
                                          prev["rstds"], prev["xt"], t)
                            if t == 1:
                                emit_qkv_half(prev, 0)
                        if ic < NIC:
                            cur_rstds.append(emit_stats_chain(tok4, t))
                            if ic == 0:
                                # chunk 0: normalize inline, right behind its
                                # own stats chain
                                emit_stage2_t(0, tok4, cur_rstds,
                                              cur_state["xt"], t)
                                if t == 1:
                                    emit_qkv_half(cur_state, 0)
                        if t == 2 and done is not None:
                            emit_vb(done["ic"], done)
                            done = None
                    if prev is not None:
                        emit_qkv_half(prev, 1)
                        emit_qkv_drain(prev["ic"], prev)
                        done = prev
                        prev = None
                    if ic == 0:
                        emit_qkv_half(cur_state, 1)
                        emit_qkv_drain(0, cur_state)
                        done = cur_state
                    elif ic < NIC:
                        prev = cur_state
                if done is not None:
                    emit_vb(done["ic"], done)
                    done = None

            # ---- phase C: attention + out-proj (j-tile pairs) ----
            with tc.tile_pool(name="c_sbuf", bufs=4) as cp, \
                 tc.tile_pool(name="pt_pool", bufs=8) as ptp, \
                 tc.tile_pool(name="opair_pool", bufs=3) as oprp, \
                 tc.tile_pool(name="outsb_pool", bufs=3) as outsbp, \
                 tc.tile_pool(name="st_psum", bufs=3, space="PSUM") as stps, \
                 tc.tile_pool(name="o_psum", bufs=2, space="PSUM") as ops:

                iters = [(ib, hl) for ib in range(NIC) for hl in range(2)]
                NIT = len(iters)
                LOOK = 2  # S^T pairs emitted ahead of exp/PV

                def emit_st(k):
                    """S^T for j-tile pair p of iteration k//NPAIR."""
                    it_idx, p = divmod(k, NPAIR)
                    ib, hl = iters[it_idx]
                    h0 = hl * 64
                    st = stps.tile([128, 2, 512], F32, tag="st", name="stg")
                    for jl in range(2):
                        jt = 2 * p + jl
                        nc.tensor.matmul(
                            st[:, jl, :],
                            KT[h0:h0 + 64, jt * 128:(jt + 1) * 128],
                            QT[h0:h0 + 64, ib * 512:(ib + 1) * 512],
                            start=True, stop=True)
                    return st

                def emit_exp(st, p, it_idx):
                    pt = ptp.tile([128, 2, 512], BF16, tag="pt", name="ptg")
                    e = (ENG_EVEN if it_idx % 2 == 0 else ENG_ODD)[p]
                    if e == "A":
                        nc.scalar.activation(
                            pt.rearrange("a b c -> a (b c)"),
                            st.rearrange("a b c -> a (b c)"), AF.Exp)
                    else:
                        nc.vector.tensor_scalar(
                            pt.bitcast(I16).rearrange("a b c -> a (b c)"),
                            st.rearrange("a b c -> a (b c)"), C1B, C2B,
                            op0=ALU.mult, op1=ALU.add)
                    return pt

                # tail state
                o_accs = {}      # iter idx -> o_acc psum tile
                o_pairs = {}     # ib -> o_pair sbuf tile
                linvs = {}       # iter idx -> linv tile
                out_sbs = {}     # ib -> out_sb tile
                ot_tiles = {}    # ib -> (ot psum tile, ots sbuf tile)
                op_tiles = {}    # it -> out-proj psum tile (transient per ib)

                def emit_tail_step(k_prev, g):
                    """Interleave iter k_prev's tail into iter k_prev+1's
                    pair-step stream at slot g (14 usable slots)."""
                    ib, hl = iters[k_prev]
                    if g == 0:
                        o_acc = o_accs[k_prev]
                        linv = cp.tile([128, 4], F32, tag="linv", name=f"lv{hl}")
                        nc.vector.reciprocal(linv, o_acc[:, :, 64])
                        linvs[k_prev] = linv
                        if hl == 0:
                            o_pairs[ib] = oprp.tile(
                                [128, 4, 128], BF16, tag="opair", name=f"op{ib}")
                    elif 1 <= g <= 4:
                        it = g - 1
                        o_acc = o_accs[k_prev]
                        nc.vector.tensor_scalar_mul(
                            o_pairs[ib][:, it, hl * 64:(hl + 1) * 64],
                            in0=o_acc[:, it, 0:64],
                            scalar1=linvs[k_prev][:, it:it + 1])
                        if it == 3:
                            o_accs.pop(k_prev)
                            linvs.pop(k_prev)
                    elif hl == 1 and g == 5:
                        # ONE 2-bank ring slot serves the whole out-proj tail:
                        # bank A holds the transposed pair-tiles (bf16 view),
                        # bank B is reused by all 4 out-proj matmuls. This
                        # steals 1 ring slot per tail window instead of 5.
                        opr = o_pairs[ib]
                        tail_t = stps.tile([128, 2, 512], F32, tag="st",
                                           name="tailt")
                        ot = tail_t[:, 0, 0:256].bitcast(BF16).rearrange(
                            "p (a b) -> p a b", a=4)
                        ot_tiles[ib] = [tail_t, ot, None]
                        for it in range(4):
                            nc.tensor.matmul(
                                ot[:, it, :], opr[:, it, :], identb,
                                is_transpose=True, start=(it == 0), stop=(it == 3),
                                skip_group_check=True)
                    elif hl == 1 and g == 6:
                        # ACT does this drain: a copy on DVE here delays its
                        # next exp pairs and stalls the PE
                        ots = cp.tile([128, 4, 128], BF16, tag="ots", name="otsb")
                        nc.scalar.copy(ots, ot_tiles[ib][1])
                        ot_tiles[ib][2] = ots
                        o_pairs.pop(ib)
                    elif hl == 1 and 7 <= g <= 11:
                        # g=7..10: out-proj matmul it=g-7; g=8..11: drain it=g-8
                        if g >= 8:
                            it_d = g - 8
                            nc.scalar.copy(out_sbs[ib][:, it_d, :],
                                           op_tiles.pop(it_d))
                            # per-it DMA so the store overlaps later drains
                            nc.sync.dma_start(out=out_r[ib, :, it_d, :],
                                              in_=out_sbs[ib][:, it_d, :])
                            if it_d == 3:
                                out_sbs.pop(ib)
                                ot_tiles.pop(ib)
                        if g <= 10:
                            it = g - 7
                            if it == 0:
                                out_sbs[ib] = outsbp.tile(
                                    [128, 4, 512], F32, tag="outsb", name=f"ob{ib}")
                            # bank B of the shared tail tile, reused per it
                            op_ps = ot_tiles[ib][0][:, 1, :]
                            nc.tensor.matmul(op_ps, ot_tiles[ib][2][:, it, :],
                                             wo_sb, start=True, stop=True)
                            op_tiles[it] = op_ps

                def emit_pv(it_idx, p, pt):
                    _, hl = iters[it_idx]
                    if p == 0:
                        o_accs[it_idx] = ops.tile(
                            [128, 4, 65], F32, tag="o", name=f"oacc{it_idx % 2}")
                    o_acc = o_accs[it_idx]
                    for jl in range(2):
                        jt = 2 * p + jl
                        for it in range(4):
                            # PSUM `start` zeroes the WHOLE bank: only the
                            # very first matmul of this o_acc bank may set it
                            nc.tensor.matmul(
                                o_acc[:, it, :],
                                pt[:, jl, it * 128:(it + 1) * 128],
                                Vb[:, jt, hl, :],
                                start=(jt == 0 and it == 0),
                                stop=(jt == NJT - 1 and it == 3),
                                skip_group_check=True)

                # ---- attention pipeline over (iteration, j-tile pair) steps:
                # PV lags exp by one step so every exp has an extra pair-step
                # of slack before the PE consumes its output ----
                TOT = NIT * NPAIR
                sts = {}
                pts = {}
                for k in range(LOOK):
                    sts[k] = emit_st(k)
                for k in range(TOT + 3):
                    if k < TOT:
                        it_idx, p = divmod(k, NPAIR)
                        pts[k] = emit_exp(sts.pop(k), p, it_idx)
                        if k + LOOK < TOT:
                            sts[k + LOOK] = emit_st(k + LOOK)
                    if k >= 3:
                        emit_pv(*divmod(k - 3, NPAIR), pts.pop(k - 3))
                    if k < TOT and it_idx > 0 and p >= 3:
                        # tail slots start at p==3: the previous iteration's
                        # final (3-step-lagged) PV lands at p==2, and the
                        # tail's linv must observe it
                        emit_tail_step(it_idx - 1, p - 3)

                # trailing tail for the final iteration: no exp traffic to
                # pace against, so run a tight per-it pipeline instead
                kl = NIT - 1
                ibl, _ = iters[kl]
                o_acc = o_accs.pop(kl)
                linv = cp.tile([128, 4], F32, tag="linv", name="lvf")
                nc.vector.reciprocal(linv, o_acc[:, :, 64])
                oprl = o_pairs[ibl]
                otl = stps.tile([128, 4, 128], BF16, tag="st", name="otpf")
                out_sbl = outsbp.tile([128, 4, 512], F32, tag="outsb", name="obf")
                opsl = {}
                for it in range(4):
                    nc.vector.tensor_scalar_mul(
                        oprl[:, it, 64:128], in0=o_acc[:, it, 0:64],
                        scalar1=linv[:, it:it + 1])
                    nc.tensor.matmul(
                        otl[:, it, :], oprl[:, it, :], identb,
                        is_transpose=True, start=(it == 0), stop=(it == 3),
                        skip_group_check=True)
                    # per-it ots copy so each out-proj starts immediately
                    otsl = cp.tile([128, 128], BF16, tag="ots", name=f"otsf{it}")
                    nc.scalar.copy(otsl, otl[:, it, :])
                    op_ps = stps.tile([128, 512], F32, tag="st", name=f"opf{it}")
                    nc.tensor.matmul(op_ps, otsl, wo_sb, start=True, stop=True)
                    opsl[it] = op_ps
                    if it >= 1:
                        nc.scalar.copy(out_sbl[:, it - 1, :], opsl.pop(it - 1))
                        nc.sync.dma_start(out=out_r[ibl, :, it - 1, :],
                                          in_=out_sbl[:, it - 1, :])
                nc.scalar.copy(out_sbl[:, 3, :], opsl.pop(3))
                nc.sync.dma_start(out=out_r[ibl, :, 3, :], in_=out_sbl[:, 3, :])

    fix_waits_nc(nc, mybir)
    return nc


def fix_waits_nc(nc, mybir):
    """Post-pass over the scheduled program: (1) remove semaphore waits that
    are transitively implied by earlier waits (Tile emits per-proc-minimal,
    not transitively-minimal, waits), (2) split any instruction still
    carrying more than one wait by injecting single-wait NoOps in front of
    it — this walrus build rejects >1 sync wait per instruction.
    Mutates nc in place so CoreSim and hardware run identical sync."""
    nop_id = [0]

    def _is_ge(w):
        return w.sync_type == "semaphore" and w.wait_mode == "sem-ge-imm"

    for fn in nc.m.functions:
        for blk in fn.blocks:
            insts = list(blk.instructions)
            n = len(insts)

            producers = {}
            cum = {}
            nonmono = set()  # sems ever decremented: counter logic invalid
            for idx, inst in enumerate(insts):
                si = inst.sync_info
                for u in (si.on_update if si else []) or []:
                    if u.sync_type != "semaphore":
                        continue
                    sid = u.id
                    if u.update_mode != "sem-inc":
                        nonmono.add(sid)
                        continue
                    cum[sid] = cum.get(sid, 0) + int(u.update_value)
                    producers.setdefault(sid, []).append((cum[sid], idx))

            def producer_of(sid, val):
                for cv, idx in producers.get(sid, ()):
                    if cv >= val:
                        return idx
                return None

            prev_eng = [None] * n
            last = {}
            for idx, inst in enumerate(insts):
                e = inst.engine
                prev_eng[idx] = last.get(e)
                last[e] = idx

            def get_waits(inst):
                si = inst.sync_info
                return list(si.on_wait) if si and si.on_wait else []

            def is_ge(w):
                return _is_ge(w) and w.id not in nonmono

            know = [dict() for _ in range(n)]
            for _ in range(3):
                changed = False
                for idx, inst in enumerate(insts):
                    k = dict(know[prev_eng[idx]]) if prev_eng[idx] is not None else {}
                    for w in get_waits(inst):
                        if not is_ge(w):
                            continue
                        sid, val = w.id, int(w.wait_value)
                        if k.get(sid, -1) < val:
                            k[sid] = val
                        p = producer_of(sid, val)
                        if p is not None:
                            for s2, v2 in know[p].items():
                                if k.get(s2, -1) < v2:
                                    k[s2] = v2
                    if k != know[idx]:
                        know[idx] = k
                        changed = True
                if not changed:
                    break

            new_insts = []
            dirty = False
            for idx, inst in enumerate(insts):
                si = inst.sync_info
                waits = get_waits(inst)
                if si is not None and waits:
                    base = dict(know[prev_eng[idx]]) if prev_eng[idx] is not None else {}
                    kept = []
                    for w in waits:
                        if is_ge(w):
                            sid, val = w.id, int(w.wait_value)
                            if base.get(sid, -1) >= val:
                                continue
                            base[sid] = val
                            p = producer_of(sid, val)
                            if p is not None:
                                for s2, v2 in know[p].items():
                                    if base.get(s2, -1) < v2:
                                        base[s2] = v2
                        kept.append(w)
                    if len(kept) != len(waits) or len(kept) > 1:
                        dirty = True
                        for w in kept[:-1]:
                            nop_id[0] += 1
                            nop = mybir.InstNoOp(
                                name=f"I-waitfix-{nop_id[0]}", ins=[], outs=[])
                            nop.engine = inst.engine
                            nop.sync_info = mybir.SyncInfo(on_wait=[w], on_update=[])
                            nc.register_instruction(nop)
                            new_insts.append(nop)
                        inst.sync_info = mybir.SyncInfo(
                            on_wait=kept[-1:],
                            on_update=list(si.on_update or []))
                new_insts.append(inst)
            if dirty:
                blk.instructions = new_insts


def get_program():
    if "nc" not in _prog_cache:
        _prog_cache["nc"] = build_program()
    return _prog_cache["nc"]


def _prep_inputs(tokens, norm_weight, w_qkv, w_out):
    import ml_dtypes
    tokens = np.ascontiguousarray(np.asarray(tokens, dtype=np.float32))
    norm_weight = np.asarray(norm_weight, dtype=np.float32)
    w_qkv = np.asarray(w_qkv, dtype=np.float32)
    w_out = np.asarray(w_out, dtype=np.float32)

    wp = w_qkv * norm_weight[:, None]  # fold RMSNorm weight into qkv weights

    in_maps = []
    for c in range(NCORES):
        b = c // 4
        h0 = 2 * (c % 4)
        m = {}
        m["tok"] = tokens[b]
        for name, off in (("wq", 0), ("wk", DI), ("wv", 2 * DI)):
            w = wp[:, off + h0 * DH: off + (h0 + 2) * DH]       # [512, 128]
            m[name] = np.ascontiguousarray(
                w.reshape(4, 128, 128).transpose(1, 0, 2))       # [128, 4, 128]
        rows = w_out[h0 * DH:(h0 + 2) * DH, :]                   # [128, 512]
        m["wo"] = np.ascontiguousarray(rows.astype(ml_dtypes.bfloat16))
        in_maps.append(m)
    return in_maps


def run(tokens, norm_weight, w_qkv, w_out, trace=False):
    from concourse.bass_utils import run_bass_kernel_spmd
    nc = get_program()
    in_maps = _prep_inputs(tokens, norm_weight, w_qkv, w_out)
    res = run_bass_kernel_spmd(nc, in_maps, core_ids=list(range(NCORES)), trace=trace)
    parts = [res.results[c]["out_part"] for c in range(NCORES)]
    out = np.empty((B, N, D), dtype=np.float32)
    for b in range(B):
        out[b] = parts[4 * b] + parts[4 * b + 1] + parts[4 * b + 2] + parts[4 * b + 3]
    return out, res


def kernel(tokens, norm_weight, w_qkv, w_out):
    out, _ = run(tokens, norm_weight, w_qkv, w_out, trace=False)
    return out
